# revision 3
# baseline (speedup 1.0000x reference)
"""Trainium2 Bass kernel for nn_DeepCPP (GAT + 2xGCN graph branch, conv1d seq
branch, fusion MLP), SPMD over 8 NeuronCores.

Sharding/strategy:
 - Nodes partitioned across cores in natural order (keeps sorted `batch`
   contiguous per core); within a core nodes are sorted by in-degree so
   128-node windows have near-uniform max degree (node-major slot grids),
   processed by segmented hardware loops.
 - GAT layer is gather-free: x[src] per edge slot is materialized host-side,
   attention logits computed on-device per slot-column via small matmuls, and
   exp(leakyrelu(a_s+a_d)) is factorized as max(P_e*T_d, R_e) with
   P=exp(a_s), R=exp(0.2*a_s), T=exp(0.8*a_d); the per-dst factor
   exp(-0.2*a_d) cancels in the softmax.
 - GCN layers gather 256B rows (dinv-prescaled h) from an AllGathered table
   via indirect DMA; aggregation is a strided vector reduction.
 - Mean-pool via one-hot selection matmuls into persistent PSUM, AllReduce of
   partials; seq branch + fusion MLP run replicated feature-major.
"""

import os
import sys

sys.path.insert(0, '/opt/trn_rl_repo')

import numpy as np
import ml_dtypes

import concourse.bass as bass
import concourse.mybir as mybir
import concourse.tile as tile
from concourse import bacc
from concourse.bass_utils import run_bass_kernel_spmd

F32 = mybir.dt.float32
BF16 = mybir.dt.bfloat16
I32 = mybir.dt.int32
AF = mybir.ActivationFunctionType
OP = mybir.AluOpType
AX = mybir.AxisListType

NC_CORES = 8
P = 128
DEBUG = False


# --------------------------------------------------------------------------
# host-side prep
# --------------------------------------------------------------------------

def _segments(Ts, max_segs=6):
    W = len(Ts)
    INF = float('inf')
    best = [[INF] * (max_segs + 1) for _ in range(W + 1)]
    arg = [[None] * (max_segs + 1) for _ in range(W + 1)]
    best[0][0] = 0.0
    for j in range(1, W + 1):
        for s in range(1, max_segs + 1):
            for i in range(j):
                if best[i][s - 1] == INF:
                    continue
                c = best[i][s - 1] + (j - i) * Ts[i]
                if c < best[j][s]:
                    best[j][s] = c
                    arg[j][s] = i
    s = min(range(1, max_segs + 1), key=lambda k: best[W][k])
    bounds = []
    j = W
    while j > 0:
        i = arg[j][s]
        bounds.append((i, j))
        j = i
        s -= 1
    bounds.reverse()
    return bounds


def host_prep(inputs):
    x = np.asarray(inputs['x'], np.float32)
    ei = np.asarray(inputs['edge_index'], np.int64)
    batch = np.asarray(inputs['batch'], np.int64)
    N = x.shape[0]
    Bsz = int(np.asarray(inputs['seq_data']).shape[0])
    assert N % NC_CORES == 0
    REAL = N // NC_CORES
    WPC = (REAL + P - 1) // P
    LOCAL = WPC * P
    NTOT = LOCAL * NC_CORES
    SENT = REAL if REAL < LOCAL else REAL - 1   # sentinel zero row in core 0

    src2 = np.concatenate([ei[0], np.arange(N)])
    dst2 = np.concatenate([ei[1], np.arange(N)])
    deg = np.bincount(dst2, minlength=N)

    local_rank = np.zeros(N, np.int64)
    rowid = np.zeros(N, np.int64)
    node_at = np.full((NC_CORES, LOCAL), -1, np.int64)
    for c in range(NC_CORES):
        ns = np.arange(c * REAL, (c + 1) * REAL)
        order = ns[np.argsort(-deg[ns], kind='stable')]
        local_rank[order] = np.arange(REAL)
        rowid[order] = c * LOCAL + np.arange(REAL)
        node_at[c, :REAL] = order

    Tw = np.ones(WPC, np.int64)
    for c in range(NC_CORES):
        first = node_at[c, ::P]
        for w in range(WPC):
            if first[w] >= 0:
                Tw[w] = max(Tw[w], deg[first[w]])
    segs = _segments([int(t) for t in Tw])
    seg_T = [int(Tw[w0]) for (w0, w1) in segs]
    col_off = np.zeros(WPC, np.int64)
    TW = np.zeros(WPC, np.int64)
    off = 0
    for (w0, w1), T in zip(segs, seg_T):
        for w in range(w0, w1):
            col_off[w] = off + (w - w0) * T
            TW[w] = T
        off += (w1 - w0) * T
    SLOTS = int(off)

    e_dst = rowid[dst2]
    e_src = src2
    o = np.argsort(e_dst, kind='stable')
    e_dst = e_dst[o]
    e_src = e_src[o]
    grp_start = np.searchsorted(e_dst, np.arange(NTOT), side='left')
    t_of = np.arange(len(e_dst)) - grp_start[e_dst]
    c_of = e_dst // LOCAL
    lrow = e_dst % LOCAL
    w_of = lrow // P
    p_of = lrow % P
    col = col_off[w_of] + t_of
    assert (t_of < TW[w_of]).all()

    slot_node = np.full((NC_CORES, P, SLOTS), N, np.int64)
    slot_node[c_of, p_of, col] = e_src

    x_pad = np.vstack([x, np.zeros((1, x.shape[1]), np.float32)])
    rowid_pad = np.concatenate([rowid, [SENT]]).astype(np.int32)

    cnt = np.bincount(batch, minlength=Bsz).astype(np.float32)
    per_core = []
    for c in range(NC_CORES):
        sn = slot_node[c]                          # [P, SLOTS], N = pad
        xs = x_pad[sn]                             # [P, SLOTS, 9]
        xslots = np.ascontiguousarray(xs.reshape(P, SLOTS * 9))
        xTl = np.zeros((16, SLOTS, P), np.float32)
        xTl[0:9] = xs.transpose(2, 1, 0)
        xTl[9] = (sn.T == N).astype(np.float32)    # pad flag
        xslotsT = np.ascontiguousarray(xTl.reshape(16, SLOTS * P))
        srcrow = rowid_pad[sn]

        valid = node_at[c] >= 0
        xloc = np.zeros((16, LOCAL), np.float32)
        xloc[0:9, valid] = x[node_at[c][valid]].T

        dg = np.full(LOCAL, 1e30, np.float32)
        dg[valid] = deg[node_at[c][valid]]
        deg_w = np.ascontiguousarray(dg.reshape(WPC, P).T)

        bl = np.full(LOCAL, -1.0, np.float32)
        b_base = int(batch[c * REAL])
        bl[valid] = batch[node_at[c][valid]] - b_base
        assert bl.max() < 256, "batch window exceeded 256"
        bl_w = np.ascontiguousarray(bl.reshape(WPC, P).T)

        cnt_l = np.ones(256, np.float32)
        hi = min(256, Bsz - b_base)
        cnt_l[:hi] = np.maximum(cnt[b_base:b_base + hi], 1.0)
        scatv = np.zeros(256, np.int32)
        for j in range(256):
            scatv[j] = b_base + j if b_base + j < Bsz else Bsz + (j % 8)

        per_core.append(dict(
            xslots=xslots, xslotsT=xslotsT, srcrow=srcrow.astype(np.int32),
            xlocT=xloc, deg_w=deg_w, bl_w=bl_w,
            cnt_l=np.ascontiguousarray(cnt_l.reshape(2, P).T),
            scat=np.ascontiguousarray(scatv.reshape(2, P).T),
        ))

    baked = dict(N=N, REAL=REAL, WPC=WPC, LOCAL=LOCAL, NTOT=NTOT,
                 SLOTS=SLOTS, segs=segs, seg_T=seg_T, Bsz=Bsz)
    return per_core, baked


def fold_weights(inputs):
    w = {k: np.asarray(v, np.float32) for k, v in inputs.items()
         if k not in ('x', 'edge_index', 'batch')}
    H, C = 4, 32
    Wg = w['W_gat']
    was = np.einsum('fhc,hc->fh', Wg.reshape(9, H, C), w['att_src'])
    wad = np.einsum('fhc,hc->fh', Wg.reshape(9, H, C), w['att_dst'])
    was_aug = np.zeros((16, 4), np.float32)
    was_aug[0:9] = was
    was_aug[9] = -80.0
    wad_aug = np.zeros((16, 4), np.float32)
    wad_aug[0:9] = wad
    wg_aug = np.zeros((128, 128), np.float32)
    for h in range(H):
        wg_aug[h * 32:h * 32 + 9, h * 32:(h + 1) * 32] = Wg[:, h * 32:(h + 1) * 32]
        wg_aug[h * 32 + 9, h * 32:(h + 1) * 32] = w['b_gat'][h * 32:(h + 1) * 32]
    W3_aug = np.zeros((65, 128), np.float32)
    W3_aug[0:64] = w['W3']
    W3_aug[64] = w['b3']

    def fold(cw, cb, g, be, m, v):
        s = g / np.sqrt(v + 1e-5)
        return cw * s[:, None, None], (cb - m) * s + be

    c1w, c1b = fold(w['conv1_w'], w['conv1_b'], w['bn1_g'], w['bn1_b'],
                    w['bn1_m'], w['bn1_v'])
    c2w, c2b = fold(w['conv2_w'], w['conv2_b'], w['bn2_g'], w['bn2_b'],
                    w['bn2_m'], w['bn2_v'])
    # [cin, k, cout] flattened so slice k -> [cin, cout]
    w1k = np.ascontiguousarray(c1w.transpose(1, 2, 0)).reshape(30, 3 * 64)
    w2k = np.ascontiguousarray(c2w.transpose(1, 2, 0)).reshape(64, 3 * 64)
    fc1_Wr = np.ascontiguousarray(w['fc1_W'].reshape(64, 16 * 64))

    seq = w['seq_data']                              # [B, 30, 20]
    xseq = np.ascontiguousarray(seq.transpose(1, 0, 2)).reshape(30, -1)

    return dict(
        was_aug=was_aug, wad_aug=wad_aug, wg_aug=wg_aug,
        W2=w['W2'], b2row=np.ascontiguousarray(np.broadcast_to(w['b2'], (P, 64))),
        W3_aug=W3_aug,
        w1k=w1k, b1=np.ascontiguousarray(c1b.reshape(64, 1)),
        w2k=w2k, b2c=np.ascontiguousarray(c2b.reshape(64, 1)),
        fc1_Wr=fc1_Wr, fc1_b=np.ascontiguousarray(w['fc1_b'].reshape(64, 1)),
        fus_W0=np.ascontiguousarray(w['fus_W'][0:128]),
        fus_W1=np.ascontiguousarray(w['fus_W'][128:192]),
        fus_b=np.ascontiguousarray(w['fus_b'].reshape(1, 128)),
        cls1_W=w['cls1_W'],
        cls1_b=np.ascontiguousarray(w['cls1_b'].reshape(1, 64)),
        cls3_W=w['cls3_W'],
        cls3_b_t=np.array([[float(w['cls3_b'][0])]], np.float32),
        xseq=xseq,
    )


# --------------------------------------------------------------------------
# device program
# --------------------------------------------------------------------------

def build_nc(baked):
    WPC, LOCAL, NTOT, SLOTS = (baked['WPC'], baked['LOCAL'], baked['NTOT'],
                               baked['SLOTS'])
    segs, seg_T = baked['segs'], baked['seg_T']
    Bsz = baked['Bsz']
    BROWS = Bsz + 8
    REALC = baked['REAL']
    RG = [list(range(NC_CORES))]

    nc = bacc.Bacc("TRN2", target_bir_lowering=False, debug=False,
                   num_devices=NC_CORES)

    def inp(name, shape, dt=F32):
        return nc.dram_tensor(name, shape, dt, kind="ExternalInput")

    xslots = inp("xslots", [P, SLOTS * 9])
    xslotsT = inp("xslotsT", [16, SLOTS * P])
    srcrow = inp("srcrow", [P, SLOTS], I32)
    xlocT = inp("xlocT", [16, LOCAL])
    deg_w = inp("deg_w", [P, WPC])
    bl_w = inp("bl_w", [P, WPC])
    cnt_l = inp("cnt_l", [P, 2])
    scat = inp("scat", [P, 2], I32)
    iota256 = inp("iota256", [P, 256])
    ident = inp("ident", [P, P])
    ones4 = inp("ones4", [P, 4])
    onesrow = inp("onesrow", [1, Bsz])
    was_aug = inp("was_aug", [16, 4])
    wad_aug = inp("wad_aug", [16, 4])
    wg_aug = inp("wg_aug", [128, 128])
    W2 = inp("W2", [128, 64])
    b2row = inp("b2row", [P, 64])
    W3_aug = inp("W3_aug", [65, 128])
    w1k = inp("w1k", [30, 3 * 64])
    b1 = inp("b1", [64, 1])
    w2k = inp("w2k", [64, 3 * 64])
    b2c = inp("b2c", [64, 1])
    fc1_Wr = inp("fc1_Wr", [64, 16 * 64])
    fc1_b = inp("fc1_b", [64, 1])
    fus_W0 = inp("fus_W0", [128, 128])
    fus_W1 = inp("fus_W1", [64, 128])
    fus_b = inp("fus_b", [1, 128])
    cls1_W = inp("cls1_W", [128, 64])
    cls1_b = inp("cls1_b", [1, 64])
    cls3_W = inp("cls3_W", [64, 1])
    cls3_b_t = inp("cls3_b_t", [1, 1])
    xseq = inp("xseq", [30, Bsz * 20])

    out = nc.dram_tensor("out", [1, Bsz], F32, kind="ExternalOutput")
    dbg_T2 = nc.dram_tensor("dbg_T2", [LOCAL, 64], F32, kind="ExternalOutput") if DEBUG else None
    dbg_T3 = nc.dram_tensor("dbg_T3", [LOCAL, 64], F32, kind="ExternalOutput") if DEBUG else None
    dbg_AR = nc.dram_tensor("dbg_AR", [BROWS, 128], F32, kind="ExternalOutput") if DEBUG else None
    dbg_sT = nc.dram_tensor("dbg_sT", [64, Bsz], F32, kind="ExternalOutput") if DEBUG else None

    T2_local = nc.dram_tensor("T2_local", [LOCAL, 64], F32)
    s1_dram = nc.dram_tensor("s1_dram", [64, Bsz * 18], F32)
    s2_dram = nc.dram_tensor("s2_dram", [64, Bsz * 16], F32)
    T2_full = nc.dram_tensor("T2_full", [NTOT, 64], F32)
    T3_local = nc.dram_tensor("T3_local", [LOCAL, 64], F32)
    T3_full = nc.dram_tensor("T3_full", [NTOT, 64], F32)
    AR_in = nc.dram_tensor("AR_in", [BROWS, 128], F32)
    AR_out = nc.dram_tensor("AR_out", [BROWS, 128], F32)

    with tile.TileContext(nc) as tc:
        with tc.tile_pool(name="const", bufs=1) as cp, \
             tc.tile_pool(name="work", bufs=2) as wp, \
             tc.tile_pool(name="gat", bufs=2) as gp, \
             tc.tile_pool(name="psum", bufs=4, space="PSUM") as pp, \
             tc.tile_pool(name="ppool", bufs=1, space="PSUM") as ppool, \
             tc.tile_pool(name="seq", bufs=1) as sq:

            def c_load(ap, shape, dt=F32):
                t = cp.tile(shape, dt, tag=f"c_{ap.name}")
                nc.sync.dma_start(t[:], ap[:])
                return t

            srcrow_sb = c_load(srcrow, [P, SLOTS], I32)
            deg_sb = c_load(deg_w, [P, WPC])
            bl_sb = c_load(bl_w, [P, WPC])
            cnt_sb = c_load(cnt_l, [P, 2])
            scat_sb = c_load(scat, [P, 2], I32)
            iota_sb = c_load(iota256, [P, 256])
            ident_sb = c_load(ident, [P, P])
            ones4_sb = c_load(ones4, [P, 4])
            was_sb = c_load(was_aug, [16, 4])
            wad_sb = c_load(wad_aug, [16, 4])
            wg_sb = c_load(wg_aug, [128, 128])
            W2_sb = c_load(W2, [128, 64])
            b2row_sb = c_load(b2row, [P, 64])
            W3_sb = c_load(W3_aug, [65, 128])

            dinv_sb = cp.tile([P, WPC], F32)
            nc.scalar.activation(dinv_sb[:], deg_sb[:], AF.Sqrt)
            nc.vector.reciprocal(dinv_sb[:], dinv_sb[:])

            # persistent pooling PSUM, zeroed via K=1 matmul (sets has_written)
            pool_ps0 = ppool.tile([P, P], F32, tag="pool0")
            pool_ps1 = ppool.tile([P, P], F32, tag="pool1")
            zrow = cp.tile([1, P], F32)
            nc.vector.memset(zrow[:], 0.0)
            nc.tensor.matmul(pool_ps0[:], zrow[:], zrow[:], start=True, stop=True)
            nc.tensor.matmul(pool_ps1[:], zrow[:], zrow[:], start=True, stop=True)

            # ================= GAT =================
            def gat_body(w, w0, T, seg_col0):
                colb = seg_col0 - w0 * T
                xw = gp.tile([16, P], F32, tag="xw")
                nc.sync.dma_start(xw[:], xlocT[:, bass.ds(w * P, P)])
                ad_ps = pp.tile([P, 4], F32, tag="ps")
                nc.tensor.matmul(ad_ps[:], xw[:], wad_sb[:],
                                 start=True, stop=True)
                T_w = gp.tile([P, 4], F32, tag="Tw")
                nc.scalar.activation(T_w[:], ad_ps[:], AF.Exp, scale=0.8)

                XT = gp.tile([16, T * P], F32, tag="XT")
                nc.sync.dma_start(XT[:],
                                  xslotsT[:, bass.ds((colb + w * T) * P, T * P)])
                as_ps = pp.tile([P, 4 * T], F32, tag="ps")
                for t in range(T):
                    nc.tensor.matmul(as_ps[:, 4 * t:4 * t + 4],
                                     XT[:, t * P:(t + 1) * P], was_sb[:],
                                     start=True, stop=True)
                Pt = gp.tile([P, 4 * T], F32, tag="Pt")
                Rt = gp.tile([P, 4 * T], F32, tag="Rt")
                nc.scalar.activation(Pt[:], as_ps[:], AF.Exp, scale=1.0)
                nc.scalar.activation(Rt[:], as_ps[:], AF.Exp, scale=0.2)

                EX = gp.tile([P, 4 * T], F32, tag="EX")
                nc.vector.tensor_tensor(
                    EX[:].rearrange("p (t h) -> p t h", h=4),
                    Pt[:].rearrange("p (t h) -> p t h", h=4),
                    T_w[:, None, :].to_broadcast([P, T, 4]),
                    op=OP.mult)
                nc.vector.tensor_tensor(EX[:], EX[:], Rt[:], op=OP.max)
                S4 = gp.tile([P, 4], F32, tag="S4")
                nc.vector.tensor_reduce(
                    S4[:, :, None],
                    EX[:].rearrange("p (t h) -> p h t", h=4),
                    axis=AX.X, op=OP.add)
                nc.vector.reciprocal(S4[:], S4[:])
                AL = gp.tile([P, 4 * T], F32, tag="AL")
                nc.vector.tensor_tensor(
                    AL[:].rearrange("p (t h) -> p t h", h=4),
                    EX[:].rearrange("p (t h) -> p t h", h=4),
                    S4[:, None, :].to_broadcast([P, T, 4]),
                    op=OP.mult)

                XS = gp.tile([P, T * 9], F32, tag="XS")
                nc.sync.dma_start(XS[:],
                                  xslots[:, bass.ds((colb + w * T) * 9, T * 9)])
                ZR = gp.tile([P, T * 36], F32, tag="ZR")
                nc.vector.tensor_tensor(
                    ZR[:].rearrange("p (t h f) -> p t h f", h=4, f=9),
                    XS[:].rearrange("p (t f) -> p t f", f=9)[:, :, None, :]
                        .to_broadcast([P, T, 4, 9]),
                    AL[:].rearrange("p (t h) -> p t h", h=4)[:, :, :, None]
                        .to_broadcast([P, T, 4, 9]),
                    op=OP.mult)
                zaug = gp.tile([P, 128], F32, tag="zaug")
                nc.vector.memset(
                    zaug[:].rearrange("p (h t) -> p h t", t=32)[:, :, 10:32], 0.0)
                nc.vector.tensor_copy(
                    zaug[:].rearrange("p (h t) -> p h t", t=32)[:, :, 9:10],
                    ones4_sb[:, :, None])
                nc.vector.tensor_reduce(
                    zaug[:].rearrange("p (h t) -> p h t", t=32)[:, :, 0:9][:, :, :, None],
                    ZR[:].rearrange("p (t h f) -> p h f t", h=4, f=9),
                    axis=AX.X, op=OP.add)
                zT_ps = pp.tile([P, P], F32, tag="ps")
                nc.tensor.transpose(out=zT_ps[:], in_=zaug[:], identity=ident_sb[:])
                zT = gp.tile([P, P], F32, tag="zT")
                nc.scalar.copy(zT[:], zT_ps[:])
                g1_ps = pp.tile([P, P], F32, tag="ps")
                nc.tensor.matmul(g1_ps[:], wg_sb[:], zT[:],
                                 start=True, stop=True)
                g1T = gp.tile([P, P], F32, tag="g1T")
                nc.scalar.activation(g1T[:], g1_ps[:], AF.Lrelu, alpha=0.01)
                h2_ps = pp.tile([P, 64], F32, tag="ps")
                nc.tensor.matmul(h2_ps[:], g1T[:], W2_sb[:], start=True, stop=True)
                T2s = gp.tile([P, 64], F32, tag="T2s")
                nc.scalar.activation(T2s[:], h2_ps[:], AF.Copy,
                                     scale=dinv_sb[:, bass.ds(w, 1)])
                nc.sync.dma_start(T2_local[bass.ds(w * P, P), :], T2s[:])

            seg_col0 = 0
            for (w0, w1), T in zip(segs, seg_T):
                with tc.For_i(w0, w1, 1) as w:
                    gat_body(w, w0, T, seg_col0)
                seg_col0 += (w1 - w0) * T

            if LOCAL > REALC:
                ztail = wp.tile([LOCAL - REALC, 64], F32, tag="ztail")
                nc.vector.memset(ztail[:], 0.0)
                nc.sync.dma_start(T2_local[REALC:LOCAL, :], ztail[:])

            tc.strict_bb_all_engine_barrier()
            nc.gpsimd.collective_compute(
                "AllGather", OP.bypass, replica_groups=RG,
                ins=[T2_local.ap().opt()], outs=[T2_full.ap().opt()])
            tc.strict_bb_all_engine_barrier()

            # ================= GCN layers =================
            def gcn_body(w, w0, T, seg_col0, table, last):
                colb = seg_col0 - w0 * T
                IDXw = wp.tile([P, T], I32, tag="IDXw")
                nc.vector.tensor_copy(IDXw[:],
                                      srcrow_sb[:, bass.ds(colb + w * T, T)])
                G = wp.tile([P, T * 64], F32, tag="G")
                for t in range(T):
                    nc.gpsimd.indirect_dma_start(
                        out=G[:, 64 * t:64 * (t + 1)], out_offset=None,
                        in_=table[:],
                        in_offset=bass.IndirectOffsetOnAxis(
                            ap=IDXw[:, t:t + 1], axis=0))
                z = wp.tile([P, 64], F32, tag="z")
                nc.vector.tensor_reduce(
                    z[:, :, None],
                    G[:].rearrange("p (t c) -> p c t", c=64),
                    axis=AX.X, op=OP.add)
                if not last:
                    nc.vector.tensor_scalar(
                        z[:], z[:], dinv_sb[:, bass.ds(w, 1)], None, OP.mult)
                    nc.vector.tensor_tensor(z[:], z[:], b2row_sb[:], op=OP.add)
                    g2 = wp.tile([P, 64], F32, tag="g2")
                    nc.scalar.activation(g2[:], z[:], AF.Lrelu, alpha=0.01)
                    T3s = wp.tile([P, 64], F32, tag="T3s")
                    nc.scalar.activation(T3s[:], g2[:], AF.Copy,
                                         scale=dinv_sb[:, bass.ds(w, 1)])
                    nc.sync.dma_start(T3_local[bass.ds(w * P, P), :], T3s[:])
                else:
                    z3s = wp.tile([P, 65], F32, tag="z3s")
                    nc.scalar.activation(z3s[:, 0:64], z[:], AF.Copy,
                                         scale=dinv_sb[:, bass.ds(w, 1)])
                    nc.vector.tensor_copy(z3s[:, 64:65], ones4_sb[:, 0:1])
                    z3T_ps = pp.tile([65, P], F32, tag="ps")
                    nc.tensor.transpose(out=z3T_ps[:], in_=z3s[:],
                                        identity=ident_sb[:])
                    z3T = wp.tile([65, P], F32, tag="z3T")
                    nc.scalar.copy(z3T[:], z3T_ps[:])
                    g3_ps = pp.tile([P, P], F32, tag="ps")
                    nc.tensor.matmul(g3_ps[:], z3T[:], W3_sb[:],
                                     start=True, stop=True)
                    g3 = wp.tile([P, P], F32, tag="g3")
                    nc.scalar.activation(g3[:], g3_ps[:], AF.Lrelu, alpha=0.01)
                    Mp = wp.tile([P, 256], F32, tag="Mp")
                    nc.vector.tensor_scalar(
                        Mp[:], iota_sb[:], bl_sb[:, bass.ds(w, 1)], None,
                        OP.is_equal)
                    nc.tensor.matmul(pool_ps0[:], Mp[:, 0:128], g3[:],
                                     start=False, stop=True)
                    nc.tensor.matmul(pool_ps1[:], Mp[:, 128:256], g3[:],
                                     start=False, stop=True)

            seg_col0 = 0
            for (w0, w1), T in zip(segs, seg_T):
                with tc.For_i(w0, w1, 1) as w:
                    gcn_body(w, w0, T, seg_col0, T2_full, last=False)
                seg_col0 += (w1 - w0) * T

            if LOCAL > REALC:
                ztail2 = wp.tile([LOCAL - REALC, 64], F32, tag="ztail")
                nc.vector.memset(ztail2[:], 0.0)
                nc.sync.dma_start(T3_local[REALC:LOCAL, :], ztail2[:])

            tc.strict_bb_all_engine_barrier()
            nc.gpsimd.collective_compute(
                "AllGather", OP.bypass, replica_groups=RG,
                ins=[T3_local.ap().opt()], outs=[T3_full.ap().opt()])
            tc.strict_bb_all_engine_barrier()

            seg_col0 = 0
            for (w0, w1), T in zip(segs, seg_T):
                with tc.For_i(w0, w1, 1) as w:
                    gcn_body(w, w0, T, seg_col0, T3_full, last=True)
                seg_col0 += (w1 - w0) * T

            # ---- pool epilogue
            zb = wp.tile([P, 128], F32, tag="zb")
            nc.vector.memset(zb[:], 0.0)
            r0 = 0
            while r0 < BROWS:
                r1 = min(r0 + P, BROWS)
                nc.sync.dma_start(AR_in[r0:r1, :], zb[:r1 - r0, :])
                r0 = r1
            crec = wp.tile([P, 2], F32, tag="crec")
            nc.vector.reciprocal(crec[:], cnt_sb[:])
            for k, pps in enumerate((pool_ps0, pool_ps1)):
                pooled = wp.tile([P, 128], F32, tag="pooled")
                nc.scalar.activation(pooled[:], pps[:], AF.Copy,
                                     scale=crec[:, k:k + 1])
                nc.gpsimd.indirect_dma_start(
                    out=AR_in[:], out_offset=bass.IndirectOffsetOnAxis(
                        ap=scat_sb[:, k:k + 1], axis=0),
                    in_=pooled[:], in_offset=None)

            tc.strict_bb_all_engine_barrier()
            nc.gpsimd.collective_compute(
                "AllReduce", OP.add, replica_groups=RG,
                ins=[AR_in.ap().opt()], outs=[AR_out.ap().opt()])
            tc.strict_bb_all_engine_barrier()

            if DEBUG:
                dtile = sq.tile([P, 64], F32, tag="dtile")
                for i in range(LOCAL // P):
                    nc.sync.dma_start(dtile[:], T2_local[i * P:(i + 1) * P, :])
                    nc.sync.dma_start(dbg_T2[i * P:(i + 1) * P, :], dtile[:])
                    nc.sync.dma_start(dtile[:], T3_local[i * P:(i + 1) * P, :])
                    nc.sync.dma_start(dbg_T3[i * P:(i + 1) * P, :], dtile[:])
                dtile2 = sq.tile([P, 128], F32, tag="dtile2")
                r0 = 0
                while r0 < BROWS:
                    r1 = min(r0 + P, BROWS)
                    nc.sync.dma_start(dtile2[:r1 - r0, :], AR_out[r0:r1, :])
                    nc.sync.dma_start(dbg_AR[r0:r1, :], dtile2[:r1 - r0, :])
                    r0 = r1
            poolT = sq.tile([P, Bsz], F32, tag="poolT")
            for i in range(Bsz // P):
                blk = sq.tile([P, P], F32, tag="blk")
                nc.sync.dma_start(blk[:], AR_out[i * P:(i + 1) * P, :])
                tp = pp.tile([P, P], F32, tag="ps")
                nc.tensor.transpose(out=tp[:], in_=blk[:], identity=ident_sb[:])
                nc.scalar.copy(poolT[:, i * P:(i + 1) * P], tp[:])

            # ---- seq branch
            w1_sb = c_load(w1k, [30, 3 * 64])
            b1_sb = c_load(b1, [64, 1])
            w2_sb = c_load(w2k, [64, 3 * 64])
            b2c_sb = c_load(b2c, [64, 1])
            fc1_sb = c_load(fc1_Wr, [64, 16 * 64])
            fc1b_sb = c_load(fc1_b, [64, 1])
            fusW0_sb = c_load(fus_W0, [128, 128])
            fusW1_sb = c_load(fus_W1, [64, 128])
            fusb_sb = c_load(fus_b, [1, 128])
            cls1W_sb = c_load(cls1_W, [128, 64])
            cls1b_sb = c_load(cls1_b, [1, 64])
            cls3W_sb = c_load(cls3_W, [64, 1])
            cls3b_sb = c_load(cls3_b_t, [1, 1])
            onesr_sb = c_load(onesrow, [1, Bsz])

            CH1 = 28
            nb1 = (Bsz + CH1 - 1) // CH1
            for ci in range(nb1):
                b0 = ci * CH1
                bn = min(CH1, Bsz - b0)
                xs_ch = sq.tile([30, CH1 * 20], F32, tag="xs_ch")
                nc.sync.dma_start(xs_ch[:30, :bn * 20],
                                  xseq[:, b0 * 20:(b0 + bn) * 20])
                cps = pp.tile([64, CH1 * 18], F32, tag="ps")
                for k in range(3):
                    nc.tensor.matmul(
                        cps[:, :bn * 18],
                        w1_sb[:, 64 * k:64 * (k + 1)],
                        xs_ch[:].rearrange("c (b t) -> c b t", t=20)[:, 0:bn, k:k + 18],
                        start=(k == 0), stop=(k == 2))
                s1c = sq.tile([64, CH1 * 18], F32, tag="s1c")
                nc.scalar.activation(
                    s1c[:, :bn * 18], cps[:, :bn * 18],
                    AF.Lrelu, bias=b1_sb[:], alpha=0.01)
                nc.sync.dma_start(s1_dram[:, b0 * 18:(b0 + bn) * 18],
                                  s1c[:, :bn * 18])
            CH2 = 31
            nb2 = (Bsz + CH2 - 1) // CH2
            for ci in range(nb2):
                b0 = ci * CH2
                bn = min(CH2, Bsz - b0)
                s1c2 = sq.tile([64, CH2 * 18], F32, tag="s1c2")
                nc.sync.dma_start(s1c2[:, :bn * 18],
                                  s1_dram[:, b0 * 18:(b0 + bn) * 18])
                cps2 = pp.tile([64, CH2 * 16], F32, tag="ps")
                for k in range(3):
                    nc.tensor.matmul(
                        cps2[:, :bn * 16],
                        w2_sb[:, 64 * k:64 * (k + 1)],
                        s1c2[:].rearrange("c (b t) -> c b t", t=18)[:, 0:bn, k:k + 16],
                        start=(k == 0), stop=(k == 2))
                s2c = sq.tile([64, CH2 * 16], F32, tag="s2c")
                nc.scalar.activation(
                    s2c[:, :bn * 16], cps2[:, :bn * 16],
                    AF.Lrelu, bias=b2c_sb[:], alpha=0.01)
                nc.sync.dma_start(s2_dram[:, b0 * 16:(b0 + bn) * 16],
                                  s2c[:, :bn * 16])
            sT = sq.tile([64, Bsz], F32, tag="sT")
            for ci in range(Bsz // 512):
                b0 = ci * 512
                s2c3 = sq.tile([64, 512 * 16], F32, tag="s2c3")
                nc.sync.dma_start(s2c3[:], s2_dram[:, b0 * 16:(b0 + 512) * 16])
                fps = pp.tile([64, 512], F32, tag="ps")
                for t in range(16):
                    nc.tensor.matmul(
                        fps[:],
                        fc1_sb[:].rearrange("c (t j) -> c t j", j=64)[:, t, :],
                        s2c3[:].rearrange("c (b t) -> c b t", t=16)[:, :, t:t + 1],
                        start=(t == 0), stop=(t == 15))
                nc.scalar.activation(sT[:, b0:b0 + 512], fps[:],
                                     AF.Identity, bias=fc1b_sb[:])

            combT = sq.tile([P, Bsz], F32, tag="combT")
            for ci in range(Bsz // 512):
                b0 = ci * 512
                ups = pp.tile([P, 512], F32, tag="ps")
                nc.tensor.matmul(ups[:], fusW0_sb[:], poolT[:, b0:b0 + 512],
                                 start=True, stop=False)
                nc.tensor.matmul(ups[:], fusW1_sb[:], sT[:, b0:b0 + 512],
                                 start=False, stop=False)
                nc.tensor.matmul(ups[:], fusb_sb[:], onesr_sb[:, b0:b0 + 512],
                                 start=False, stop=True)
                nc.scalar.activation(combT[:, b0:b0 + 512], ups[:],
                                     AF.Lrelu, alpha=0.01)
            c1T = sq.tile([64, Bsz], F32, tag="c1T")
            for ci in range(Bsz // 512):
                b0 = ci * 512
                vps = pp.tile([64, 512], F32, tag="ps")
                nc.tensor.matmul(vps[:], cls1W_sb[:], combT[:, b0:b0 + 512],
                                 start=True, stop=False)
                nc.tensor.matmul(vps[:], cls1b_sb[:], onesr_sb[:, b0:b0 + 512],
                                 start=False, stop=True)
                nc.scalar.activation(c1T[:, b0:b0 + 512], vps[:],
                                     AF.Lrelu, alpha=0.01)
            out_sb = sq.tile([1, Bsz], F32, tag="out_sb")
            for ci in range(Bsz // 512):
                b0 = ci * 512
                ops_ = pp.tile([1, 512], F32, tag="ps")
                nc.tensor.matmul(ops_[:], cls3W_sb[:], c1T[:, b0:b0 + 512],
                                 start=True, stop=True)
                nc.vector.tensor_scalar(
                    out_sb[:, b0:b0 + 512], ops_[:], cls3b_sb[0:1, 0:1], None,
                    OP.add)
            if DEBUG:
                nc.sync.dma_start(dbg_sT[:], sT[:])
            nc.sync.dma_start(out[:], out_sb[:])

    nc.compile()
    return nc


# --------------------------------------------------------------------------
# entry point
# --------------------------------------------------------------------------

_CACHE = {}


def kernel(**inputs):
    key = (np.asarray(inputs['edge_index']).tobytes(),)
    kh = hash(key)
    if kh not in _CACHE:
        per_core, baked = host_prep(inputs)
        nc = build_nc(baked)
        _CACHE[kh] = (per_core, baked, nc)
    per_core, baked, nc = _CACHE[kh]

    wts = fold_weights(inputs)
    Bsz = baked['Bsz']
    shared = dict(
        iota256=np.ascontiguousarray(
            np.broadcast_to(np.arange(256, dtype=np.float32), (P, 256))),
        ident=np.eye(P, dtype=np.float32),
        ones4=np.ones((P, 4), np.float32),
        onesrow=np.ones((1, Bsz), np.float32),
        **wts)
    in_maps = []
    for c in range(NC_CORES):
        m = dict(shared)
        m.update(per_core[c])
        in_maps.append(m)

    global LAST_RESULT
    res = run_bass_kernel_spmd(
        nc, in_maps, core_ids=list(range(NC_CORES)),
        trace=bool(os.environ.get('BASS_KERNEL_TRACE')))
    LAST_RESULT = res
    o = res.results[0]["out"].reshape(Bsz, 1).astype(np.float32)
    return o


LAST_RESULT = None



# revision 11
# speedup vs baseline: 5.2378x; 5.2378x over previous
"""Trainium2 Bass kernel for nn_DeepCPP (GAT + 2xGCN graph branch, conv1d seq
branch, fusion MLP), SPMD over 8 NeuronCores.

Sharding/strategy:
 - Nodes partitioned across cores in natural order (keeps sorted `batch`
   contiguous per core); within a core nodes are sorted by in-degree so
   128-node windows have near-uniform max degree (node-major slot grids).
 - GAT attention logits per edge slot are computed with block-diagonal
   batched matmuls (8 slot-columns per matmul); exp(leakyrelu(a_s+a_d)) is
   factorized as max(P_e*T_d, R_e) with P=exp(a_s), R=exp(0.2*a_s),
   T=exp(0.8*a_d); the per-dst factor exp(-0.2*a_d) cancels in the softmax.
 - GCN layers gather 256B rows (dinv-prescaled h) from an AllGathered table
   with ONE batched indirect DMA per pair of 128-node windows; aggregation
   is a strided vector reduction.
 - Mean-pool via one-hot selection matmuls into persistent PSUM, AllReduce
   of partials; seq branch and fusion MLP are sharded by batch (128/core).
 - All loops fully unrolled (no hardware loops); non-Exp pointwise work runs
   on the Vector engine so the Scalar activation table stays loaded.
"""

import os
import sys

sys.path.insert(0, '/opt/trn_rl_repo')

import numpy as np

import concourse.bass as bass
import concourse.mybir as mybir
import concourse.tile as tile
from concourse import bacc
from concourse.bass_utils import run_bass_kernel_spmd

F32 = mybir.dt.float32
I32 = mybir.dt.int32
AF = mybir.ActivationFunctionType
OP = mybir.AluOpType
AX = mybir.AxisListType

NC_CORES = 8
P = 128


# --------------------------------------------------------------------------
# host-side prep (layout/indexing only; cached per (x, edge_index))
# --------------------------------------------------------------------------

def host_prep(inputs):
    x = np.asarray(inputs['x'], np.float32)
    ei = np.asarray(inputs['edge_index'], np.int64)
    batch = np.asarray(inputs['batch'], np.int64)
    N = x.shape[0]
    Bsz = int(np.asarray(inputs['seq_data']).shape[0])
    assert N % NC_CORES == 0
    REAL = N // NC_CORES
    WPC = (REAL + P - 1) // P
    LOCAL = WPC * P
    NTOT = LOCAL * NC_CORES
    SENT = REAL if REAL < LOCAL else REAL - 1   # sentinel zero row in core 0

    src2 = np.concatenate([ei[0], np.arange(N)])
    dst2 = np.concatenate([ei[1], np.arange(N)])
    deg = np.bincount(dst2, minlength=N)

    local_rank = np.zeros(N, np.int64)
    rowid = np.zeros(N, np.int64)
    node_at = np.full((NC_CORES, LOCAL), -1, np.int64)
    for c in range(NC_CORES):
        ns = np.arange(c * REAL, (c + 1) * REAL)
        order = ns[np.argsort(-deg[ns], kind='stable')]
        local_rank[order] = np.arange(REAL)
        rowid[order] = c * LOCAL + np.arange(REAL)
        node_at[c, :REAL] = order

    # per-window max degree (shared across cores so the program is SPMD)
    Tw = np.ones(WPC, np.int64)
    for c in range(NC_CORES):
        first = node_at[c, ::P]
        for w in range(WPC):
            if first[w] >= 0:
                Tw[w] = max(Tw[w], deg[first[w]])
    T8w = ((Tw + 7) // 8) * 8
    gcol = np.concatenate([[0], np.cumsum(Tw)])       # GCN grid col offsets
    acol = np.concatenate([[0], np.cumsum(T8w)])      # GAT grid col offsets
    SLOTS = int(gcol[-1])
    SLOTS8 = int(acol[-1])
    GTOT = SLOTS8 // 8
    assert T8w.max() * 4 <= 512, "as_ps would exceed one PSUM bank"

    e_dst = rowid[dst2]
    e_src = src2
    o = np.argsort(e_dst, kind='stable')
    e_dst = e_dst[o]
    e_src = e_src[o]
    grp_start = np.searchsorted(e_dst, np.arange(NTOT), side='left')
    t_of = np.arange(len(e_dst)) - grp_start[e_dst]
    c_of = e_dst // LOCAL
    lrow = e_dst % LOCAL
    w_of = lrow // P
    p_of = lrow % P
    assert (t_of < Tw[w_of]).all()
    col_g = gcol[w_of] + t_of
    col_a = acol[w_of] + t_of

    slot_node_g = np.full((NC_CORES, P, SLOTS), N, np.int64)
    slot_node_g[c_of, p_of, col_g] = e_src
    slot_node_a = np.full((NC_CORES, P, SLOTS8), N, np.int64)
    slot_node_a[c_of, p_of, col_a] = e_src

    x_pad = np.vstack([x, np.zeros((1, x.shape[1]), np.float32)])
    rowid_pad = np.concatenate([rowid, [SENT]]).astype(np.int32)

    cnt = np.bincount(batch, minlength=Bsz).astype(np.float32)
    per_core = []
    for c in range(NC_CORES):
        sna = slot_node_a[c]                       # [P, SLOTS8], N = pad
        xs = x_pad[sna]                            # [P, SLOTS8, 9]
        xslots = np.ascontiguousarray(xs.reshape(P, SLOTS8 * 9))
        xTl = np.zeros((16, SLOTS8, P), np.float32)
        xTl[0:9] = xs.transpose(2, 1, 0)
        xTl[9] = (sna.T == N).astype(np.float32)   # pad flag
        # [16j+f, (group)*128 + p] = xTl[f, 8*group+j, p]
        xgrp = np.ascontiguousarray(
            xTl.reshape(16, GTOT, 8, P).transpose(2, 0, 1, 3)
               .reshape(128, GTOT * P))
        srcrow = rowid_pad[slot_node_g[c]]         # [P, SLOTS]

        valid = node_at[c] >= 0
        xloc = np.zeros((9, LOCAL), np.float32)
        xloc[0:9, valid] = x[node_at[c][valid]].T

        dinv = np.zeros(LOCAL, np.float32)
        dinv[valid] = 1.0 / np.sqrt(deg[node_at[c][valid]])
        dinv_w = np.ascontiguousarray(dinv.reshape(WPC, P).T)

        bl = np.full(LOCAL, -1.0, np.float32)
        b_base = int(batch[c * REAL])
        bl[valid] = batch[node_at[c][valid]] - b_base
        assert bl.max() < 256, "batch window exceeded 256"
        bl_w = np.ascontiguousarray(bl.reshape(WPC, P).T)

        cnt_l = np.ones(256, np.float32)
        hi = min(256, Bsz - b_base)
        cnt_l[:hi] = np.maximum(cnt[b_base:b_base + hi], 1.0)
        scatv = np.zeros(256, np.int32)
        for j in range(256):
            scatv[j] = b_base + j if b_base + j < Bsz else Bsz + (j % 8)

        per_core.append(dict(
            xslots=xslots, xgrp=xgrp, srcrow=srcrow.astype(np.int32),
            xlocT=xloc, dinv_w=dinv_w, bl_w=bl_w,
            cnt_l=np.ascontiguousarray(cnt_l.reshape(2, P).T),
            scat=np.ascontiguousarray(scatv.reshape(2, P).T),
            rows128=(c * P + np.arange(P, dtype=np.int32)).reshape(P, 1),
        ))

    baked = dict(N=N, REAL=REAL, WPC=WPC, LOCAL=LOCAL, NTOT=NTOT,
                 SLOTS=SLOTS, SLOTS8=SLOTS8, GTOT=GTOT,
                 Tw=[int(t) for t in Tw], T8w=[int(t) for t in T8w],
                 gcol=[int(t) for t in gcol], acol=[int(t) for t in acol],
                 Bsz=Bsz)
    return per_core, baked


def fold_weights(inputs):
    w = {k: np.asarray(v, np.float32) for k, v in inputs.items()
         if k not in ('x', 'edge_index', 'batch')}
    H, C = 4, 32
    Wg = w['W_gat']
    was = np.einsum('fhc,hc->fh', Wg.reshape(9, H, C), w['att_src'])
    wad = np.einsum('fhc,hc->fh', Wg.reshape(9, H, C), w['att_dst'])
    was_aug = np.zeros((16, 4), np.float32)
    was_aug[0:9] = was
    was_aug[9] = -80.0
    wad_aug = np.zeros((9, 4), np.float32)
    wad_aug[0:9] = wad
    # block-diagonal was for 8 slot-columns per matmul
    wasD = np.zeros((128, 32), np.float32)
    for j in range(8):
        wasD[16 * j:16 * j + 16, 4 * j:4 * j + 4] = was_aug
    # [40,128] compact GAT weight: rows (10h+f) f<9 = W_gat, f=9 = bias
    wg40 = np.zeros((40, 128), np.float32)
    for h in range(H):
        wg40[h * 10:h * 10 + 9, h * 32:(h + 1) * 32] = Wg[:, h * 32:(h + 1) * 32]
        wg40[h * 10 + 9, h * 32:(h + 1) * 32] = w['b_gat'][h * 32:(h + 1) * 32]
    W3_aug = np.zeros((65, 128), np.float32)
    W3_aug[0:64] = w['W3']
    W3_aug[64] = w['b3']

    def fold(cw, cb, g, be, m, v):
        s = g / np.sqrt(v + 1e-5)
        return cw * s[:, None, None], (cb - m) * s + be

    c1w, c1b = fold(w['conv1_w'], w['conv1_b'], w['bn1_g'], w['bn1_b'],
                    w['bn1_m'], w['bn1_v'])
    c2w, c2b = fold(w['conv2_w'], w['conv2_b'], w['bn2_g'], w['bn2_b'],
                    w['bn2_m'], w['bn2_v'])
    # [cin, k, cout] flattened so slice k -> [cin, cout]
    w1k = np.ascontiguousarray(c1w.transpose(1, 2, 0)).reshape(30, 3 * 64)
    w2k = np.ascontiguousarray(c2w.transpose(1, 2, 0)).reshape(64, 3 * 64)
    fc1_Wr = np.ascontiguousarray(w['fc1_W'].reshape(64, 16 * 64))

    return dict(
        wasD=wasD, wad_aug=wad_aug, wg40=wg40,
        W2=w['W2'], b2row=np.ascontiguousarray(np.broadcast_to(w['b2'], (P, 64))),
        W3_aug=W3_aug,
        w1k=w1k, b1=np.ascontiguousarray(c1b.reshape(64, 1)),
        w2k=w2k, b2c=np.ascontiguousarray(c2b.reshape(64, 1)),
        fc1_Wr=fc1_Wr, fc1_b=np.ascontiguousarray(w['fc1_b'].reshape(64, 1)),
        fus_W0=np.ascontiguousarray(w['fus_W'][0:128]),
        fus_W1=np.ascontiguousarray(w['fus_W'][128:192]),
        fus_b=np.ascontiguousarray(w['fus_b'].reshape(1, 128)),
        cls1_W=w['cls1_W'],
        cls1_b=np.ascontiguousarray(w['cls1_b'].reshape(1, 64)),
        cls3_W=w['cls3_W'],
        cls3_b_t=np.array([[float(w['cls3_b'][0])]], np.float32),
    )


# --------------------------------------------------------------------------
# device program
# --------------------------------------------------------------------------

def build_nc(baked):
    WPC, LOCAL, NTOT = baked['WPC'], baked['LOCAL'], baked['NTOT']
    SLOTS, SLOTS8, GTOT = baked['SLOTS'], baked['SLOTS8'], baked['GTOT']
    Tw, T8w, gcol, acol = baked['Tw'], baked['T8w'], baked['gcol'], baked['acol']
    Bsz = baked['Bsz']
    BROWS = Bsz + 8
    BPC = Bsz // NC_CORES                      # batches per core (fusion/seq)
    RG = [list(range(NC_CORES))]
    T8MAX = max(T8w)
    GMAX = T8MAX // 8
    # adaptive window groups for batched GCN gathers (cap on slot columns)
    GCAP = max(56, max(Tw))
    GROUPS = []
    cur, curT = [], 0
    for w in range(WPC):
        if cur and curT + Tw[w] > GCAP:
            GROUPS.append(cur)
            cur, curT = [], 0
        cur.append(w)
        curT += Tw[w]
    if cur:
        GROUPS.append(cur)

    nc = bacc.Bacc("TRN2", target_bir_lowering=False, debug=False,
                   num_devices=NC_CORES)

    def inp(name, shape, dt=F32):
        return nc.dram_tensor(name, shape, dt, kind="ExternalInput")

    xgrp = inp("xgrp", [128, GTOT * P])
    xslots = inp("xslots", [P, SLOTS8 * 9])
    srcrow = inp("srcrow", [P, SLOTS], I32)
    xlocT = inp("xlocT", [9, LOCAL])
    dinv_w = inp("dinv_w", [P, WPC])
    bl_w = inp("bl_w", [P, WPC])
    cnt_l = inp("cnt_l", [P, 2])
    scat = inp("scat", [P, 2], I32)
    rows128 = inp("rows128", [P, 1], I32)
    iota256 = inp("iota256", [P, 256])
    ident = inp("ident", [P, P])
    ones4 = inp("ones4", [P, 4])
    onesrow = inp("onesrow", [1, BPC])
    wasD = inp("wasD", [128, 32])
    wad_aug = inp("wad_aug", [9, 4])
    wg40 = inp("wg40", [40, 128])
    W2 = inp("W2", [128, 64])
    b2row = inp("b2row", [P, 64])
    W3_aug = inp("W3_aug", [65, 128])
    w1k = inp("w1k", [30, 3 * 64])
    b1 = inp("b1", [64, 1])
    w2k = inp("w2k", [64, 3 * 64])
    b2c = inp("b2c", [64, 1])
    fc1_Wr = inp("fc1_Wr", [64, 16 * 64])
    fc1_b = inp("fc1_b", [64, 1])
    fus_W0 = inp("fus_W0", [128, 128])
    fus_W1 = inp("fus_W1", [64, 128])
    fus_b = inp("fus_b", [1, 128])
    cls1_W = inp("cls1_W", [128, 64])
    cls1_b = inp("cls1_b", [1, 64])
    cls3_W = inp("cls3_W", [64, 1])
    cls3_b_t = inp("cls3_b_t", [1, 1])
    xseq = inp("xseq", [30, BPC * 20])

    out = nc.dram_tensor("out", [1, BPC], F32, kind="ExternalOutput")

    T2_local = nc.dram_tensor("T2_local", [LOCAL, 64], F32)
    T2_full = nc.dram_tensor("T2_full", [NTOT, 64], F32)
    T3_local = nc.dram_tensor("T3_local", [LOCAL, 64], F32)
    T3_full = nc.dram_tensor("T3_full", [NTOT, 64], F32)
    AR_in = nc.dram_tensor("AR_in", [BROWS, 128], F32)
    AR_out = nc.dram_tensor("AR_out", [BROWS, 128], F32)

    with tile.TileContext(nc) as tc:
        with tc.tile_pool(name="const", bufs=1) as cp, \
             tc.tile_pool(name="work", bufs=3) as wp, \
             tc.tile_pool(name="gath", bufs=2) as g2p, \
             tc.tile_pool(name="gat", bufs=2) as gp, \
             tc.tile_pool(name="psum", bufs=4, space="PSUM") as pp, \
             tc.tile_pool(name="ppool", bufs=1, space="PSUM") as ppool, \
             tc.tile_pool(name="seq", bufs=1) as sq:

            def c_load(ap, shape, dt=F32):
                t = cp.tile(shape, dt, tag=f"c_{ap.name}")
                nc.sync.dma_start(t[:], ap[:])
                return t

            srcrow_sb = c_load(srcrow, [P, SLOTS], I32)
            dinv_sb = c_load(dinv_w, [P, WPC])
            bl_sb = c_load(bl_w, [P, WPC])
            cnt_sb = c_load(cnt_l, [P, 2])
            scat_sb = c_load(scat, [P, 2], I32)
            rows_sb = c_load(rows128, [P, 1], I32)
            iota_sb = c_load(iota256, [P, 256])
            ident_sb = c_load(ident, [P, P])
            ones4_sb = c_load(ones4, [P, 4])
            onesr_sb = c_load(onesrow, [1, BPC])
            wasD_sb = c_load(wasD, [128, 32])
            wad_sb = c_load(wad_aug, [9, 4])
            wg40_sb = c_load(wg40, [40, 128])
            W2_sb = c_load(W2, [128, 64])
            b2row_sb = c_load(b2row, [P, 64])
            W3_sb = c_load(W3_aug, [65, 128])
            xloc_sb = c_load(xlocT, [9, LOCAL])
            w1_sb = c_load(w1k, [30, 3 * 64])
            b1_sb = c_load(b1, [64, 1])
            w2_sb = c_load(w2k, [64, 3 * 64])
            b2c_sb = c_load(b2c, [64, 1])
            fc1_sb = c_load(fc1_Wr, [64, 16 * 64])
            fc1b_sb = c_load(fc1_b, [64, 1])
            fusW0_sb = c_load(fus_W0, [128, 128])
            fusW1_sb = c_load(fus_W1, [64, 128])
            fusb_sb = c_load(fus_b, [1, 128])
            cls1W_sb = c_load(cls1_W, [128, 64])
            cls1b_sb = c_load(cls1_b, [1, 64])
            cls3W_sb = c_load(cls3_W, [64, 1])
            cls3b_sb = c_load(cls3_b_t, [1, 1])

            # persistent pooling PSUM, zeroed via K=1 matmul (sets has_written)
            pool_ps0 = ppool.tile([P, P], F32, tag="pool0")
            pool_ps1 = ppool.tile([P, P], F32, tag="pool1")
            zrow = cp.tile([1, P], F32)
            nc.vector.memset(zrow[:], 0.0)
            nc.tensor.matmul(pool_ps0[:], zrow[:], zrow[:], start=True, stop=True)
            nc.tensor.matmul(pool_ps1[:], zrow[:], zrow[:], start=True, stop=True)

            def vlrelu(dst, src, tmp_tag, pool, n):
                """dst = leakyrelu(src, 0.01) on the vector engine."""
                t = pool.tile([src.shape[0], n], F32, tag=tmp_tag)
                nc.vector.tensor_scalar(t[:], src, 0.01, None, OP.mult)
                nc.vector.tensor_tensor(dst, src, t[:], op=OP.max)

            # ================= seq branch (BPC batches, overlaps GAT) =====
            xsf = sq.tile([30, BPC * 20], F32, tag="xsf")
            nc.sync.dma_start(xsf[:], xseq[:])
            s1_sb = sq.tile([64, BPC * 18], F32, tag="s1")
            CH1 = 28
            for ci in range((BPC + CH1 - 1) // CH1):
                b0 = ci * CH1
                bn = min(CH1, BPC - b0)
                cps = pp.tile([64, CH1 * 18], F32, tag="ps")
                for k in range(3):
                    nc.tensor.matmul(
                        cps[:, :bn * 18],
                        w1_sb[:, 64 * k:64 * (k + 1)],
                        xsf[:].rearrange("c (b t) -> c b t", t=20)[:, b0:b0 + bn, k:k + 18],
                        start=(k == 0), stop=(k == 2))
                t0 = sq.tile([64, CH1 * 18], F32, tag="sq_t0")
                t1 = sq.tile([64, CH1 * 18], F32, tag="sq_t1")
                nc.vector.tensor_scalar(t0[:, :bn * 18], cps[:, :bn * 18],
                                        b1_sb[:, 0:1], None, OP.add)
                nc.vector.tensor_scalar(t1[:, :bn * 18], cps[:, :bn * 18],
                                        b1_sb[:, 0:1], 0.01, OP.add, OP.mult)
                nc.vector.tensor_tensor(s1_sb[:, b0 * 18:(b0 + bn) * 18],
                                        t0[:, :bn * 18], t1[:, :bn * 18],
                                        op=OP.max)
            s2_sb = sq.tile([64, BPC * 16], F32, tag="s2")
            CH2 = 32
            for ci in range((BPC + CH2 - 1) // CH2):
                b0 = ci * CH2
                bn = min(CH2, BPC - b0)
                cps2 = pp.tile([64, CH2 * 16], F32, tag="ps")
                for k in range(3):
                    nc.tensor.matmul(
                        cps2[:, :bn * 16],
                        w2_sb[:, 64 * k:64 * (k + 1)],
                        s1_sb[:].rearrange("c (b t) -> c b t", t=18)[:, b0:b0 + bn, k:k + 16],
                        start=(k == 0), stop=(k == 2))
                t0 = sq.tile([64, CH2 * 16], F32, tag="sq_u0")
                t1 = sq.tile([64, CH2 * 16], F32, tag="sq_u1")
                nc.vector.tensor_scalar(t0[:, :bn * 16], cps2[:, :bn * 16],
                                        b2c_sb[:, 0:1], None, OP.add)
                nc.vector.tensor_scalar(t1[:, :bn * 16], cps2[:, :bn * 16],
                                        b2c_sb[:, 0:1], 0.01, OP.add, OP.mult)
                nc.vector.tensor_tensor(s2_sb[:, b0 * 16:(b0 + bn) * 16],
                                        t0[:, :bn * 16], t1[:, :bn * 16],
                                        op=OP.max)
            fps = pp.tile([64, BPC], F32, tag="ps")
            for t in range(16):
                nc.tensor.matmul(
                    fps[:],
                    fc1_sb[:].rearrange("c (t j) -> c t j", j=64)[:, t, :],
                    s2_sb[:].rearrange("c (b t) -> c b t", t=16)[:, :, t:t + 1],
                    start=(t == 0), stop=(t == 15))
            sT = sq.tile([64, BPC], F32, tag="sT")
            nc.vector.tensor_scalar(sT[:], fps[:], fc1b_sb[:, 0:1], None, OP.add)

            # ================= GAT =================
            def gat_body(w):
                T8 = T8w[w]
                G = T8 // 8
                gbase = acol[w] // 8
                ad_ps = pp.tile([P, 4], F32, tag="ps")
                nc.tensor.matmul(ad_ps[:], xloc_sb[:, bass.ds(w * P, P)],
                                 wad_sb[:], start=True, stop=True)
                T_d = gp.tile([P, 4], F32, tag="Td")
                nc.scalar.activation(T_d[:], ad_ps[:], AF.Exp, scale=0.8)

                xg = gp.tile([128, GMAX * P], F32, tag="xg")
                nc.sync.dma_start(xg[:, :G * P],
                                  xgrp[:, bass.ds(gbase * P, G * P)])
                as_ps = pp.tile([P, 4 * T8MAX], F32, tag="ps")
                for g in range(G):
                    nc.tensor.matmul(as_ps[:, 32 * g:32 * g + 32],
                                     xg[:, P * g:P * (g + 1)], wasD_sb[:],
                                     start=True, stop=True)
                Pt = gp.tile([P, 4 * T8MAX], F32, tag="Pt")
                Rt = gp.tile([P, 4 * T8MAX], F32, tag="Rt")
                nc.scalar.activation(Pt[:, :4 * T8], as_ps[:, :4 * T8],
                                     AF.Exp, scale=1.0)
                nc.scalar.activation(Rt[:, :4 * T8], as_ps[:, :4 * T8],
                                     AF.Exp, scale=0.2)

                EX = gp.tile([P, 4 * T8MAX], F32, tag="EX")
                nc.vector.tensor_tensor(
                    EX[:, :4 * T8].rearrange("p (t h) -> p t h", h=4),
                    Pt[:, :4 * T8].rearrange("p (t h) -> p t h", h=4),
                    T_d[:, None, :].to_broadcast([P, T8, 4]),
                    op=OP.mult)
                nc.vector.tensor_tensor(EX[:, :4 * T8], EX[:, :4 * T8],
                                        Rt[:, :4 * T8], op=OP.max)
                S4 = gp.tile([P, 4], F32, tag="S4")
                nc.vector.tensor_reduce(
                    S4[:, :, None],
                    EX[:, :4 * T8].rearrange("p (t h) -> p h t", h=4),
                    axis=AX.X, op=OP.add)
                nc.vector.reciprocal(S4[:], S4[:])
                AL = gp.tile([P, 4 * T8MAX], F32, tag="AL")
                nc.vector.tensor_tensor(
                    AL[:, :4 * T8].rearrange("p (t h) -> p t h", h=4),
                    EX[:, :4 * T8].rearrange("p (t h) -> p t h", h=4),
                    S4[:, None, :].to_broadcast([P, T8, 4]),
                    op=OP.mult)

                XS = gp.tile([P, 9 * T8MAX], F32, tag="XS")
                nc.sync.dma_start(XS[:, :9 * T8],
                                  xslots[:, bass.ds(acol[w] * 9, T8 * 9)])
                ZR = gp.tile([P, 36 * T8MAX], F32, tag="ZR")
                nc.vector.tensor_tensor(
                    ZR[:, :36 * T8].rearrange("p (t h f) -> p t h f", h=4, f=9),
                    XS[:, :9 * T8].rearrange("p (t f) -> p t f", f=9)[:, :, None, :]
                        .to_broadcast([P, T8, 4, 9]),
                    AL[:, :4 * T8].rearrange("p (t h) -> p t h", h=4)[:, :, :, None]
                        .to_broadcast([P, T8, 4, 9]),
                    op=OP.mult)
                zaug = gp.tile([P, 40], F32, tag="zaug")
                nc.vector.tensor_copy(
                    zaug[:].rearrange("p (h t) -> p h t", t=10)[:, :, 9:10],
                    ones4_sb[:, :, None])
                nc.vector.tensor_reduce(
                    zaug[:].rearrange("p (h t) -> p h t", t=10)[:, :, 0:9][:, :, :, None],
                    ZR[:, :36 * T8].rearrange("p (t h f) -> p h f t", h=4, f=9),
                    axis=AX.X, op=OP.add)
                zT_ps = pp.tile([40, P], F32, tag="ps")
                nc.tensor.transpose(out=zT_ps[:], in_=zaug[:], identity=ident_sb[:])
                zT = gp.tile([40, P], F32, tag="zTs")
                nc.vector.tensor_copy(zT[:], zT_ps[:])
                g1_ps = pp.tile([P, P], F32, tag="ps")
                nc.tensor.matmul(g1_ps[:], zT[:], wg40_sb[:],
                                 start=True, stop=True)
                g1T = gp.tile([P, P], F32, tag="g1T")
                vlrelu(g1T[:], g1_ps[:], "g1a", gp, P)
                h2_ps = pp.tile([P, 64], F32, tag="ps")
                nc.tensor.matmul(h2_ps[:], g1T[:], W2_sb[:], start=True, stop=True)
                T2s = gp.tile([P, 64], F32, tag="T2s")
                nc.vector.tensor_scalar(T2s[:], h2_ps[:],
                                        dinv_sb[:, bass.ds(w, 1)], None, OP.mult)
                nc.sync.dma_start(T2_local[bass.ds(w * P, P), :], T2s[:])

            with nc.named_scope("gat"):
                for w in range(WPC):
                    gat_body(w)

            tc.strict_bb_all_engine_barrier()
            with nc.named_scope("ag1"):
                nc.gpsimd.collective_compute(
                    "AllGather", OP.bypass, replica_groups=RG,
                    ins=[T2_local.ap().opt()], outs=[T2_full.ap().opt()])
            tc.strict_bb_all_engine_barrier()

            # ================= GCN layers =================
            def gcn_group(ws, table, last):
                TT = sum(Tw[w] for w in ws)
                base = gcol[ws[0]]
                G2 = g2p.tile([P, GCAP * 64], F32, tag="G2")
                nc.gpsimd.indirect_dma_start(
                    out=G2[:, :TT * 64], out_offset=None,
                    in_=table[:],
                    in_offset=bass.IndirectOffsetOnAxis(
                        ap=srcrow_sb[:, base:base + TT], axis=0))
                toff = 0
                for w in ws:
                    T = Tw[w]
                    if not last:
                        z = wp.tile([P, 64], F32, tag="z")
                        nc.vector.tensor_reduce(
                            z[:, :, None],
                            G2[:, toff * 64:(toff + T) * 64]
                                .rearrange("p (t c) -> p c t", c=64),
                            axis=AX.X, op=OP.add)
                        nc.vector.tensor_scalar(
                            z[:], z[:], dinv_sb[:, bass.ds(w, 1)], None, OP.mult)
                        nc.vector.tensor_tensor(z[:], z[:], b2row_sb[:], op=OP.add)
                        za = wp.tile([P, 64], F32, tag="za")
                        nc.vector.tensor_scalar(za[:], z[:], 0.01, None, OP.mult)
                        nc.vector.tensor_tensor(z[:], z[:], za[:], op=OP.max)
                        T3s = wp.tile([P, 64], F32, tag="T3s")
                        nc.vector.tensor_scalar(
                            T3s[:], z[:], dinv_sb[:, bass.ds(w, 1)], None, OP.mult)
                        nc.sync.dma_start(T3_local[bass.ds(w * P, P), :], T3s[:])
                    else:
                        z3s = wp.tile([P, 65], F32, tag="z3s")
                        nc.vector.tensor_reduce(
                            z3s[:, 0:64][:, :, None],
                            G2[:, toff * 64:(toff + T) * 64]
                                .rearrange("p (t c) -> p c t", c=64),
                            axis=AX.X, op=OP.add)
                        nc.vector.tensor_scalar(
                            z3s[:, 0:64], z3s[:, 0:64],
                            dinv_sb[:, bass.ds(w, 1)], None, OP.mult)
                        nc.vector.tensor_copy(z3s[:, 64:65], ones4_sb[:, 0:1])
                        z3T_ps = pp.tile([65, P], F32, tag="ps")
                        nc.tensor.transpose(out=z3T_ps[:], in_=z3s[:],
                                            identity=ident_sb[:])
                        z3T = wp.tile([65, P], F32, tag="z3Ts")
                        nc.vector.tensor_copy(z3T[:], z3T_ps[:])
                        g3_ps = pp.tile([P, P], F32, tag="ps")
                        nc.tensor.matmul(g3_ps[:], z3T[:], W3_sb[:],
                                         start=True, stop=True)
                        g3 = wp.tile([P, P], F32, tag="g3s")
                        vlrelu(g3[:], g3_ps[:], "g3a", wp, P)
                        Mp = wp.tile([P, 256], F32, tag="Mp")
                        nc.vector.tensor_scalar(
                            Mp[:], iota_sb[:], bl_sb[:, bass.ds(w, 1)], None,
                            OP.is_equal)
                        nc.tensor.matmul(pool_ps0[:], Mp[:, 0:128], g3[:],
                                         start=False, stop=True)
                        nc.tensor.matmul(pool_ps1[:], Mp[:, 128:256], g3[:],
                                         start=False, stop=True)
                    toff += T

            with nc.named_scope("gcn1"):
                for ws in GROUPS:
                    gcn_group(ws, T2_full, last=False)

            tc.strict_bb_all_engine_barrier()
            with nc.named_scope("ag2"):
                nc.gpsimd.collective_compute(
                    "AllGather", OP.bypass, replica_groups=RG,
                    ins=[T3_local.ap().opt()], outs=[T3_full.ap().opt()])
            tc.strict_bb_all_engine_barrier()

            with nc.named_scope("gcn2"):
                # zero the AllReduce input (rows not covered by this core)
                zb = wp.tile([P, 128], F32, tag="zb")
                nc.vector.memset(zb[:], 0.0)
                r0 = 0
                while r0 < BROWS:
                    r1 = min(r0 + P, BROWS)
                    nc.sync.dma_start(AR_in[r0:r1, :], zb[:r1 - r0, :])
                    r0 = r1
                for ws in GROUPS:
                    gcn_group(ws, T3_full, last=True)

                crec = wp.tile([P, 2], F32, tag="crec")
                nc.vector.reciprocal(crec[:], cnt_sb[:])
                for k, pps in enumerate((pool_ps0, pool_ps1)):
                    pooled = wp.tile([P, 128], F32, tag="pooled")
                    nc.vector.tensor_scalar(pooled[:], pps[:],
                                            crec[:, k:k + 1], None, OP.mult)
                    nc.gpsimd.indirect_dma_start(
                        out=AR_in[:], out_offset=bass.IndirectOffsetOnAxis(
                            ap=scat_sb[:, k:k + 1], axis=0),
                        in_=pooled[:], in_offset=None)

            tc.strict_bb_all_engine_barrier()
            with nc.named_scope("ar"):
                nc.gpsimd.collective_compute(
                    "AllReduce", OP.add, replica_groups=RG,
                    ins=[AR_in.ap().opt()], outs=[AR_out.ap().opt()])
            tc.strict_bb_all_engine_barrier()

            # ================= fusion + classifier (BPC batches) ==========
            with nc.named_scope("fuse"):
                prow = sq.tile([P, 128], F32, tag="prow")
                nc.gpsimd.indirect_dma_start(
                    out=prow[:], out_offset=None,
                    in_=AR_out[:],
                    in_offset=bass.IndirectOffsetOnAxis(
                        ap=rows_sb[:, 0:1], axis=0))
                tp_ps = pp.tile([P, P], F32, tag="ps")
                nc.tensor.transpose(out=tp_ps[:], in_=prow[:], identity=ident_sb[:])
                poolT = sq.tile([P, BPC], F32, tag="poolT")
                nc.vector.tensor_copy(poolT[:], tp_ps[:])

                ups = pp.tile([P, BPC], F32, tag="ps")
                nc.tensor.matmul(ups[:], fusW0_sb[:], poolT[:],
                                 start=True, stop=False)
                nc.tensor.matmul(ups[:], fusW1_sb[:], sT[:],
                                 start=False, stop=False)
                nc.tensor.matmul(ups[:], fusb_sb[:], onesr_sb[:],
                                 start=False, stop=True)
                combT = sq.tile([P, BPC], F32, tag="combT")
                vlrelu(combT[:], ups[:], "fu_a", sq, BPC)
                vps = pp.tile([64, BPC], F32, tag="ps")
                nc.tensor.matmul(vps[:], cls1W_sb[:], combT[:],
                                 start=True, stop=False)
                nc.tensor.matmul(vps[:], cls1b_sb[:], onesr_sb[:],
                                 start=False, stop=True)
                c1T = sq.tile([64, BPC], F32, tag="c1T")
                vlrelu(c1T[:], vps[:], "fu_b", sq, BPC)
                ops_ = pp.tile([1, BPC], F32, tag="ps")
                nc.tensor.matmul(ops_[:], cls3W_sb[:], c1T[:],
                                 start=True, stop=True)
                out_sb = sq.tile([1, BPC], F32, tag="out_sb")
                nc.vector.tensor_scalar(
                    out_sb[:], ops_[:], cls3b_sb[0:1, 0:1], None, OP.add)
                nc.sync.dma_start(out[:], out_sb[:])

    nc.compile()
    return nc


# --------------------------------------------------------------------------
# entry point
# --------------------------------------------------------------------------

_CACHE = {}
LAST_RESULT = None


def kernel(**inputs):
    kh = hash((np.asarray(inputs['edge_index']).tobytes(),
               np.asarray(inputs['x']).tobytes()))
    if kh not in _CACHE:
        per_core, baked = host_prep(inputs)
        nc = build_nc(baked)
        _CACHE[kh] = (per_core, baked, nc)
    per_core, baked, nc = _CACHE[kh]

    wts = fold_weights(inputs)
    Bsz = baked['Bsz']
    BPC = Bsz // NC_CORES
    seq = np.asarray(inputs['seq_data'], np.float32)      # [B, 30, 20]
    seqT = np.ascontiguousarray(seq.transpose(1, 0, 2))   # [30, B, 20]
    shared = dict(
        iota256=np.ascontiguousarray(
            np.broadcast_to(np.arange(256, dtype=np.float32), (P, 256))),
        ident=np.eye(P, dtype=np.float32),
        ones4=np.ones((P, 4), np.float32),
        onesrow=np.ones((1, BPC), np.float32),
        **wts)
    in_maps = []
    for c in range(NC_CORES):
        m = dict(shared)
        m.update(per_core[c])
        m['xseq'] = np.ascontiguousarray(
            seqT[:, c * BPC:(c + 1) * BPC, :]).reshape(30, BPC * 20)
        in_maps.append(m)

    global LAST_RESULT
    res = run_bass_kernel_spmd(
        nc, in_maps, core_ids=list(range(NC_CORES)),
        trace=bool(os.environ.get('BASS_KERNEL_TRACE')))
    LAST_RESULT = res
    o = np.concatenate([res.results[c]["out"].reshape(-1)
                        for c in range(NC_CORES)]).reshape(Bsz, 1)
    return o.astype(np.float32)


# revision 14
# speedup vs baseline: 7.9406x; 1.5160x over previous
"""Trainium2 Bass kernel for nn_DeepCPP (GAT + 2xGCN graph branch, conv1d seq
branch, fusion MLP), SPMD over 8 NeuronCores.

Sharding/strategy:
 - Nodes partitioned across cores in natural order (keeps sorted `batch`
   contiguous per core); within a core nodes are sorted by in-degree so
   128-node windows have near-uniform max degree (node-major slot grids).
 - GAT attention logits per edge slot are computed with block-diagonal
   batched matmuls (8 slot-columns per matmul); exp(leakyrelu(a_s+a_d)) is
   factorized as max(P_e*T_d, R_e) with P=exp(a_s), R=exp(0.2*a_s),
   T=exp(0.8*a_d); the per-dst factor exp(-0.2*a_d) cancels in the softmax.
 - GCN layers gather 256B rows (dinv-prescaled h) from an AllGathered table
   with ONE batched indirect DMA per pair of 128-node windows; aggregation
   is a strided vector reduction.
 - Mean-pool via one-hot selection matmuls into persistent PSUM, AllReduce
   of partials; seq branch and fusion MLP are sharded by batch (128/core).
 - All loops fully unrolled (no hardware loops); non-Exp pointwise work runs
   on the Vector engine so the Scalar activation table stays loaded.
"""

import os
import sys

sys.path.insert(0, '/opt/trn_rl_repo')

import numpy as np
import ml_dtypes

import concourse.bass as bass
import concourse.mybir as mybir
import concourse.tile as tile
from concourse import bacc
from concourse.bass_utils import run_bass_kernel_spmd

F32 = mybir.dt.float32
BF16 = mybir.dt.bfloat16
I32 = mybir.dt.int32
AF = mybir.ActivationFunctionType
OP = mybir.AluOpType
AX = mybir.AxisListType

NC_CORES = 8
P = 128


# --------------------------------------------------------------------------
# host-side prep (layout/indexing only; cached per (x, edge_index))
# --------------------------------------------------------------------------

def host_prep(inputs):
    x = np.asarray(inputs['x'], np.float32)
    ei = np.asarray(inputs['edge_index'], np.int64)
    batch = np.asarray(inputs['batch'], np.int64)
    N = x.shape[0]
    Bsz = int(np.asarray(inputs['seq_data']).shape[0])
    assert N % NC_CORES == 0
    REAL = N // NC_CORES
    WPC = (REAL + P - 1) // P
    LOCAL = WPC * P
    NTOT = LOCAL * NC_CORES
    SENT = REAL if REAL < LOCAL else REAL - 1   # sentinel zero row in core 0

    src2 = np.concatenate([ei[0], np.arange(N)])
    dst2 = np.concatenate([ei[1], np.arange(N)])
    deg = np.bincount(dst2, minlength=N)

    local_rank = np.zeros(N, np.int64)
    rowid = np.zeros(N, np.int64)
    node_at = np.full((NC_CORES, LOCAL), -1, np.int64)
    for c in range(NC_CORES):
        ns = np.arange(c * REAL, (c + 1) * REAL)
        order = ns[np.argsort(-deg[ns], kind='stable')]
        local_rank[order] = np.arange(REAL)
        rowid[order] = c * LOCAL + np.arange(REAL)
        node_at[c, :REAL] = order

    # per-window max degree (shared across cores so the program is SPMD)
    Tw = np.ones(WPC, np.int64)
    for c in range(NC_CORES):
        first = node_at[c, ::P]
        for w in range(WPC):
            if first[w] >= 0:
                Tw[w] = max(Tw[w], deg[first[w]])
    T8w = ((Tw + 7) // 8) * 8
    gcol = np.concatenate([[0], np.cumsum(Tw)])       # GCN grid col offsets
    acol = np.concatenate([[0], np.cumsum(T8w)])      # GAT grid col offsets
    SLOTS = int(gcol[-1])
    SLOTS8 = int(acol[-1])
    GTOT = SLOTS8 // 8
    assert T8w.max() * 4 <= 160, "packed PSUM layout needs T8 <= 40"

    e_dst = rowid[dst2]
    e_src = src2
    o = np.argsort(e_dst, kind='stable')
    e_dst = e_dst[o]
    e_src = e_src[o]
    grp_start = np.searchsorted(e_dst, np.arange(NTOT), side='left')
    t_of = np.arange(len(e_dst)) - grp_start[e_dst]
    c_of = e_dst // LOCAL
    lrow = e_dst % LOCAL
    w_of = lrow // P
    p_of = lrow % P
    assert (t_of < Tw[w_of]).all()
    col_g = gcol[w_of] + t_of
    col_a = acol[w_of] + t_of

    slot_node_g = np.full((NC_CORES, P, SLOTS), N, np.int64)
    slot_node_g[c_of, p_of, col_g] = e_src
    slot_node_a = np.full((NC_CORES, P, SLOTS8), N, np.int64)
    slot_node_a[c_of, p_of, col_a] = e_src

    x_pad = np.vstack([x, np.zeros((1, x.shape[1]), np.float32)])
    rowid_pad = np.concatenate([rowid, [SENT]]).astype(np.int32)

    cnt = np.bincount(batch, minlength=Bsz).astype(np.float32)
    per_core = []
    for c in range(NC_CORES):
        sna = slot_node_a[c]                       # [P, SLOTS8], N = pad
        xs = x_pad[sna]                            # [P, SLOTS8, 9]
        xslots = np.ascontiguousarray(
            xs.reshape(P, SLOTS8 * 9)).astype(ml_dtypes.bfloat16)
        xTl = np.zeros((16, SLOTS8, P), np.float32)
        xTl[0:9] = xs.transpose(2, 1, 0)
        xTl[9] = (sna.T == N).astype(np.float32)   # pad flag
        # [16j+f, (group)*128 + p] = xTl[f, 8*group+j, p]
        xgrp = np.ascontiguousarray(
            xTl.reshape(16, GTOT, 8, P).transpose(2, 0, 1, 3)
               .reshape(128, GTOT * P)).astype(ml_dtypes.bfloat16)
        srcrow = rowid_pad[slot_node_g[c]]         # [P, SLOTS]

        valid = node_at[c] >= 0
        xloc = np.zeros((9, LOCAL), np.float32)
        xloc[0:9, valid] = x[node_at[c][valid]].T

        dinv = np.zeros(LOCAL, np.float32)
        dinv[valid] = 1.0 / np.sqrt(deg[node_at[c][valid]])
        dinv_w = np.ascontiguousarray(dinv.reshape(WPC, P).T)

        bl = np.full(LOCAL, -1.0, np.float32)
        b_base = int(batch[c * REAL])
        bl[valid] = batch[node_at[c][valid]] - b_base
        assert bl.max() < 256, "batch window exceeded 256"
        bl_w = np.ascontiguousarray(bl.reshape(WPC, P).T)

        cnt_l = np.ones(256, np.float32)
        hi = min(256, Bsz - b_base)
        cnt_l[:hi] = np.maximum(cnt[b_base:b_base + hi], 1.0)
        scatv = np.zeros(256, np.int32)
        for j in range(256):
            scatv[j] = b_base + j if b_base + j < Bsz else Bsz + (j % 8)

        per_core.append(dict(
            xslots=xslots, xgrp=xgrp, srcrow=srcrow.astype(np.int32),
            xlocT=xloc, dinv_w=dinv_w, bl_w=bl_w,
            cnt_l=np.ascontiguousarray(cnt_l.reshape(2, P).T),
            scat=np.ascontiguousarray(scatv.reshape(2, P).T),
            rows128=(c * P + np.arange(P, dtype=np.int32)).reshape(P, 1),
        ))

    baked = dict(N=N, REAL=REAL, WPC=WPC, LOCAL=LOCAL, NTOT=NTOT,
                 SLOTS=SLOTS, SLOTS8=SLOTS8, GTOT=GTOT,
                 Tw=[int(t) for t in Tw], T8w=[int(t) for t in T8w],
                 gcol=[int(t) for t in gcol], acol=[int(t) for t in acol],
                 Bsz=Bsz)
    return per_core, baked


def fold_weights(inputs):
    w = {k: np.asarray(v, np.float32) for k, v in inputs.items()
         if k not in ('x', 'edge_index', 'batch')}
    H, C = 4, 32
    Wg = w['W_gat']
    was = np.einsum('fhc,hc->fh', Wg.reshape(9, H, C), w['att_src'])
    wad = np.einsum('fhc,hc->fh', Wg.reshape(9, H, C), w['att_dst'])
    was_aug = np.zeros((16, 4), np.float32)
    was_aug[0:9] = was
    was_aug[9] = -80.0
    wad_aug = np.zeros((9, 4), np.float32)
    wad_aug[0:9] = wad
    # block-diagonal was for 8 slot-columns per matmul
    wasD = np.zeros((128, 32), np.float32)
    for j in range(8):
        wasD[16 * j:16 * j + 16, 4 * j:4 * j + 4] = was_aug
    # [40,128] compact GAT weight: rows (10h+f) f<9 = W_gat, f=9 = bias
    wg40 = np.zeros((40, 128), np.float32)
    for h in range(H):
        wg40[h * 10:h * 10 + 9, h * 32:(h + 1) * 32] = Wg[:, h * 32:(h + 1) * 32]
        wg40[h * 10 + 9, h * 32:(h + 1) * 32] = w['b_gat'][h * 32:(h + 1) * 32]
    W3_aug = np.zeros((65, 128), np.float32)
    W3_aug[0:64] = w['W3']
    W3_aug[64] = w['b3']

    def fold(cw, cb, g, be, m, v):
        s = g / np.sqrt(v + 1e-5)
        return cw * s[:, None, None], (cb - m) * s + be

    c1w, c1b = fold(w['conv1_w'], w['conv1_b'], w['bn1_g'], w['bn1_b'],
                    w['bn1_m'], w['bn1_v'])
    c2w, c2b = fold(w['conv2_w'], w['conv2_b'], w['bn2_g'], w['bn2_b'],
                    w['bn2_m'], w['bn2_v'])
    # [cin, k, cout] flattened so slice k -> [cin, cout]
    w1k = np.ascontiguousarray(c1w.transpose(1, 2, 0)).reshape(30, 3 * 64)
    w2k = np.ascontiguousarray(c2w.transpose(1, 2, 0)).reshape(64, 3 * 64)
    fc1_Wr = np.ascontiguousarray(w['fc1_W'].reshape(64, 16 * 64))

    return dict(
        wasD=wasD.astype(ml_dtypes.bfloat16), wad_aug=wad_aug, wg40=wg40,
        W2=w['W2'], b2row=np.ascontiguousarray(np.broadcast_to(w['b2'], (P, 64))),
        W3_aug=W3_aug,
        w1k=w1k, b1=np.ascontiguousarray(c1b.reshape(64, 1)),
        w2k=w2k, b2c=np.ascontiguousarray(c2b.reshape(64, 1)),
        fc1_Wr=fc1_Wr, fc1_b=np.ascontiguousarray(w['fc1_b'].reshape(64, 1)),
        fus_W0=np.ascontiguousarray(w['fus_W'][0:128]),
        fus_W1=np.ascontiguousarray(w['fus_W'][128:192]),
        fus_b=np.ascontiguousarray(w['fus_b'].reshape(1, 128)),
        cls1_W=w['cls1_W'],
        cls1_b=np.ascontiguousarray(w['cls1_b'].reshape(1, 64)),
        cls3_W=w['cls3_W'],
        cls3_b_t=np.array([[float(w['cls3_b'][0])]], np.float32),
    )


# --------------------------------------------------------------------------
# device program
# --------------------------------------------------------------------------

def build_nc(baked):
    WPC, LOCAL, NTOT = baked['WPC'], baked['LOCAL'], baked['NTOT']
    SLOTS, SLOTS8, GTOT = baked['SLOTS'], baked['SLOTS8'], baked['GTOT']
    Tw, T8w, gcol, acol = baked['Tw'], baked['T8w'], baked['gcol'], baked['acol']
    Bsz = baked['Bsz']
    BROWS = Bsz + 8
    BPC = Bsz // NC_CORES                      # batches per core (fusion/seq)
    RG = [list(range(NC_CORES))]
    T8MAX = max(T8w)
    GMAX = T8MAX // 8
    # adaptive window groups for batched GCN gathers (cap on slot columns)
    GCAP = max(56, max(Tw))
    GROUPS = []
    cur, curT = [], 0
    for w in range(WPC):
        if cur and curT + Tw[w] > GCAP:
            GROUPS.append(cur)
            cur, curT = [], 0
        cur.append(w)
        curT += Tw[w]
    if cur:
        GROUPS.append(cur)

    nc = bacc.Bacc("TRN2", target_bir_lowering=False, debug=False,
                   num_devices=NC_CORES)

    def inp(name, shape, dt=F32):
        return nc.dram_tensor(name, shape, dt, kind="ExternalInput")

    xgrp = inp("xgrp", [128, GTOT * P], BF16)
    xslots = inp("xslots", [P, SLOTS8 * 9], BF16)
    srcrow = inp("srcrow", [P, SLOTS], I32)
    xlocT = inp("xlocT", [9, LOCAL])
    dinv_w = inp("dinv_w", [P, WPC])
    bl_w = inp("bl_w", [P, WPC])
    cnt_l = inp("cnt_l", [P, 2])
    scat = inp("scat", [P, 2], I32)
    rows128 = inp("rows128", [P, 1], I32)
    iota256 = inp("iota256", [P, 256])
    ident = inp("ident", [P, P])
    ones4 = inp("ones4", [P, 4])
    onesrow = inp("onesrow", [1, BPC])
    wasD = inp("wasD", [128, 32], BF16)
    wad_aug = inp("wad_aug", [9, 4])
    wg40 = inp("wg40", [40, 128])
    W2 = inp("W2", [128, 64])
    b2row = inp("b2row", [P, 64])
    W3_aug = inp("W3_aug", [65, 128])
    w1k = inp("w1k", [30, 3 * 64])
    b1 = inp("b1", [64, 1])
    w2k = inp("w2k", [64, 3 * 64])
    b2c = inp("b2c", [64, 1])
    fc1_Wr = inp("fc1_Wr", [64, 16 * 64])
    fc1_b = inp("fc1_b", [64, 1])
    fus_W0 = inp("fus_W0", [128, 128])
    fus_W1 = inp("fus_W1", [64, 128])
    fus_b = inp("fus_b", [1, 128])
    cls1_W = inp("cls1_W", [128, 64])
    cls1_b = inp("cls1_b", [1, 64])
    cls3_W = inp("cls3_W", [64, 1])
    cls3_b_t = inp("cls3_b_t", [1, 1])
    xseq = inp("xseq", [30, BPC * 20])

    out = nc.dram_tensor("out", [1, BPC], F32, kind="ExternalOutput")

    T2_local = nc.dram_tensor("T2_local", [LOCAL, 64], BF16)
    T2_full = nc.dram_tensor("T2_full", [NTOT, 64], BF16)
    T3_local = nc.dram_tensor("T3_local", [LOCAL, 64], BF16)
    T3_full = nc.dram_tensor("T3_full", [NTOT, 64], BF16)
    AR_in = nc.dram_tensor("AR_in", [BROWS, 128], BF16)
    AR_out = nc.dram_tensor("AR_out", [BROWS, 128], BF16)

    with tile.TileContext(nc) as tc:
        with tc.tile_pool(name="const", bufs=1) as cp, \
             tc.tile_pool(name="work", bufs=3) as wp, \
             tc.tile_pool(name="gath", bufs=3) as g2p, \
             tc.tile_pool(name="gat", bufs=3) as gp, \
             tc.tile_pool(name="psum", bufs=4, space="PSUM") as pp, \
             tc.tile_pool(name="spsum", bufs=2, space="PSUM") as spp, \
             tc.tile_pool(name="ppool", bufs=1, space="PSUM") as ppool, \
             tc.tile_pool(name="seq", bufs=1) as sq:

            def c_load(ap, shape, dt=F32):
                t = cp.tile(shape, dt, tag=f"c_{ap.name}")
                nc.sync.dma_start(t[:], ap[:])
                return t

            srcrow_sb = c_load(srcrow, [P, SLOTS], I32)
            dinv_sb = c_load(dinv_w, [P, WPC])
            bl_sb = c_load(bl_w, [P, WPC])
            cnt_sb = c_load(cnt_l, [P, 2])
            scat_sb = c_load(scat, [P, 2], I32)
            rows_sb = c_load(rows128, [P, 1], I32)
            iota_sb = c_load(iota256, [P, 256])
            ident_sb = c_load(ident, [P, P])
            ones4_sb = c_load(ones4, [P, 4])
            onesr_sb = c_load(onesrow, [1, BPC])
            wasD_sb = c_load(wasD, [128, 32], BF16)
            wad_sb = c_load(wad_aug, [9, 4])
            wg40_sb = c_load(wg40, [40, 128])
            W2_sb = c_load(W2, [128, 64])
            b2row_sb = c_load(b2row, [P, 64])
            W3_sb = c_load(W3_aug, [65, 128])
            xloc_sb = c_load(xlocT, [9, LOCAL])
            w1_sb = c_load(w1k, [30, 3 * 64])
            b1_sb = c_load(b1, [64, 1])
            w2_sb = c_load(w2k, [64, 3 * 64])
            b2c_sb = c_load(b2c, [64, 1])
            fc1_sb = c_load(fc1_Wr, [64, 16 * 64])
            fc1b_sb = c_load(fc1_b, [64, 1])
            fusW0_sb = c_load(fus_W0, [128, 128])
            fusW1_sb = c_load(fus_W1, [64, 128])
            fusb_sb = c_load(fus_b, [1, 128])
            cls1W_sb = c_load(cls1_W, [128, 64])
            cls1b_sb = c_load(cls1_b, [1, 64])
            cls3W_sb = c_load(cls3_W, [64, 1])
            cls3b_sb = c_load(cls3_b_t, [1, 1])

            # persistent pooling PSUM, zeroed via K=1 matmul (sets has_written)
            pool_ps0 = ppool.tile([P, P], F32, tag="pool0")
            pool_ps1 = ppool.tile([P, P], F32, tag="pool1")
            zrow = cp.tile([1, P], F32)
            nc.vector.memset(zrow[:], 0.0)
            nc.tensor.matmul(pool_ps0[:], zrow[:], zrow[:], start=True, stop=True)
            nc.tensor.matmul(pool_ps1[:], zrow[:], zrow[:], start=True, stop=True)

            def vlrelu(dst, src, tmp_tag, pool, n):
                """dst = leakyrelu(src, 0.01) on the vector engine."""
                t = pool.tile([src.shape[0], n], F32, tag=tmp_tag)
                nc.vector.tensor_scalar(t[:], src, 0.01, None, OP.mult)
                nc.vector.tensor_tensor(dst, src, t[:], op=OP.max)

            # ================= seq branch (BPC batches, overlaps GAT) =====
            xsf = sq.tile([30, BPC * 20], F32, tag="xsf")
            nc.sync.dma_start(xsf[:], xseq[:])
            s1_sb = sq.tile([64, BPC * 18], F32, tag="s1")
            CH1 = 28
            for ci in range((BPC + CH1 - 1) // CH1):
                b0 = ci * CH1
                bn = min(CH1, BPC - b0)
                cps = spp.tile([64, 512], F32, tag="sps")
                for k in range(3):
                    nc.tensor.matmul(
                        cps[:, :bn * 18],
                        w1_sb[:, 64 * k:64 * (k + 1)],
                        xsf[:].rearrange("c (b t) -> c b t", t=20)[:, b0:b0 + bn, k:k + 18],
                        start=(k == 0), stop=(k == 2))
                t0 = sq.tile([64, CH1 * 18], F32, tag="sq_t0")
                t1 = sq.tile([64, CH1 * 18], F32, tag="sq_t1")
                nc.vector.tensor_scalar(t0[:, :bn * 18], cps[:, :bn * 18],
                                        b1_sb[:, 0:1], None, OP.add)
                nc.vector.tensor_scalar(t1[:, :bn * 18], cps[:, :bn * 18],
                                        b1_sb[:, 0:1], 0.01, OP.add, OP.mult)
                nc.vector.tensor_tensor(s1_sb[:, b0 * 18:(b0 + bn) * 18],
                                        t0[:, :bn * 18], t1[:, :bn * 18],
                                        op=OP.max)
            s2_sb = sq.tile([64, BPC * 16], F32, tag="s2")
            CH2 = 32
            for ci in range((BPC + CH2 - 1) // CH2):
                b0 = ci * CH2
                bn = min(CH2, BPC - b0)
                cps2 = spp.tile([64, 512], F32, tag="sps")
                for k in range(3):
                    nc.tensor.matmul(
                        cps2[:, :bn * 16],
                        w2_sb[:, 64 * k:64 * (k + 1)],
                        s1_sb[:].rearrange("c (b t) -> c b t", t=18)[:, b0:b0 + bn, k:k + 16],
                        start=(k == 0), stop=(k == 2))
                t0 = sq.tile([64, CH2 * 16], F32, tag="sq_u0")
                t1 = sq.tile([64, CH2 * 16], F32, tag="sq_u1")
                nc.vector.tensor_scalar(t0[:, :bn * 16], cps2[:, :bn * 16],
                                        b2c_sb[:, 0:1], None, OP.add)
                nc.vector.tensor_scalar(t1[:, :bn * 16], cps2[:, :bn * 16],
                                        b2c_sb[:, 0:1], 0.01, OP.add, OP.mult)
                nc.vector.tensor_tensor(s2_sb[:, b0 * 16:(b0 + bn) * 16],
                                        t0[:, :bn * 16], t1[:, :bn * 16],
                                        op=OP.max)
            fps = spp.tile([64, 512], F32, tag="sps")
            for t in range(16):
                nc.tensor.matmul(
                    fps[:, :BPC],
                    fc1_sb[:].rearrange("c (t j) -> c t j", j=64)[:, t, :],
                    s2_sb[:].rearrange("c (b t) -> c b t", t=16)[:, :, t:t + 1],
                    start=(t == 0), stop=(t == 15))
            sT = sq.tile([64, BPC], F32, tag="sT")
            nc.vector.tensor_scalar(sT[:], fps[:, :BPC], fc1b_sb[:, 0:1], None, OP.add)

            # ================= GAT =================
            def gat_body(w):
                T8 = T8w[w]
                G = T8 // 8
                gbase = acol[w] // 8
                ps = pp.tile([P, 512], F32, tag="ps")   # one PSUM bank/window
                ad_ps = ps[:, 0:4]
                as_ps = ps[:, 32:32 + 4 * T8MAX]
                zT_ps = ps[0:40, 192:320]
                g1_ps = ps[:, 320:448]
                h2_ps = ps[:, 448:512]
                nc.tensor.matmul(ad_ps, xloc_sb[:, bass.ds(w * P, P)],
                                 wad_sb[:], start=True, stop=True)
                T_d = gp.tile([P, 4], BF16, tag="Td")
                nc.scalar.activation(T_d[:], ad_ps, AF.Exp, scale=0.8)

                xg = gp.tile([128, GMAX * P], BF16, tag="xg")
                nc.sync.dma_start(xg[:, :G * P],
                                  xgrp[:, bass.ds(gbase * P, G * P)])
                for g in range(G):
                    nc.tensor.matmul(as_ps[:, 32 * g:32 * g + 32],
                                     xg[:, P * g:P * (g + 1)], wasD_sb[:],
                                     start=True, stop=True)
                Pt = gp.tile([P, 4 * T8MAX], BF16, tag="Pt")
                Rt = gp.tile([P, 4 * T8MAX], BF16, tag="Rt")
                nc.scalar.activation(Pt[:, :4 * T8], as_ps[:, :4 * T8],
                                     AF.Exp, scale=1.0)
                nc.scalar.activation(Rt[:, :4 * T8], as_ps[:, :4 * T8],
                                     AF.Exp, scale=0.2)

                EX = gp.tile([P, 4 * T8MAX], BF16, tag="EX")
                nc.vector.tensor_tensor(
                    EX[:, :4 * T8].rearrange("p (t h) -> p t h", h=4),
                    Pt[:, :4 * T8].rearrange("p (t h) -> p t h", h=4),
                    T_d[:, None, :].to_broadcast([P, T8, 4]),
                    op=OP.mult)
                nc.vector.tensor_tensor(EX[:, :4 * T8], EX[:, :4 * T8],
                                        Rt[:, :4 * T8], op=OP.max)
                S4 = gp.tile([P, 4], F32, tag="S4")
                nc.vector.tensor_reduce(
                    S4[:, :, None],
                    EX[:, :4 * T8].rearrange("p (t h) -> p h t", h=4),
                    axis=AX.X, op=OP.add)
                nc.vector.reciprocal(S4[:], S4[:])
                S4b = gp.tile([P, 4], BF16, tag="S4b")
                nc.vector.tensor_copy(S4b[:], S4[:])
                AL = gp.tile([P, 4 * T8MAX], BF16, tag="AL")
                nc.vector.tensor_tensor(
                    AL[:, :4 * T8].rearrange("p (t h) -> p t h", h=4),
                    EX[:, :4 * T8].rearrange("p (t h) -> p t h", h=4),
                    S4b[:, None, :].to_broadcast([P, T8, 4]),
                    op=OP.mult)

                XS = gp.tile([P, 9 * T8MAX], BF16, tag="XS")
                nc.sync.dma_start(XS[:, :9 * T8],
                                  xslots[:, bass.ds(acol[w] * 9, T8 * 9)])
                ZR = gp.tile([P, 36 * T8MAX], BF16, tag="ZR")
                nc.vector.tensor_tensor(
                    ZR[:, :36 * T8].rearrange("p (t h f) -> p t h f", h=4, f=9),
                    XS[:, :9 * T8].rearrange("p (t f) -> p t f", f=9)[:, :, None, :]
                        .to_broadcast([P, T8, 4, 9]),
                    AL[:, :4 * T8].rearrange("p (t h) -> p t h", h=4)[:, :, :, None]
                        .to_broadcast([P, T8, 4, 9]),
                    op=OP.mult)
                zaug = gp.tile([P, 40], F32, tag="zaug")
                nc.vector.tensor_copy(
                    zaug[:].rearrange("p (h t) -> p h t", t=10)[:, :, 9:10],
                    ones4_sb[:, :, None])
                nc.vector.tensor_reduce(
                    zaug[:].rearrange("p (h t) -> p h t", t=10)[:, :, 0:9][:, :, :, None],
                    ZR[:, :36 * T8].rearrange("p (t h f) -> p h f t", h=4, f=9),
                    axis=AX.X, op=OP.add)
                nc.tensor.transpose(out=zT_ps, in_=zaug[:], identity=ident_sb[:])
                zT = gp.tile([40, P], F32, tag="zTs")
                nc.vector.tensor_copy(zT[:], zT_ps)
                nc.tensor.matmul(g1_ps, zT[:], wg40_sb[:],
                                 start=True, stop=True)
                g1T = gp.tile([P, P], F32, tag="g1T")
                vlrelu(g1T[:], g1_ps, "g1a", gp, P)
                nc.tensor.matmul(h2_ps, g1T[:], W2_sb[:], start=True, stop=True)
                T2s = gp.tile([P, 64], BF16, tag="T2s")
                nc.vector.tensor_scalar(T2s[:], h2_ps,
                                        dinv_sb[:, bass.ds(w, 1)], None, OP.mult)
                nc.sync.dma_start(T2_local[bass.ds(w * P, P), :], T2s[:])

            with nc.named_scope("gat"):
                for w in range(WPC):
                    gat_body(w)

            tc.strict_bb_all_engine_barrier()
            with nc.named_scope("ag1"):
                nc.gpsimd.collective_compute(
                    "AllGather", OP.bypass, replica_groups=RG,
                    ins=[T2_local.ap().opt()], outs=[T2_full.ap().opt()])
            tc.strict_bb_all_engine_barrier()

            # ================= GCN layers =================
            def gcn_group(ws, table, last):
                TT = sum(Tw[w] for w in ws)
                base = gcol[ws[0]]
                G2 = g2p.tile([P, GCAP * 64], BF16, tag="G2")
                nc.gpsimd.indirect_dma_start(
                    out=G2[:, :TT * 64], out_offset=None,
                    in_=table[:],
                    in_offset=bass.IndirectOffsetOnAxis(
                        ap=srcrow_sb[:, base:base + TT], axis=0))
                toff = 0
                for w in ws:
                    T = Tw[w]
                    if not last:
                        z = wp.tile([P, 64], F32, tag="z")
                        nc.vector.tensor_reduce(
                            z[:, :, None],
                            G2[:, toff * 64:(toff + T) * 64]
                                .rearrange("p (t c) -> p c t", c=64),
                            axis=AX.X, op=OP.add)
                        nc.vector.tensor_scalar(
                            z[:], z[:], dinv_sb[:, bass.ds(w, 1)], None, OP.mult)
                        nc.vector.tensor_tensor(z[:], z[:], b2row_sb[:], op=OP.add)
                        za = wp.tile([P, 64], F32, tag="za")
                        nc.vector.tensor_scalar(za[:], z[:], 0.01, None, OP.mult)
                        nc.vector.tensor_tensor(z[:], z[:], za[:], op=OP.max)
                        T3s = wp.tile([P, 64], BF16, tag="T3s")
                        nc.vector.tensor_scalar(
                            T3s[:], z[:], dinv_sb[:, bass.ds(w, 1)], None, OP.mult)
                        nc.sync.dma_start(T3_local[bass.ds(w * P, P), :], T3s[:])
                    else:
                        z3s = wp.tile([P, 65], F32, tag="z3s")
                        nc.vector.tensor_reduce(
                            z3s[:, 0:64][:, :, None],
                            G2[:, toff * 64:(toff + T) * 64]
                                .rearrange("p (t c) -> p c t", c=64),
                            axis=AX.X, op=OP.add)
                        nc.vector.tensor_scalar(
                            z3s[:, 0:64], z3s[:, 0:64],
                            dinv_sb[:, bass.ds(w, 1)], None, OP.mult)
                        nc.vector.tensor_copy(z3s[:, 64:65], ones4_sb[:, 0:1])
                        ps2 = pp.tile([P, 512], F32, tag="ps")
                        z3T_ps = ps2[0:65, 0:128]
                        g3_ps = ps2[:, 128:256]
                        nc.tensor.transpose(out=z3T_ps, in_=z3s[:],
                                            identity=ident_sb[:])
                        z3T = wp.tile([65, P], F32, tag="z3Ts")
                        nc.vector.tensor_copy(z3T[:], z3T_ps)
                        nc.tensor.matmul(g3_ps, z3T[:], W3_sb[:],
                                         start=True, stop=True)
                        g3 = wp.tile([P, P], F32, tag="g3s")
                        vlrelu(g3[:], g3_ps, "g3a", wp, P)
                        Mp = wp.tile([P, 256], F32, tag="Mp")
                        nc.vector.tensor_scalar(
                            Mp[:], iota_sb[:], bl_sb[:, bass.ds(w, 1)], None,
                            OP.is_equal)
                        nc.tensor.matmul(pool_ps0[:], Mp[:, 0:128], g3[:],
                                         start=False, stop=True)
                        nc.tensor.matmul(pool_ps1[:], Mp[:, 128:256], g3[:],
                                         start=False, stop=True)
                    toff += T

            with nc.named_scope("gcn1"):
                for ws in GROUPS:
                    gcn_group(ws, T2_full, last=False)

            tc.strict_bb_all_engine_barrier()
            with nc.named_scope("ag2"):
                nc.gpsimd.collective_compute(
                    "AllGather", OP.bypass, replica_groups=RG,
                    ins=[T3_local.ap().opt()], outs=[T3_full.ap().opt()])
            tc.strict_bb_all_engine_barrier()

            with nc.named_scope("gcn2"):
                # zero the AllReduce input (rows not covered by this core)
                zb = wp.tile([P, 128], BF16, tag="zb")
                nc.vector.memset(zb[:], 0.0)
                r0 = 0
                while r0 < BROWS:
                    r1 = min(r0 + P, BROWS)
                    nc.sync.dma_start(AR_in[r0:r1, :], zb[:r1 - r0, :])
                    r0 = r1
                for ws in GROUPS:
                    gcn_group(ws, T3_full, last=True)

                crec = wp.tile([P, 2], F32, tag="crec")
                nc.vector.reciprocal(crec[:], cnt_sb[:])
                for k, pps in enumerate((pool_ps0, pool_ps1)):
                    pooled = wp.tile([P, 128], BF16, tag="pooled")
                    nc.vector.tensor_scalar(pooled[:], pps[:],
                                            crec[:, k:k + 1], None, OP.mult)
                    nc.gpsimd.indirect_dma_start(
                        out=AR_in[:], out_offset=bass.IndirectOffsetOnAxis(
                            ap=scat_sb[:, k:k + 1], axis=0),
                        in_=pooled[:], in_offset=None)

            tc.strict_bb_all_engine_barrier()
            with nc.named_scope("ar"):
                nc.gpsimd.collective_compute(
                    "AllReduce", OP.add, replica_groups=RG,
                    ins=[AR_in.ap().opt()], outs=[AR_out.ap().opt()])
            tc.strict_bb_all_engine_barrier()

            # ================= fusion + classifier (BPC batches) ==========
            with nc.named_scope("fuse"):
                prow = sq.tile([P, 128], BF16, tag="prow")
                nc.gpsimd.indirect_dma_start(
                    out=prow[:], out_offset=None,
                    in_=AR_out[:],
                    in_offset=bass.IndirectOffsetOnAxis(
                        ap=rows_sb[:, 0:1], axis=0))
                prow32 = sq.tile([P, 128], F32, tag="prow32")
                nc.vector.tensor_copy(prow32[:], prow[:])
                fps_ = pp.tile([P, 512], F32, tag="ps")
                tp_ps = fps_[:, 0:128]
                ups = fps_[:, 128:256]
                vps = fps_[0:64, 256:384]
                ops_ = fps_[0:1, 384:512]
                nc.tensor.transpose(out=tp_ps, in_=prow32[:], identity=ident_sb[:])
                poolT = sq.tile([P, BPC], F32, tag="poolT")
                nc.vector.tensor_copy(poolT[:], tp_ps)

                nc.tensor.matmul(ups[:, :BPC], fusW0_sb[:], poolT[:],
                                 start=True, stop=False)
                nc.tensor.matmul(ups[:, :BPC], fusW1_sb[:], sT[:],
                                 start=False, stop=False)
                nc.tensor.matmul(ups[:, :BPC], fusb_sb[:], onesr_sb[:],
                                 start=False, stop=True)
                combT = sq.tile([P, BPC], F32, tag="combT")
                vlrelu(combT[:], ups[:, :BPC], "fu_a", sq, BPC)
                nc.tensor.matmul(vps[:, :BPC], cls1W_sb[:], combT[:],
                                 start=True, stop=False)
                nc.tensor.matmul(vps[:, :BPC], cls1b_sb[:], onesr_sb[:],
                                 start=False, stop=True)
                c1T = sq.tile([64, BPC], F32, tag="c1T")
                vlrelu(c1T[:], vps[:, :BPC], "fu_b", sq, BPC)
                nc.tensor.matmul(ops_[:, :BPC], cls3W_sb[:], c1T[:],
                                 start=True, stop=True)
                out_sb = sq.tile([1, BPC], F32, tag="out_sb")
                nc.vector.tensor_scalar(
                    out_sb[:], ops_[:, :BPC], cls3b_sb[0:1, 0:1], None, OP.add)
                nc.sync.dma_start(out[:], out_sb[:])

    nc.compile()
    return nc


# --------------------------------------------------------------------------
# entry point
# --------------------------------------------------------------------------

_CACHE = {}
LAST_RESULT = None


def kernel(**inputs):
    kh = hash((np.asarray(inputs['edge_index']).tobytes(),
               np.asarray(inputs['x']).tobytes()))
    if kh not in _CACHE:
        per_core, baked = host_prep(inputs)
        nc = build_nc(baked)
        _CACHE[kh] = (per_core, baked, nc)
    per_core, baked, nc = _CACHE[kh]

    wts = fold_weights(inputs)
    Bsz = baked['Bsz']
    BPC = Bsz // NC_CORES
    seq = np.asarray(inputs['seq_data'], np.float32)      # [B, 30, 20]
    seqT = np.ascontiguousarray(seq.transpose(1, 0, 2))   # [30, B, 20]
    shared = dict(
        iota256=np.ascontiguousarray(
            np.broadcast_to(np.arange(256, dtype=np.float32), (P, 256))),
        ident=np.eye(P, dtype=np.float32),
        ones4=np.ones((P, 4), np.float32),
        onesrow=np.ones((1, BPC), np.float32),
        **wts)
    in_maps = []
    for c in range(NC_CORES):
        m = dict(shared)
        m.update(per_core[c])
        m['xseq'] = np.ascontiguousarray(
            seqT[:, c * BPC:(c + 1) * BPC, :]).reshape(30, BPC * 20)
        in_maps.append(m)

    global LAST_RESULT
    res = run_bass_kernel_spmd(
        nc, in_maps, core_ids=list(range(NC_CORES)),
        trace=bool(os.environ.get('BASS_KERNEL_TRACE')))
    LAST_RESULT = res
    o = np.concatenate([res.results[c]["out"].reshape(-1)
                        for c in range(NC_CORES)]).reshape(Bsz, 1)
    return o.astype(np.float32)


# revision 20
# speedup vs baseline: 9.5836x; 1.2069x over previous
"""Trainium2 Bass kernel for nn_DeepCPP (GAT + 2xGCN graph branch, conv1d seq
branch, fusion MLP), SPMD over 8 NeuronCores.

Sharding/strategy:
 - Nodes partitioned across cores in natural order (keeps sorted `batch`
   contiguous per core); within a core nodes are sorted by in-degree so
   128-node windows have near-uniform max degree (node-major slot grids).
 - GAT attention logits per edge slot are computed with block-diagonal
   batched matmuls (8 slot-columns per matmul); exp(leakyrelu(a_s+a_d)) is
   factorized as max(P_e*T_d, R_e) with P=exp(a_s), R=exp(0.2*a_s),
   T=exp(0.8*a_d); the per-dst factor exp(-0.2*a_d) cancels in the softmax.
 - GCN layers gather 256B rows (dinv-prescaled h) from an AllGathered table
   with ONE batched indirect DMA per pair of 128-node windows; aggregation
   is a strided vector reduction.
 - Mean-pool via one-hot selection matmuls into persistent PSUM, AllReduce
   of partials; seq branch and fusion MLP are sharded by batch (128/core).
 - All loops fully unrolled (no hardware loops); non-Exp pointwise work runs
   on the Vector engine so the Scalar activation table stays loaded.
"""

import os
import sys

sys.path.insert(0, '/opt/trn_rl_repo')

import numpy as np
import ml_dtypes

import concourse.bass as bass
import concourse.mybir as mybir
import concourse.tile as tile
from concourse import bacc
from concourse.bass_utils import run_bass_kernel_spmd

F32 = mybir.dt.float32
BF16 = mybir.dt.bfloat16
I32 = mybir.dt.int32
AF = mybir.ActivationFunctionType
OP = mybir.AluOpType
AX = mybir.AxisListType

NC_CORES = 8
P = 128


# --------------------------------------------------------------------------
# host-side prep (layout/indexing only; cached per (x, edge_index))
# --------------------------------------------------------------------------

def host_prep(inputs):
    x = np.asarray(inputs['x'], np.float32)
    ei = np.asarray(inputs['edge_index'], np.int64)
    batch = np.asarray(inputs['batch'], np.int64)
    N = x.shape[0]
    Bsz = int(np.asarray(inputs['seq_data']).shape[0])
    assert N % NC_CORES == 0
    REAL = N // NC_CORES
    WPC = (REAL + P - 1) // P
    LOCAL = WPC * P
    NTOT = LOCAL * NC_CORES
    SENT = REAL if REAL < LOCAL else REAL - 1   # sentinel zero row in core 0

    src2 = np.concatenate([ei[0], np.arange(N)])
    dst2 = np.concatenate([ei[1], np.arange(N)])
    deg = np.bincount(dst2, minlength=N)

    local_rank = np.zeros(N, np.int64)
    rowid = np.zeros(N, np.int64)
    node_at = np.full((NC_CORES, LOCAL), -1, np.int64)
    for c in range(NC_CORES):
        ns = np.arange(c * REAL, (c + 1) * REAL)
        order = ns[np.argsort(-deg[ns], kind='stable')]
        local_rank[order] = np.arange(REAL)
        rowid[order] = c * LOCAL + np.arange(REAL)
        node_at[c, :REAL] = order

    # per-window max degree (shared across cores so the program is SPMD)
    Tw = np.ones(WPC, np.int64)
    for c in range(NC_CORES):
        first = node_at[c, ::P]
        for w in range(WPC):
            if first[w] >= 0:
                Tw[w] = max(Tw[w], deg[first[w]])
    T8w = ((Tw + 7) // 8) * 8
    gcol = np.concatenate([[0], np.cumsum(Tw)])       # GCN grid col offsets
    acol = np.concatenate([[0], np.cumsum(T8w)])      # GAT grid col offsets
    SLOTS = int(gcol[-1])
    SLOTS8 = int(acol[-1])
    GTOT = SLOTS8 // 8
    assert T8w.max() * 4 <= 160, "packed PSUM layout needs T8 <= 40"

    # GCN gather: R_ACC accumulate-rounds, windows grouped by round width
    R_ACC = int(os.environ.get('K_RACC', '1'))
    RCAP = 56
    TRw = [(int(t) + R_ACC - 1) // R_ACC for t in Tw]
    RGROUPS = []          # list of (ws, woffs, GTT)
    cur, woffs, acc = [], [], 0
    for w in range(WPC):
        if cur and acc + TRw[w] > RCAP:
            RGROUPS.append((cur, woffs, acc))
            cur, woffs, acc = [], [], 0
        cur.append(w)
        woffs.append(acc)
        acc += TRw[w]
    if cur:
        RGROUPS.append((cur, woffs, acc))
    roff = []
    off = 0
    for (_, _, GTT) in RGROUPS:
        roff.append(off)
        off += R_ACC * GTT
    SLOTSR = off

    e_dst = rowid[dst2]
    e_src = src2
    o = np.argsort(e_dst, kind='stable')
    e_dst = e_dst[o]
    e_src = e_src[o]
    grp_start = np.searchsorted(e_dst, np.arange(NTOT), side='left')
    t_of = np.arange(len(e_dst)) - grp_start[e_dst]
    c_of = e_dst // LOCAL
    lrow = e_dst % LOCAL
    w_of = lrow // P
    p_of = lrow % P
    assert (t_of < Tw[w_of]).all()
    col_g = gcol[w_of] + t_of
    col_a = acol[w_of] + t_of

    slot_node_g = np.full((NC_CORES, P, SLOTS), N, np.int64)
    slot_node_g[c_of, p_of, col_g] = e_src
    slot_node_a = np.full((NC_CORES, P, SLOTS8), N, np.int64)
    slot_node_a[c_of, p_of, col_a] = e_src

    x_pad = np.vstack([x, np.zeros((1, x.shape[1]), np.float32)])
    rowid_pad = np.concatenate([rowid, [SENT]]).astype(np.int32)

    cnt = np.bincount(batch, minlength=Bsz).astype(np.float32)
    per_core = []
    for c in range(NC_CORES):
        sna = slot_node_a[c]                       # [P, SLOTS8], N = pad
        xs = x_pad[sna]                            # [P, SLOTS8, 9]
        xslots = np.empty((P, SLOTS8 * 9), np.float32)
        for w in range(WPC):
            a0, T8 = int(acol[w]), int(T8w[w])
            xslots[:, a0 * 9:(a0 + T8) * 9] = np.ascontiguousarray(
                xs[:, a0:a0 + T8, :].transpose(0, 2, 1)).reshape(P, T8 * 9)
        xslots = xslots.astype(ml_dtypes.bfloat16)
        xTl = np.zeros((16, SLOTS8, P), np.float32)
        xTl[0:9] = xs.transpose(2, 1, 0)
        xTl[9] = (sna.T == N).astype(np.float32)   # pad flag
        # [16j+f, (group)*128 + p] = xTl[f, 8*group+j, p]
        xgrp = np.ascontiguousarray(
            xTl.reshape(16, GTOT, 8, P).transpose(2, 0, 1, 3)
               .reshape(128, GTOT * P)).astype(ml_dtypes.bfloat16)
        srg = rowid_pad[slot_node_g[c]]            # [P, SLOTS]
        srcrowR = np.full((P, SLOTSR), SENT, np.int32)
        for g, (ws, wo, GTT) in enumerate(RGROUPS):
            for r in range(R_ACC):
                for w, woff in zip(ws, wo):
                    t0 = r * TRw[w]
                    t1 = min(t0 + TRw[w], int(Tw[w]))
                    if t1 > t0:
                        col = roff[g] + r * GTT + woff
                        srcrowR[:, col:col + (t1 - t0)] = \
                            srg[:, gcol[w] + t0:gcol[w] + t1]

        valid = node_at[c] >= 0
        xloc = np.zeros((9, LOCAL), np.float32)
        xloc[0:9, valid] = x[node_at[c][valid]].T

        dinv = np.zeros(LOCAL, np.float32)
        dinv[valid] = 1.0 / np.sqrt(deg[node_at[c][valid]])
        dinv_w = np.ascontiguousarray(dinv.reshape(WPC, P).T)

        bl = np.full(LOCAL, -1.0, np.float32)
        b_base = int(batch[c * REAL])
        bl[valid] = batch[node_at[c][valid]] - b_base
        assert bl.max() < 256, "batch window exceeded 256"
        bl_w = np.ascontiguousarray(bl.reshape(WPC, P).T)

        cnt_l = np.ones(256, np.float32)
        hi = min(256, Bsz - b_base)
        cnt_l[:hi] = np.maximum(cnt[b_base:b_base + hi], 1.0)
        scatv = np.zeros(256, np.int32)
        for j in range(256):
            scatv[j] = b_base + j if b_base + j < Bsz else Bsz + (j % 8)

        per_core.append(dict(
            xslots=xslots, xgrp=xgrp, srcrow=srcrowR,
            xlocT=xloc, dinv_w=dinv_w, bl_w=bl_w,
            cnt_l=np.ascontiguousarray(cnt_l.reshape(2, P).T),
            scat=np.ascontiguousarray(scatv.reshape(2, P).T),
            rows128=(c * P + np.arange(P, dtype=np.int32)).reshape(P, 1),
        ))

    baked = dict(N=N, REAL=REAL, WPC=WPC, LOCAL=LOCAL, NTOT=NTOT,
                 SLOTS=SLOTS, SLOTS8=SLOTS8, GTOT=GTOT, SLOTSR=SLOTSR,
                 R_ACC=R_ACC, RCAP=RCAP, TRw=TRw,
                 RGROUPS=[(list(ws), list(wo), int(g)) for ws, wo, g in RGROUPS],
                 roff=roff,
                 Tw=[int(t) for t in Tw], T8w=[int(t) for t in T8w],
                 gcol=[int(t) for t in gcol], acol=[int(t) for t in acol],
                 Bsz=Bsz)
    return per_core, baked


def fold_weights(inputs):
    w = {k: np.asarray(v, np.float32) for k, v in inputs.items()
         if k not in ('x', 'edge_index', 'batch')}
    H, C = 4, 32
    Wg = w['W_gat']
    was = np.einsum('fhc,hc->fh', Wg.reshape(9, H, C), w['att_src'])
    wad = np.einsum('fhc,hc->fh', Wg.reshape(9, H, C), w['att_dst'])
    was_aug = np.zeros((16, 4), np.float32)
    was_aug[0:9] = was
    was_aug[9] = -80.0
    wad_aug = np.zeros((9, 4), np.float32)
    wad_aug[0:9] = wad
    # block-diagonal was for 8 slot-columns per matmul
    wasD = np.zeros((128, 32), np.float32)
    for j in range(8):
        wasD[16 * j:16 * j + 16, 4 * j:4 * j + 4] = was_aug
    # [40,128] compact GAT weight: rows (10h+f) f<9 = W_gat, f=9 = bias
    wg40 = np.zeros((40, 128), np.float32)
    for h in range(H):
        wg40[h * 9:h * 9 + 9, h * 32:(h + 1) * 32] = Wg[:, h * 32:(h + 1) * 32]
        wg40[36 + h, h * 32:(h + 1) * 32] = w['b_gat'][h * 32:(h + 1) * 32]
    W3_aug = np.zeros((65, 128), np.float32)
    W3_aug[0:64] = w['W3']
    W3_aug[64] = w['b3']

    def fold(cw, cb, g, be, m, v):
        s = g / np.sqrt(v + 1e-5)
        return cw * s[:, None, None], (cb - m) * s + be

    c1w, c1b = fold(w['conv1_w'], w['conv1_b'], w['bn1_g'], w['bn1_b'],
                    w['bn1_m'], w['bn1_v'])
    c2w, c2b = fold(w['conv2_w'], w['conv2_b'], w['bn2_g'], w['bn2_b'],
                    w['bn2_m'], w['bn2_v'])
    # [cin, k, cout] flattened so slice k -> [cin, cout]
    w1k = np.ascontiguousarray(c1w.transpose(1, 2, 0)).reshape(30, 3 * 64)
    w2k = np.ascontiguousarray(c2w.transpose(1, 2, 0)).reshape(64, 3 * 64)
    fc1_Wr = np.ascontiguousarray(w['fc1_W'].reshape(64, 16 * 64))

    return dict(
        wasD=wasD.astype(ml_dtypes.bfloat16), wad_aug=wad_aug, wg40=wg40,
        W2=w['W2'], b2row=np.ascontiguousarray(np.broadcast_to(w['b2'], (P, 64))),
        W3_aug=W3_aug,
        w1k=w1k, b1=np.ascontiguousarray(c1b.reshape(64, 1)),
        w2k=w2k, b2c=np.ascontiguousarray(c2b.reshape(64, 1)),
        fc1_Wr=fc1_Wr, fc1_b=np.ascontiguousarray(w['fc1_b'].reshape(64, 1)),
        fus_W0=np.ascontiguousarray(w['fus_W'][0:128]),
        fus_W1=np.ascontiguousarray(w['fus_W'][128:192]),
        fus_b=np.ascontiguousarray(w['fus_b'].reshape(1, 128)),
        cls1_W=w['cls1_W'],
        cls1_b=np.ascontiguousarray(w['cls1_b'].reshape(1, 64)),
        cls3_W=w['cls3_W'],
        cls3_b_t=np.array([[float(w['cls3_b'][0])]], np.float32),
    )


# --------------------------------------------------------------------------
# device program
# --------------------------------------------------------------------------

def build_nc(baked):
    WPC, LOCAL, NTOT = baked['WPC'], baked['LOCAL'], baked['NTOT']
    SLOTS, SLOTS8, GTOT = baked['SLOTS'], baked['SLOTS8'], baked['GTOT']
    Tw, T8w, gcol, acol = baked['Tw'], baked['T8w'], baked['gcol'], baked['acol']
    SLOTSR, R_ACC, RCAP = baked['SLOTSR'], baked['R_ACC'], baked['RCAP']
    TRw, RGROUPS, roff = baked['TRw'], baked['RGROUPS'], baked['roff']
    Bsz = baked['Bsz']
    BROWS = Bsz + 8
    BPC = Bsz // NC_CORES                      # batches per core (fusion/seq)
    RG = [list(range(NC_CORES))]
    T8MAX = max(T8w)
    GMAX = T8MAX // 8

    nc = bacc.Bacc("TRN2", target_bir_lowering=False, debug=False,
                   num_devices=NC_CORES)

    def inp(name, shape, dt=F32):
        return nc.dram_tensor(name, shape, dt, kind="ExternalInput")

    xgrp = inp("xgrp", [128, GTOT * P], BF16)
    xslots = inp("xslots", [P, SLOTS8 * 9], BF16)
    srcrow = inp("srcrow", [P, SLOTSR], I32)
    xlocT = inp("xlocT", [9, LOCAL])
    dinv_w = inp("dinv_w", [P, WPC])
    bl_w = inp("bl_w", [P, WPC])
    cnt_l = inp("cnt_l", [P, 2])
    scat = inp("scat", [P, 2], I32)
    rows128 = inp("rows128", [P, 1], I32)
    iota256 = inp("iota256", [P, 256])
    ident = inp("ident", [P, P])
    ones4 = inp("ones4", [P, 4])
    onesrow = inp("onesrow", [1, BPC])
    wasD = inp("wasD", [128, 32], BF16)
    wad_aug = inp("wad_aug", [9, 4])
    wg40 = inp("wg40", [40, 128])
    W2 = inp("W2", [128, 64])
    b2row = inp("b2row", [P, 64])
    W3_aug = inp("W3_aug", [65, 128])
    w1k = inp("w1k", [30, 3 * 64])
    b1 = inp("b1", [64, 1])
    w2k = inp("w2k", [64, 3 * 64])
    b2c = inp("b2c", [64, 1])
    fc1_Wr = inp("fc1_Wr", [64, 16 * 64])
    fc1_b = inp("fc1_b", [64, 1])
    fus_W0 = inp("fus_W0", [128, 128])
    fus_W1 = inp("fus_W1", [64, 128])
    fus_b = inp("fus_b", [1, 128])
    cls1_W = inp("cls1_W", [128, 64])
    cls1_b = inp("cls1_b", [1, 64])
    cls3_W = inp("cls3_W", [64, 1])
    cls3_b_t = inp("cls3_b_t", [1, 1])
    xseq = inp("xseq", [30, BPC * 20])

    out = nc.dram_tensor("out", [1, BPC], F32, kind="ExternalOutput")

    T2_local = nc.dram_tensor("T2_local", [LOCAL, 64], BF16)
    T2_full = nc.dram_tensor("T2_full", [NTOT, 64], BF16)
    T3_local = nc.dram_tensor("T3_local", [LOCAL, 64], BF16)
    T3_full = nc.dram_tensor("T3_full", [NTOT, 64], BF16)
    AR_in = nc.dram_tensor("AR_in", [BROWS, 128], BF16)
    AR_out = nc.dram_tensor("AR_out", [BROWS, 128], BF16)

    with tile.TileContext(nc) as tc:
        with tc.tile_pool(name="const", bufs=1) as cp, \
             tc.tile_pool(name="work", bufs=3) as wp, \
             tc.tile_pool(name="gath", bufs=3) as g2p, \
             tc.tile_pool(name="gat", bufs=3) as gp, \
             tc.tile_pool(name="psum", bufs=4, space="PSUM") as pp, \
             tc.tile_pool(name="spsum", bufs=2, space="PSUM") as spp, \
             tc.tile_pool(name="ppool", bufs=1, space="PSUM") as ppool, \
             tc.tile_pool(name="seq", bufs=1) as sq:

            def c_load(ap, shape, dt=F32):
                t = cp.tile(shape, dt, tag=f"c_{ap.name}")
                nc.sync.dma_start(t[:], ap[:])
                return t

            srcrow_sb = c_load(srcrow, [P, SLOTSR], I32)
            dinv_sb = c_load(dinv_w, [P, WPC])
            bl_sb = c_load(bl_w, [P, WPC])
            cnt_sb = c_load(cnt_l, [P, 2])
            scat_sb = c_load(scat, [P, 2], I32)
            rows_sb = c_load(rows128, [P, 1], I32)
            iota_sb = c_load(iota256, [P, 256])
            ident_sb = c_load(ident, [P, P])
            ones4_sb = c_load(ones4, [P, 4])
            onesr_sb = c_load(onesrow, [1, BPC])
            wasD_sb = c_load(wasD, [128, 32], BF16)
            wad_sb = c_load(wad_aug, [9, 4])
            wg40_sb = c_load(wg40, [40, 128])
            W2_sb = c_load(W2, [128, 64])
            b2row_sb = c_load(b2row, [P, 64])
            W3_sb = c_load(W3_aug, [65, 128])
            xloc_sb = c_load(xlocT, [9, LOCAL])
            w1_sb = c_load(w1k, [30, 3 * 64])
            b1_sb = c_load(b1, [64, 1])
            w2_sb = c_load(w2k, [64, 3 * 64])
            b2c_sb = c_load(b2c, [64, 1])
            fc1_sb = c_load(fc1_Wr, [64, 16 * 64])
            fc1b_sb = c_load(fc1_b, [64, 1])
            fusW0_sb = c_load(fus_W0, [128, 128])
            fusW1_sb = c_load(fus_W1, [64, 128])
            fusb_sb = c_load(fus_b, [1, 128])
            cls1W_sb = c_load(cls1_W, [128, 64])
            cls1b_sb = c_load(cls1_b, [1, 64])
            cls3W_sb = c_load(cls3_W, [64, 1])
            cls3b_sb = c_load(cls3_b_t, [1, 1])

            # persistent pooling PSUM, zeroed via K=1 matmul (sets has_written)
            pool_ps0 = ppool.tile([P, P], F32, tag="pool0")
            pool_ps1 = ppool.tile([P, P], F32, tag="pool1")
            zrow = cp.tile([1, P], F32)
            nc.vector.memset(zrow[:], 0.0)
            nc.tensor.matmul(pool_ps0[:], zrow[:], zrow[:], start=True, stop=True)
            nc.tensor.matmul(pool_ps1[:], zrow[:], zrow[:], start=True, stop=True)

            def vlrelu(dst, src, tmp_tag, pool, n):
                """dst = leakyrelu(src, 0.01) on the vector engine."""
                t = pool.tile([src.shape[0], n], F32, tag=tmp_tag)
                nc.vector.tensor_scalar(t[:], src, 0.01, None, OP.mult)
                nc.vector.tensor_tensor(dst, src, t[:], op=OP.max)

            # ================= seq branch (BPC batches, overlaps GAT) =====
            xsf = sq.tile([30, BPC * 20], F32, tag="xsf")
            nc.sync.dma_start(xsf[:], xseq[:])
            s1_sb = sq.tile([64, BPC * 18], F32, tag="s1")
            CH1 = 28
            for ci in range((BPC + CH1 - 1) // CH1):
                b0 = ci * CH1
                bn = min(CH1, BPC - b0)
                cps = spp.tile([64, 512], F32, tag="sps")
                for k in range(3):
                    nc.tensor.matmul(
                        cps[:, :bn * 18],
                        w1_sb[:, 64 * k:64 * (k + 1)],
                        xsf[:].rearrange("c (b t) -> c b t", t=20)[:, b0:b0 + bn, k:k + 18],
                        start=(k == 0), stop=(k == 2))
                t0 = sq.tile([64, CH1 * 18], F32, tag="sq_t0")
                t1 = sq.tile([64, CH1 * 18], F32, tag="sq_t1")
                nc.vector.tensor_scalar(t0[:, :bn * 18], cps[:, :bn * 18],
                                        b1_sb[:, 0:1], None, OP.add)
                nc.vector.tensor_scalar(t1[:, :bn * 18], cps[:, :bn * 18],
                                        b1_sb[:, 0:1], 0.01, OP.add, OP.mult)
                nc.vector.tensor_tensor(s1_sb[:, b0 * 18:(b0 + bn) * 18],
                                        t0[:, :bn * 18], t1[:, :bn * 18],
                                        op=OP.max)
            s2_sb = sq.tile([64, BPC * 16], F32, tag="s2")
            CH2 = 32
            for ci in range((BPC + CH2 - 1) // CH2):
                b0 = ci * CH2
                bn = min(CH2, BPC - b0)
                cps2 = spp.tile([64, 512], F32, tag="sps")
                for k in range(3):
                    nc.tensor.matmul(
                        cps2[:, :bn * 16],
                        w2_sb[:, 64 * k:64 * (k + 1)],
                        s1_sb[:].rearrange("c (b t) -> c b t", t=18)[:, b0:b0 + bn, k:k + 16],
                        start=(k == 0), stop=(k == 2))
                t0 = sq.tile([64, CH2 * 16], F32, tag="sq_u0")
                t1 = sq.tile([64, CH2 * 16], F32, tag="sq_u1")
                nc.vector.tensor_scalar(t0[:, :bn * 16], cps2[:, :bn * 16],
                                        b2c_sb[:, 0:1], None, OP.add)
                nc.vector.tensor_scalar(t1[:, :bn * 16], cps2[:, :bn * 16],
                                        b2c_sb[:, 0:1], 0.01, OP.add, OP.mult)
                nc.vector.tensor_tensor(s2_sb[:, b0 * 16:(b0 + bn) * 16],
                                        t0[:, :bn * 16], t1[:, :bn * 16],
                                        op=OP.max)
            fps = spp.tile([64, 512], F32, tag="sps")
            for t in range(16):
                nc.tensor.matmul(
                    fps[:, :BPC],
                    fc1_sb[:].rearrange("c (t j) -> c t j", j=64)[:, t, :],
                    s2_sb[:].rearrange("c (b t) -> c b t", t=16)[:, :, t:t + 1],
                    start=(t == 0), stop=(t == 15))
            sT = sq.tile([64, BPC], F32, tag="sT")
            nc.vector.tensor_scalar(sT[:], fps[:, :BPC], fc1b_sb[:, 0:1], None, OP.add)

            # ================= GAT =================
            def gat_body(w):
                T8 = T8w[w]
                G = T8 // 8
                gbase = acol[w] // 8
                ps = pp.tile([P, 512], F32, tag="ps")   # one PSUM bank/window
                ad_ps = ps[:, 0:4]
                as_ps = ps[:, 32:32 + 4 * T8MAX]
                zT_ps = ps[0:40, 192:320]
                g1_ps = ps[:, 320:448]
                h2_ps = ps[:, 448:512]
                nc.tensor.matmul(ad_ps, xloc_sb[:, bass.ds(w * P, P)],
                                 wad_sb[:], start=True, stop=True)
                T_d = gp.tile([P, 4], BF16, tag="Td")
                nc.scalar.activation(T_d[:], ad_ps, AF.Exp, scale=0.8)

                xg = gp.tile([128, GMAX * P], BF16, tag="xg")
                nc.sync.dma_start(xg[:, :G * P],
                                  xgrp[:, bass.ds(gbase * P, G * P)])
                for g in range(G):
                    nc.tensor.matmul(as_ps[:, 32 * g:32 * g + 32],
                                     xg[:, P * g:P * (g + 1)], wasD_sb[:],
                                     start=True, stop=True)
                Pt = gp.tile([P, 4 * T8MAX], BF16, tag="Pt")
                Rt = gp.tile([P, 4 * T8MAX], BF16, tag="Rt")
                nc.scalar.activation(Pt[:, :4 * T8], as_ps[:, :4 * T8],
                                     AF.Exp, scale=1.0)
                nc.scalar.activation(Rt[:, :4 * T8], as_ps[:, :4 * T8],
                                     AF.Exp, scale=0.2)

                EX = gp.tile([P, 4 * T8MAX], BF16, tag="EX")
                nc.vector.tensor_tensor(
                    EX[:, :4 * T8].rearrange("p (t h) -> p t h", h=4),
                    Pt[:, :4 * T8].rearrange("p (t h) -> p t h", h=4),
                    T_d[:, None, :].to_broadcast([P, T8, 4]),
                    op=OP.mult)
                nc.vector.tensor_tensor(EX[:, :4 * T8], EX[:, :4 * T8],
                                        Rt[:, :4 * T8], op=OP.max)
                S4 = gp.tile([P, 4], F32, tag="S4")
                nc.vector.tensor_reduce(
                    S4[:, :, None],
                    EX[:, :4 * T8].rearrange("p (t h) -> p h t", h=4),
                    axis=AX.X, op=OP.add)
                nc.vector.reciprocal(S4[:], S4[:])
                S4b = gp.tile([P, 4], BF16, tag="S4b")
                nc.vector.tensor_copy(S4b[:], S4[:])
                # AL in (h, t) layout -> ZR/zaug reduce become stride-1
                AL = gp.tile([P, 4 * T8MAX], BF16, tag="AL")
                nc.vector.tensor_tensor(
                    AL[:, :4 * T8].rearrange("p (h t) -> p t h", t=T8),
                    EX[:, :4 * T8].rearrange("p (t h) -> p t h", h=4),
                    S4b[:, None, :].to_broadcast([P, T8, 4]),
                    op=OP.mult)

                XS = gp.tile([P, 9 * T8MAX], BF16, tag="XS")
                nc.sync.dma_start(XS[:, :9 * T8],
                                  xslots[:, bass.ds(acol[w] * 9, T8 * 9)])
                ZR = gp.tile([P, 36 * T8MAX], BF16, tag="ZR")
                nc.vector.tensor_tensor(
                    ZR[:, :36 * T8].rearrange("p (h f t) -> p h f t", f=9, t=T8),
                    XS[:, :9 * T8].rearrange("p (f t) -> p f t", t=T8)[:, None, :, :]
                        .to_broadcast([P, 4, 9, T8]),
                    AL[:, :4 * T8].rearrange("p (h t) -> p h t", t=T8)[:, :, None, :]
                        .to_broadcast([P, 4, 9, T8]),
                    op=OP.mult)
                zaug = gp.tile([P, 40], F32, tag="zaug")
                nc.vector.tensor_copy(zaug[:, 36:40], ones4_sb[:])
                nc.vector.tensor_reduce(
                    zaug[:, 0:36][:, :, None],
                    ZR[:, :36 * T8].rearrange("p (q t) -> p q t", t=T8),
                    axis=AX.X, op=OP.add)
                nc.tensor.transpose(out=zT_ps, in_=zaug[:], identity=ident_sb[:])
                zT = gp.tile([40, P], F32, tag="zTs")
                nc.vector.tensor_copy(zT[:], zT_ps)
                nc.tensor.matmul(g1_ps, zT[:], wg40_sb[:],
                                 start=True, stop=True)
                g1T = gp.tile([P, P], F32, tag="g1T")
                vlrelu(g1T[:], g1_ps, "g1a", gp, P)
                nc.tensor.matmul(h2_ps, g1T[:], W2_sb[:], start=True, stop=True)
                T2s = gp.tile([P, 64], BF16, tag="T2s")
                nc.vector.tensor_scalar(T2s[:], h2_ps,
                                        dinv_sb[:, bass.ds(w, 1)], None, OP.mult)
                nc.sync.dma_start(T2_local[bass.ds(w * P, P), :], T2s[:])

            with nc.named_scope("gat"):
                for w in range(WPC):
                    gat_body(w)

            tc.strict_bb_all_engine_barrier()
            with nc.named_scope("ag1"):
                nc.gpsimd.collective_compute(
                    "AllGather", OP.bypass, replica_groups=RG,
                    ins=[T2_local.ap().opt()], outs=[T2_full.ap().opt()])
            tc.strict_bb_all_engine_barrier()

            # ================= GCN layers =================
            def gcn_group(gi, table, last):
                ws, wo, GTT = RGROUPS[gi]
                G2 = g2p.tile([P, RCAP * 64], BF16, tag="G2")
                for r in range(R_ACC):
                    nc.gpsimd.indirect_dma_start(
                        out=G2[:, :GTT * 64], out_offset=None,
                        in_=table[:],
                        in_offset=bass.IndirectOffsetOnAxis(
                            ap=srcrow_sb[:, roff[gi] + r * GTT:
                                         roff[gi] + r * GTT + GTT], axis=0),
                        compute_op=(OP.bypass if r == 0 else OP.add))
                def tree_sum(woff, T):
                    """In-place halving tree over window slot columns; returns
                    the [P, 64] bf16 slice holding the sum."""
                    Tc = T
                    while Tc > 1:
                        if Tc & 1:
                            nc.vector.tensor_tensor(
                                G2[:, woff * 64:(woff + 1) * 64],
                                G2[:, woff * 64:(woff + 1) * 64],
                                G2[:, (woff + Tc - 1) * 64:(woff + Tc) * 64],
                                op=OP.add)
                            Tc -= 1
                        H = Tc // 2
                        nc.vector.tensor_tensor(
                            G2[:, woff * 64:(woff + H) * 64],
                            G2[:, woff * 64:(woff + H) * 64],
                            G2[:, (woff + H) * 64:(woff + 2 * H) * 64],
                            op=OP.add)
                        Tc = H
                    return G2[:, woff * 64:(woff + 1) * 64]

                for w, woff in zip(ws, wo):
                    TR = TRw[w]
                    if not last:
                        zs = tree_sum(woff, TR)
                        z = wp.tile([P, 64], F32, tag="z")
                        nc.vector.tensor_scalar(
                            z[:], zs, dinv_sb[:, bass.ds(w, 1)], None, OP.mult)
                        nc.vector.tensor_tensor(z[:], z[:], b2row_sb[:], op=OP.add)
                        T3s = wp.tile([P, 64], BF16, tag="T3s")
                        nc.scalar.activation(T3s[:], z[:], AF.Lrelu, alpha=0.01,
                                             scale=dinv_sb[:, bass.ds(w, 1)])
                        nc.sync.dma_start(T3_local[bass.ds(w * P, P), :], T3s[:])
                    else:
                        zs = tree_sum(woff, TR)
                        z3s = wp.tile([P, 65], F32, tag="z3s")
                        nc.vector.tensor_scalar(
                            z3s[:, 0:64], zs,
                            dinv_sb[:, bass.ds(w, 1)], None, OP.mult)
                        nc.vector.tensor_copy(z3s[:, 64:65], ones4_sb[:, 0:1])
                        ps2 = pp.tile([P, 512], F32, tag="ps")
                        z3T_ps = ps2[0:65, 0:128]
                        g3_ps = ps2[:, 128:256]
                        nc.tensor.transpose(out=z3T_ps, in_=z3s[:],
                                            identity=ident_sb[:])
                        z3T = wp.tile([65, P], F32, tag="z3Ts")
                        nc.vector.tensor_copy(z3T[:], z3T_ps)
                        nc.tensor.matmul(g3_ps, z3T[:], W3_sb[:],
                                         start=True, stop=True)
                        g3 = wp.tile([P, P], F32, tag="g3s")
                        nc.scalar.activation(g3[:], g3_ps, AF.Lrelu, alpha=0.01)
                        Mp = wp.tile([P, 256], F32, tag="Mp")
                        nc.vector.tensor_scalar(
                            Mp[:], iota_sb[:], bl_sb[:, bass.ds(w, 1)], None,
                            OP.is_equal)
                        nc.tensor.matmul(pool_ps0[:], Mp[:, 0:128], g3[:],
                                         start=False, stop=True)
                        nc.tensor.matmul(pool_ps1[:], Mp[:, 128:256], g3[:],
                                         start=False, stop=True)

            with nc.named_scope("gcn1"):
                for gi in range(len(RGROUPS)):
                    gcn_group(gi, T2_full, last=False)

            tc.strict_bb_all_engine_barrier()
            with nc.named_scope("ag2"):
                nc.gpsimd.collective_compute(
                    "AllGather", OP.bypass, replica_groups=RG,
                    ins=[T3_local.ap().opt()], outs=[T3_full.ap().opt()])
            tc.strict_bb_all_engine_barrier()

            with nc.named_scope("gcn2"):
                # zero the AllReduce input (rows not covered by this core)
                zb = wp.tile([P, 128], BF16, tag="zb")
                nc.vector.memset(zb[:], 0.0)
                r0 = 0
                while r0 < BROWS:
                    r1 = min(r0 + P, BROWS)
                    nc.sync.dma_start(AR_in[r0:r1, :], zb[:r1 - r0, :])
                    r0 = r1
                for gi in range(len(RGROUPS)):
                    gcn_group(gi, T3_full, last=True)

                crec = wp.tile([P, 2], F32, tag="crec")
                nc.vector.reciprocal(crec[:], cnt_sb[:])
                for k, pps in enumerate((pool_ps0, pool_ps1)):
                    pooled = wp.tile([P, 128], BF16, tag="pooled")
                    nc.vector.tensor_scalar(pooled[:], pps[:],
                                            crec[:, k:k + 1], None, OP.mult)
                    nc.gpsimd.indirect_dma_start(
                        out=AR_in[:], out_offset=bass.IndirectOffsetOnAxis(
                            ap=scat_sb[:, k:k + 1], axis=0),
                        in_=pooled[:], in_offset=None)

            tc.strict_bb_all_engine_barrier()
            with nc.named_scope("ar"):
                nc.gpsimd.collective_compute(
                    "AllReduce", OP.add, replica_groups=RG,
                    ins=[AR_in.ap().opt()], outs=[AR_out.ap().opt()])
            tc.strict_bb_all_engine_barrier()

            # ================= fusion + classifier (BPC batches) ==========
            with nc.named_scope("fuse"):
                prow = sq.tile([P, 128], BF16, tag="prow")
                nc.gpsimd.indirect_dma_start(
                    out=prow[:], out_offset=None,
                    in_=AR_out[:],
                    in_offset=bass.IndirectOffsetOnAxis(
                        ap=rows_sb[:, 0:1], axis=0))
                prow32 = sq.tile([P, 128], F32, tag="prow32")
                nc.vector.tensor_copy(prow32[:], prow[:])
                fps_ = pp.tile([P, 512], F32, tag="ps")
                tp_ps = fps_[:, 0:128]
                ups = fps_[:, 128:256]
                vps = fps_[0:64, 256:384]
                ops_ = fps_[0:1, 384:512]
                nc.tensor.transpose(out=tp_ps, in_=prow32[:], identity=ident_sb[:])
                poolT = sq.tile([P, BPC], F32, tag="poolT")
                nc.vector.tensor_copy(poolT[:], tp_ps)

                nc.tensor.matmul(ups[:, :BPC], fusW0_sb[:], poolT[:],
                                 start=True, stop=False)
                nc.tensor.matmul(ups[:, :BPC], fusW1_sb[:], sT[:],
                                 start=False, stop=False)
                nc.tensor.matmul(ups[:, :BPC], fusb_sb[:], onesr_sb[:],
                                 start=False, stop=True)
                combT = sq.tile([P, BPC], F32, tag="combT")
                vlrelu(combT[:], ups[:, :BPC], "fu_a", sq, BPC)
                nc.tensor.matmul(vps[:, :BPC], cls1W_sb[:], combT[:],
                                 start=True, stop=False)
                nc.tensor.matmul(vps[:, :BPC], cls1b_sb[:], onesr_sb[:],
                                 start=False, stop=True)
                c1T = sq.tile([64, BPC], F32, tag="c1T")
                vlrelu(c1T[:], vps[:, :BPC], "fu_b", sq, BPC)
                nc.tensor.matmul(ops_[:, :BPC], cls3W_sb[:], c1T[:],
                                 start=True, stop=True)
                out_sb = sq.tile([1, BPC], F32, tag="out_sb")
                nc.vector.tensor_scalar(
                    out_sb[:], ops_[:, :BPC], cls3b_sb[0:1, 0:1], None, OP.add)
                nc.sync.dma_start(out[:], out_sb[:])

    nc.compile()
    return nc


# --------------------------------------------------------------------------
# entry point
# --------------------------------------------------------------------------

_CACHE = {}
LAST_RESULT = None


def kernel(**inputs):
    kh = hash((np.asarray(inputs['edge_index']).tobytes(),
               np.asarray(inputs['x']).tobytes()))
    if kh not in _CACHE:
        per_core, baked = host_prep(inputs)
        nc = build_nc(baked)
        _CACHE[kh] = (per_core, baked, nc)
    per_core, baked, nc = _CACHE[kh]

    wts = fold_weights(inputs)
    Bsz = baked['Bsz']
    BPC = Bsz // NC_CORES
    seq = np.asarray(inputs['seq_data'], np.float32)      # [B, 30, 20]
    seqT = np.ascontiguousarray(seq.transpose(1, 0, 2))   # [30, B, 20]
    shared = dict(
        iota256=np.ascontiguousarray(
            np.broadcast_to(np.arange(256, dtype=np.float32), (P, 256))),
        ident=np.eye(P, dtype=np.float32),
        ones4=np.ones((P, 4), np.float32),
        onesrow=np.ones((1, BPC), np.float32),
        **wts)
    in_maps = []
    for c in range(NC_CORES):
        m = dict(shared)
        m.update(per_core[c])
        m['xseq'] = np.ascontiguousarray(
            seqT[:, c * BPC:(c + 1) * BPC, :]).reshape(30, BPC * 20)
        in_maps.append(m)

    global LAST_RESULT
    res = run_bass_kernel_spmd(
        nc, in_maps, core_ids=list(range(NC_CORES)),
        trace=bool(os.environ.get('BASS_KERNEL_TRACE')))
    LAST_RESULT = res
    o = np.concatenate([res.results[c]["out"].reshape(-1)
                        for c in range(NC_CORES)]).reshape(Bsz, 1)
    return o.astype(np.float32)


# revision 27
# speedup vs baseline: 10.2089x; 1.0652x over previous
"""Trainium2 Bass kernel for nn_DeepCPP (GAT + 2xGCN graph branch, conv1d seq
branch, fusion MLP), SPMD over 8 NeuronCores.

Sharding/strategy:
 - Nodes partitioned across cores in natural order (keeps sorted `batch`
   contiguous per core); within a core nodes are sorted by in-degree so
   128-node windows have near-uniform max degree (node-major slot grids).
 - GAT attention logits per edge slot are computed with block-diagonal
   batched matmuls (8 slot-columns per matmul); exp(leakyrelu(a_s+a_d)) is
   factorized as max(P_e*T_d, R_e) with P=exp(a_s), R=exp(0.2*a_s),
   T=exp(0.8*a_d); the per-dst factor exp(-0.2*a_d) cancels in the softmax.
 - GCN layers gather 256B rows (dinv-prescaled h) from an AllGathered table
   with ONE batched indirect DMA per pair of 128-node windows; aggregation
   is a strided vector reduction.
 - Mean-pool via one-hot selection matmuls into persistent PSUM, AllReduce
   of partials; seq branch and fusion MLP are sharded by batch (128/core).
 - All loops fully unrolled (no hardware loops); non-Exp pointwise work runs
   on the Vector engine so the Scalar activation table stays loaded.
"""

import os
import sys

sys.path.insert(0, '/opt/trn_rl_repo')

import numpy as np
import ml_dtypes

import concourse.bass as bass
import concourse.mybir as mybir
import concourse.tile as tile
from concourse import bacc
from concourse.bass_utils import run_bass_kernel_spmd

F32 = mybir.dt.float32
BF16 = mybir.dt.bfloat16
I32 = mybir.dt.int32
AF = mybir.ActivationFunctionType
OP = mybir.AluOpType
AX = mybir.AxisListType

NC_CORES = 8
P = 128


# --------------------------------------------------------------------------
# host-side prep (layout/indexing only; cached per (x, edge_index))
# --------------------------------------------------------------------------

def host_prep(inputs):
    x = np.asarray(inputs['x'], np.float32)
    ei = np.asarray(inputs['edge_index'], np.int64)
    batch = np.asarray(inputs['batch'], np.int64)
    N = x.shape[0]
    Bsz = int(np.asarray(inputs['seq_data']).shape[0])
    assert N % NC_CORES == 0
    REAL = N // NC_CORES
    WPC = (REAL + P - 1) // P
    LOCAL = WPC * P
    NTOT = LOCAL * NC_CORES
    SENT = REAL if REAL < LOCAL else REAL - 1   # sentinel zero row in core 0

    src2 = np.concatenate([ei[0], np.arange(N)])
    dst2 = np.concatenate([ei[1], np.arange(N)])
    deg = np.bincount(dst2, minlength=N)

    local_rank = np.zeros(N, np.int64)
    rowid = np.zeros(N, np.int64)
    node_at = np.full((NC_CORES, LOCAL), -1, np.int64)
    for c in range(NC_CORES):
        ns = np.arange(c * REAL, (c + 1) * REAL)
        order = ns[np.argsort(-deg[ns], kind='stable')]
        local_rank[order] = np.arange(REAL)
        rowid[order] = c * LOCAL + np.arange(REAL)
        node_at[c, :REAL] = order

    # per-window max degree (shared across cores so the program is SPMD)
    Tw = np.ones(WPC, np.int64)
    for c in range(NC_CORES):
        first = node_at[c, ::P]
        for w in range(WPC):
            if first[w] >= 0:
                Tw[w] = max(Tw[w], deg[first[w]])
    T8w = ((Tw + 7) // 8) * 8
    gcol = np.concatenate([[0], np.cumsum(Tw)])       # GCN grid col offsets
    acol = np.concatenate([[0], np.cumsum(T8w)])      # GAT grid col offsets
    SLOTS = int(gcol[-1])
    SLOTS8 = int(acol[-1])
    GTOT = SLOTS8 // 8
    assert T8w.max() * 4 <= 160, "packed PSUM layout needs T8 <= 40"

    # GCN gather: equal-width window groups (pad to group-max T) so the
    # tree-sum and z-postprocessing batch across windows in single ops
    EQCAP = int(os.environ.get('K_EQCAP', '56'))
    EQGROUPS = []          # list of (w0, nw, Te)
    w = 0
    while w < WPC:
        Te = int(Tw[w])    # Tw nonincreasing -> group max
        nw = min(max(1, EQCAP // Te), WPC - w)
        EQGROUPS.append((w, nw, Te))
        w += nw
    eoff = []
    off = 0
    for (_, nw, Te) in EQGROUPS:
        eoff.append(off)
        off += nw * Te
    SLOTSR = off

    e_dst = rowid[dst2]
    e_src = src2
    o = np.argsort(e_dst, kind='stable')
    e_dst = e_dst[o]
    e_src = e_src[o]
    grp_start = np.searchsorted(e_dst, np.arange(NTOT), side='left')
    t_of = np.arange(len(e_dst)) - grp_start[e_dst]
    c_of = e_dst // LOCAL
    lrow = e_dst % LOCAL
    w_of = lrow // P
    p_of = lrow % P
    assert (t_of < Tw[w_of]).all()
    col_g = gcol[w_of] + t_of
    col_a = acol[w_of] + t_of

    slot_node_g = np.full((NC_CORES, P, SLOTS), N, np.int64)
    slot_node_g[c_of, p_of, col_g] = e_src
    slot_node_a = np.full((NC_CORES, P, SLOTS8), N, np.int64)
    slot_node_a[c_of, p_of, col_a] = e_src

    x_pad = np.vstack([x, np.zeros((1, x.shape[1]), np.float32)])
    rowid_pad = np.concatenate([rowid, [SENT]]).astype(np.int32)

    cnt = np.bincount(batch, minlength=Bsz).astype(np.float32)
    per_core = []
    for c in range(NC_CORES):
        sna = slot_node_a[c]                       # [P, SLOTS8], N = pad
        xs = x_pad[sna]                            # [P, SLOTS8, 9]
        xslots = np.empty((P, SLOTS8 * 9), np.float32)
        for w in range(WPC):
            a0, T8 = int(acol[w]), int(T8w[w])
            xslots[:, a0 * 9:(a0 + T8) * 9] = np.ascontiguousarray(
                xs[:, a0:a0 + T8, :].transpose(0, 2, 1)).reshape(P, T8 * 9)
        xslots = xslots.astype(ml_dtypes.bfloat16)
        xTl = np.zeros((16, SLOTS8, P), np.float32)
        xTl[0:9] = xs.transpose(2, 1, 0)
        xTl[9] = (sna.T == N).astype(np.float32)   # pad flag
        # [16j+f, (group)*128 + p] = xTl[f, 8*group+j, p]
        xgrp = np.ascontiguousarray(
            xTl.reshape(16, GTOT, 8, P).transpose(2, 0, 1, 3)
               .reshape(128, GTOT * P)).astype(ml_dtypes.bfloat16)
        srg = rowid_pad[slot_node_g[c]]            # [P, SLOTS]
        srcrowR = np.full((P, SLOTSR), SENT, np.int32)
        for g, (w0, nw, Te) in enumerate(EQGROUPS):
            for wi in range(nw):
                w = w0 + wi
                col = eoff[g] + wi * Te
                srcrowR[:, col:col + int(Tw[w])] = \
                    srg[:, gcol[w]:gcol[w] + int(Tw[w])]

        valid = node_at[c] >= 0
        xloc = np.zeros((9, LOCAL), np.float32)
        xloc[0:9, valid] = x[node_at[c][valid]].T

        dinv = np.zeros(LOCAL, np.float32)
        dinv[valid] = 1.0 / np.sqrt(deg[node_at[c][valid]])
        dinv_w = np.ascontiguousarray(dinv.reshape(WPC, P).T)

        bl = np.full(LOCAL, -1.0, np.float32)
        b_base = int(batch[c * REAL])
        bl[valid] = batch[node_at[c][valid]] - b_base
        assert bl.max() < 256, "batch window exceeded 256"
        bl_w = np.ascontiguousarray(bl.reshape(WPC, P).T)

        cnt_l = np.ones(256, np.float32)
        hi = min(256, Bsz - b_base)
        cnt_l[:hi] = np.maximum(cnt[b_base:b_base + hi], 1.0)
        scatv = np.zeros(256, np.int32)
        for j in range(256):
            scatv[j] = b_base + j if b_base + j < Bsz else Bsz + (j % 8)

        per_core.append(dict(
            xslots=xslots, xgrp=xgrp, srcrow=srcrowR,
            xlocT=xloc, dinv_w=dinv_w, bl_w=bl_w,
            cnt_l=np.ascontiguousarray(cnt_l.reshape(2, P).T),
            scat=np.ascontiguousarray(scatv.reshape(2, P).T),
            rows128=(c * P + np.arange(P, dtype=np.int32)).reshape(P, 1),
        ))

    baked = dict(N=N, REAL=REAL, WPC=WPC, LOCAL=LOCAL, NTOT=NTOT,
                 SLOTS=SLOTS, SLOTS8=SLOTS8, GTOT=GTOT, SLOTSR=SLOTSR,
                 EQGROUPS=EQGROUPS, eoff=eoff,
                 Tw=[int(t) for t in Tw], T8w=[int(t) for t in T8w],
                 gcol=[int(t) for t in gcol], acol=[int(t) for t in acol],
                 Bsz=Bsz)
    return per_core, baked


def fold_weights(inputs):
    w = {k: np.asarray(v, np.float32) for k, v in inputs.items()
         if k not in ('x', 'edge_index', 'batch')}
    H, C = 4, 32
    Wg = w['W_gat']
    was = np.einsum('fhc,hc->fh', Wg.reshape(9, H, C), w['att_src'])
    wad = np.einsum('fhc,hc->fh', Wg.reshape(9, H, C), w['att_dst'])
    was_aug = np.zeros((16, 4), np.float32)
    was_aug[0:9] = was
    was_aug[9] = -80.0
    wad_aug = np.zeros((9, 4), np.float32)
    wad_aug[0:9] = wad
    # block-diagonal was for 8 slot-columns per matmul
    wasD = np.zeros((128, 32), np.float32)
    for j in range(8):
        wasD[16 * j:16 * j + 16, 4 * j:4 * j + 4] = was_aug
    # [40,128] compact GAT weight: rows (10h+f) f<9 = W_gat, f=9 = bias
    wg40 = np.zeros((40, 128), np.float32)
    for h in range(H):
        wg40[h * 9:h * 9 + 9, h * 32:(h + 1) * 32] = Wg[:, h * 32:(h + 1) * 32]
        wg40[36 + h, h * 32:(h + 1) * 32] = w['b_gat'][h * 32:(h + 1) * 32]
    W3_aug = np.zeros((65, 128), np.float32)
    W3_aug[0:64] = w['W3']
    W3_aug[64] = w['b3']

    def fold(cw, cb, g, be, m, v):
        s = g / np.sqrt(v + 1e-5)
        return cw * s[:, None, None], (cb - m) * s + be

    c1w, c1b = fold(w['conv1_w'], w['conv1_b'], w['bn1_g'], w['bn1_b'],
                    w['bn1_m'], w['bn1_v'])
    c2w, c2b = fold(w['conv2_w'], w['conv2_b'], w['bn2_g'], w['bn2_b'],
                    w['bn2_m'], w['bn2_v'])
    # [cin, k, cout] flattened so slice k -> [cin, cout]
    w1k = np.ascontiguousarray(c1w.transpose(1, 2, 0)).reshape(30, 3 * 64)
    w2k = np.ascontiguousarray(c2w.transpose(1, 2, 0)).reshape(64, 3 * 64)
    fc1_Wr = np.ascontiguousarray(w['fc1_W'].reshape(64, 16 * 64))

    return dict(
        wasD=wasD.astype(ml_dtypes.bfloat16), wad_aug=wad_aug, wg40=wg40,
        W2=w['W2'], b2row=np.ascontiguousarray(np.broadcast_to(w['b2'], (P, 64))),
        W3_aug=W3_aug,
        w1k=w1k, b1=np.ascontiguousarray(c1b.reshape(64, 1)),
        w2k=w2k, b2c=np.ascontiguousarray(c2b.reshape(64, 1)),
        fc1_Wr=fc1_Wr, fc1_b=np.ascontiguousarray(w['fc1_b'].reshape(64, 1)),
        fus_W0=np.ascontiguousarray(w['fus_W'][0:128]),
        fus_W1=np.ascontiguousarray(w['fus_W'][128:192]),
        fus_b=np.ascontiguousarray(w['fus_b'].reshape(1, 128)),
        cls1_W=w['cls1_W'],
        cls1_b=np.ascontiguousarray(w['cls1_b'].reshape(1, 64)),
        cls3_W=w['cls3_W'],
        cls3_b_t=np.array([[float(w['cls3_b'][0])]], np.float32),
    )


# --------------------------------------------------------------------------
# device program
# --------------------------------------------------------------------------

def build_nc(baked):
    WPC, LOCAL, NTOT = baked['WPC'], baked['LOCAL'], baked['NTOT']
    SLOTS, SLOTS8, GTOT = baked['SLOTS'], baked['SLOTS8'], baked['GTOT']
    Tw, T8w, gcol, acol = baked['Tw'], baked['T8w'], baked['gcol'], baked['acol']
    SLOTSR = baked['SLOTSR']
    EQGROUPS, eoff = baked['EQGROUPS'], baked['eoff']
    NWMAX = max(nw for (_, nw, _) in EQGROUPS)
    ECAPMAX = max(nw * Te for (_, nw, Te) in EQGROUPS)
    Bsz = baked['Bsz']
    BROWS = Bsz + 8
    BPC = Bsz // NC_CORES                      # batches per core (fusion/seq)
    RG = [list(range(NC_CORES))]
    T8MAX = max(T8w)
    GMAX = T8MAX // 8

    nc = bacc.Bacc("TRN2", target_bir_lowering=False, debug=False,
                   num_devices=NC_CORES)

    def inp(name, shape, dt=F32):
        return nc.dram_tensor(name, shape, dt, kind="ExternalInput")

    xgrp = inp("xgrp", [128, GTOT * P], BF16)
    xslots = inp("xslots", [P, SLOTS8 * 9], BF16)
    srcrow = inp("srcrow", [P, SLOTSR], I32)
    xlocT = inp("xlocT", [9, LOCAL])
    dinv_w = inp("dinv_w", [P, WPC])
    bl_w = inp("bl_w", [P, WPC])
    cnt_l = inp("cnt_l", [P, 2])
    scat = inp("scat", [P, 2], I32)
    rows128 = inp("rows128", [P, 1], I32)
    iota256 = inp("iota256", [P, 256])
    ident = inp("ident", [P, P])
    ones4 = inp("ones4", [P, 4])
    onesrow = inp("onesrow", [1, BPC])
    wasD = inp("wasD", [128, 32], BF16)
    wad_aug = inp("wad_aug", [9, 4])
    wg40 = inp("wg40", [40, 128])
    W2 = inp("W2", [128, 64])
    b2row = inp("b2row", [P, 64])
    W3_aug = inp("W3_aug", [65, 128])
    w1k = inp("w1k", [30, 3 * 64])
    b1 = inp("b1", [64, 1])
    w2k = inp("w2k", [64, 3 * 64])
    b2c = inp("b2c", [64, 1])
    fc1_Wr = inp("fc1_Wr", [64, 16 * 64])
    fc1_b = inp("fc1_b", [64, 1])
    fus_W0 = inp("fus_W0", [128, 128])
    fus_W1 = inp("fus_W1", [64, 128])
    fus_b = inp("fus_b", [1, 128])
    cls1_W = inp("cls1_W", [128, 64])
    cls1_b = inp("cls1_b", [1, 64])
    cls3_W = inp("cls3_W", [64, 1])
    cls3_b_t = inp("cls3_b_t", [1, 1])
    xseq = inp("xseq", [30, BPC * 20])

    out = nc.dram_tensor("out", [1, BPC], F32, kind="ExternalOutput")
    DBG = bool(os.environ.get('K_DEBUG'))
    dbg_T2 = (nc.dram_tensor("dbg_T2", [LOCAL, 64], BF16, kind="ExternalOutput")
              if DBG else None)
    dbg_T3 = (nc.dram_tensor("dbg_T3", [LOCAL, 64], BF16, kind="ExternalOutput")
              if DBG else None)
    dbg_AR = (nc.dram_tensor("dbg_AR", [BROWS, 128], BF16, kind="ExternalOutput")
              if DBG else None)

    T2_local = nc.dram_tensor("T2_local", [LOCAL, 64], BF16)
    T2_full = nc.dram_tensor("T2_full", [NTOT, 64], BF16)
    T3_local = nc.dram_tensor("T3_local", [LOCAL, 64], BF16)
    T3_full = nc.dram_tensor("T3_full", [NTOT, 64], BF16)
    AR_in = nc.dram_tensor("AR_in", [BROWS, 128], BF16)
    AR_out = nc.dram_tensor("AR_out", [BROWS, 128], BF16)

    with tile.TileContext(nc) as tc:
        with tc.tile_pool(name="const", bufs=1) as cp, \
             tc.tile_pool(name="work", bufs=3) as wp, \
             tc.tile_pool(name="gath", bufs=3) as g2p, \
             tc.tile_pool(name="gat", bufs=3) as gp, \
             tc.tile_pool(name="psum", bufs=4, space="PSUM") as pp, \
             tc.tile_pool(name="spsum", bufs=2, space="PSUM") as spp, \
             tc.tile_pool(name="ppool", bufs=1, space="PSUM") as ppool, \
             tc.tile_pool(name="seq", bufs=1) as sq:

            def c_load(ap, shape, dt=F32):
                t = cp.tile(shape, dt, tag=f"c_{ap.name}")
                nc.sync.dma_start(t[:], ap[:])
                return t

            srcrow_sb = c_load(srcrow, [P, SLOTSR], I32)
            dinv_sb = c_load(dinv_w, [P, WPC])
            bl_sb = c_load(bl_w, [P, WPC])
            cnt_sb = c_load(cnt_l, [P, 2])
            scat_sb = c_load(scat, [P, 2], I32)
            rows_sb = c_load(rows128, [P, 1], I32)
            iota_sb = c_load(iota256, [P, 256])
            ident_sb = c_load(ident, [P, P])
            ones4_sb = c_load(ones4, [P, 4])
            onesr_sb = c_load(onesrow, [1, BPC])
            wasD_sb = c_load(wasD, [128, 32], BF16)
            wad_sb = c_load(wad_aug, [9, 4])
            wg40_sb = c_load(wg40, [40, 128])
            W2_sb = c_load(W2, [128, 64])
            b2row_sb = c_load(b2row, [P, 64])
            W3_sb = c_load(W3_aug, [65, 128])
            xloc_sb = c_load(xlocT, [9, LOCAL])
            w1_sb = c_load(w1k, [30, 3 * 64])
            b1_sb = c_load(b1, [64, 1])
            w2_sb = c_load(w2k, [64, 3 * 64])
            b2c_sb = c_load(b2c, [64, 1])
            fc1_sb = c_load(fc1_Wr, [64, 16 * 64])
            fc1b_sb = c_load(fc1_b, [64, 1])
            fusW0_sb = c_load(fus_W0, [128, 128])
            fusW1_sb = c_load(fus_W1, [64, 128])
            fusb_sb = c_load(fus_b, [1, 128])
            cls1W_sb = c_load(cls1_W, [128, 64])
            cls1b_sb = c_load(cls1_b, [1, 64])
            cls3W_sb = c_load(cls3_W, [64, 1])
            cls3b_sb = c_load(cls3_b_t, [1, 1])

            # persistent pooling PSUM [feat, 256 local batches], zeroed via
            # K=1 matmul (sets has_written)
            pool_psT = ppool.tile([P, 256], F32, tag="poolT")
            zrow = cp.tile([1, P], F32)
            zrow256 = cp.tile([1, 256], F32)
            nc.vector.memset(zrow[:], 0.0)
            nc.vector.memset(zrow256[:], 0.0)
            nc.tensor.matmul(pool_psT[:], zrow[:], zrow256[:],
                             start=True, stop=True)

            def vlrelu(dst, src, tmp_tag, pool, n):
                """dst = leakyrelu(src, 0.01) on the vector engine."""
                t = pool.tile([src.shape[0], n], F32, tag=tmp_tag)
                nc.vector.tensor_scalar(t[:], src, 0.01, None, OP.mult)
                nc.vector.tensor_tensor(dst, src, t[:], op=OP.max)

            # ================= seq branch (BPC batches, overlaps GAT) =====
            xsf = sq.tile([30, BPC * 20], F32, tag="xsf")
            nc.sync.dma_start(xsf[:], xseq[:])
            s1_sb = sq.tile([64, BPC * 18], F32, tag="s1")
            CH1 = 28
            for ci in range((BPC + CH1 - 1) // CH1):
                b0 = ci * CH1
                bn = min(CH1, BPC - b0)
                cps = spp.tile([64, 512], F32, tag="sps")
                for k in range(3):
                    nc.tensor.matmul(
                        cps[:, :bn * 18],
                        w1_sb[:, 64 * k:64 * (k + 1)],
                        xsf[:].rearrange("c (b t) -> c b t", t=20)[:, b0:b0 + bn, k:k + 18],
                        start=(k == 0), stop=(k == 2))
                t0 = sq.tile([64, CH1 * 18], F32, tag="sq_t0")
                t1 = sq.tile([64, CH1 * 18], F32, tag="sq_t1")
                nc.vector.tensor_scalar(t0[:, :bn * 18], cps[:, :bn * 18],
                                        b1_sb[:, 0:1], None, OP.add)
                nc.vector.tensor_scalar(t1[:, :bn * 18], cps[:, :bn * 18],
                                        b1_sb[:, 0:1], 0.01, OP.add, OP.mult)
                nc.vector.tensor_tensor(s1_sb[:, b0 * 18:(b0 + bn) * 18],
                                        t0[:, :bn * 18], t1[:, :bn * 18],
                                        op=OP.max)
            s2_sb = sq.tile([64, BPC * 16], F32, tag="s2")
            CH2 = 32
            for ci in range((BPC + CH2 - 1) // CH2):
                b0 = ci * CH2
                bn = min(CH2, BPC - b0)
                cps2 = spp.tile([64, 512], F32, tag="sps")
                for k in range(3):
                    nc.tensor.matmul(
                        cps2[:, :bn * 16],
                        w2_sb[:, 64 * k:64 * (k + 1)],
                        s1_sb[:].rearrange("c (b t) -> c b t", t=18)[:, b0:b0 + bn, k:k + 16],
                        start=(k == 0), stop=(k == 2))
                t0 = sq.tile([64, CH2 * 16], F32, tag="sq_u0")
                t1 = sq.tile([64, CH2 * 16], F32, tag="sq_u1")
                nc.vector.tensor_scalar(t0[:, :bn * 16], cps2[:, :bn * 16],
                                        b2c_sb[:, 0:1], None, OP.add)
                nc.vector.tensor_scalar(t1[:, :bn * 16], cps2[:, :bn * 16],
                                        b2c_sb[:, 0:1], 0.01, OP.add, OP.mult)
                nc.vector.tensor_tensor(s2_sb[:, b0 * 16:(b0 + bn) * 16],
                                        t0[:, :bn * 16], t1[:, :bn * 16],
                                        op=OP.max)
            fps = spp.tile([64, 512], F32, tag="sps")
            for t in range(16):
                nc.tensor.matmul(
                    fps[:, :BPC],
                    fc1_sb[:].rearrange("c (t j) -> c t j", j=64)[:, t, :],
                    s2_sb[:].rearrange("c (b t) -> c b t", t=16)[:, :, t:t + 1],
                    start=(t == 0), stop=(t == 15))
            sT = sq.tile([64, BPC], F32, tag="sT")
            nc.vector.tensor_scalar(sT[:], fps[:, :BPC], fc1b_sb[:, 0:1], None, OP.add)

            # ================= GAT =================
            def gat_body(w):
                T8 = T8w[w]
                G = T8 // 8
                gbase = acol[w] // 8
                ps = pp.tile([P, 512], F32, tag="ps")   # one PSUM bank/window
                ad_ps = ps[:, 0:4]
                as_ps = ps[:, 32:32 + 4 * T8MAX]
                zT_ps = ps[0:40, 192:320]
                g1_ps = ps[:, 320:448]
                h2_ps = ps[:, 448:512]
                nc.tensor.matmul(ad_ps, xloc_sb[:, bass.ds(w * P, P)],
                                 wad_sb[:], start=True, stop=True)
                T_d = gp.tile([P, 4], BF16, tag="Td")
                nc.scalar.activation(T_d[:], ad_ps, AF.Exp, scale=0.8)

                xg = gp.tile([128, GMAX * P], BF16, tag="xg")
                nc.sync.dma_start(xg[:, :G * P],
                                  xgrp[:, bass.ds(gbase * P, G * P)])
                for g in range(G):
                    nc.tensor.matmul(as_ps[:, 32 * g:32 * g + 32],
                                     xg[:, P * g:P * (g + 1)], wasD_sb[:],
                                     start=True, stop=True)
                Pt = gp.tile([P, 4 * T8MAX], BF16, tag="Pt")
                Rt = gp.tile([P, 4 * T8MAX], BF16, tag="Rt")
                nc.scalar.activation(Pt[:, :4 * T8], as_ps[:, :4 * T8],
                                     AF.Exp, scale=1.0)
                nc.scalar.activation(Rt[:, :4 * T8], as_ps[:, :4 * T8],
                                     AF.Exp, scale=0.2)

                EX = gp.tile([P, 4 * T8MAX], BF16, tag="EX")
                nc.vector.tensor_tensor(
                    EX[:, :4 * T8].rearrange("p (t h) -> p t h", h=4),
                    Pt[:, :4 * T8].rearrange("p (t h) -> p t h", h=4),
                    T_d[:, None, :].to_broadcast([P, T8, 4]),
                    op=OP.mult)
                nc.vector.tensor_tensor(EX[:, :4 * T8], EX[:, :4 * T8],
                                        Rt[:, :4 * T8], op=OP.max)
                S4 = gp.tile([P, 4], F32, tag="S4")
                nc.vector.tensor_reduce(
                    S4[:, :, None],
                    EX[:, :4 * T8].rearrange("p (t h) -> p h t", h=4),
                    axis=AX.X, op=OP.add)
                nc.vector.reciprocal(S4[:], S4[:])
                S4b = gp.tile([P, 4], BF16, tag="S4b")
                nc.vector.tensor_copy(S4b[:], S4[:])
                # AL in (h, t) layout -> ZR/zaug reduce become stride-1
                AL = gp.tile([P, 4 * T8MAX], BF16, tag="AL")
                nc.vector.tensor_tensor(
                    AL[:, :4 * T8].rearrange("p (h t) -> p t h", t=T8),
                    EX[:, :4 * T8].rearrange("p (t h) -> p t h", h=4),
                    S4b[:, None, :].to_broadcast([P, T8, 4]),
                    op=OP.mult)

                XS = gp.tile([P, 9 * T8MAX], BF16, tag="XS")
                nc.sync.dma_start(XS[:, :9 * T8],
                                  xslots[:, bass.ds(acol[w] * 9, T8 * 9)])
                ZR = gp.tile([P, 36 * T8MAX], BF16, tag="ZR")
                nc.vector.tensor_tensor(
                    ZR[:, :36 * T8].rearrange("p (h f t) -> p h f t", f=9, t=T8),
                    XS[:, :9 * T8].rearrange("p (f t) -> p f t", t=T8)[:, None, :, :]
                        .to_broadcast([P, 4, 9, T8]),
                    AL[:, :4 * T8].rearrange("p (h t) -> p h t", t=T8)[:, :, None, :]
                        .to_broadcast([P, 4, 9, T8]),
                    op=OP.mult)
                zaug = gp.tile([P, 40], F32, tag="zaug")
                nc.vector.tensor_copy(zaug[:, 36:40], ones4_sb[:])
                nc.vector.tensor_reduce(
                    zaug[:, 0:36][:, :, None],
                    ZR[:, :36 * T8].rearrange("p (q t) -> p q t", t=T8),
                    axis=AX.X, op=OP.add)
                nc.tensor.transpose(out=zT_ps, in_=zaug[:], identity=ident_sb[:])
                zT = gp.tile([40, P], F32, tag="zTs")
                nc.vector.tensor_copy(zT[:], zT_ps)
                nc.tensor.matmul(g1_ps, zT[:], wg40_sb[:],
                                 start=True, stop=True)
                g1T = gp.tile([P, P], F32, tag="g1T")
                vlrelu(g1T[:], g1_ps, "g1a", gp, P)
                nc.tensor.matmul(h2_ps, g1T[:], W2_sb[:], start=True, stop=True)
                T2s = gp.tile([P, 64], BF16, tag="T2s")
                nc.vector.tensor_scalar(T2s[:], h2_ps,
                                        dinv_sb[:, bass.ds(w, 1)], None, OP.mult)
                nc.sync.dma_start(T2_local[bass.ds(w * P, P), :], T2s[:])

            with nc.named_scope("gat"):
                for w in range(WPC):
                    gat_body(w)

            tc.strict_bb_all_engine_barrier()
            with nc.named_scope("ag1"):
                nc.gpsimd.collective_compute(
                    "AllGather", OP.bypass, replica_groups=RG,
                    ins=[T2_local.ap().opt()], outs=[T2_full.ap().opt()])
            tc.strict_bb_all_engine_barrier()

            # ================= GCN layers =================
            dinv_bw = cp.tile([P, WPC], BF16, tag="dinv_bw")
            nc.vector.tensor_copy(dinv_bw[:], dinv_sb[:])

            def gcn_group(gi, table, last):
                w0, nw, Te = EQGROUPS[gi]
                G2 = g2p.tile([P, ECAPMAX * 64], BF16, tag="G2")
                nc.gpsimd.indirect_dma_start(
                    out=G2[:, :nw * Te * 64], out_offset=None,
                    in_=table[:],
                    in_offset=bass.IndirectOffsetOnAxis(
                        ap=srcrow_sb[:, eoff[gi]:eoff[gi] + nw * Te], axis=0))

                def gv(t0, t1):
                    return G2[:, :nw * Te * 64].rearrange(
                        "p (w t c) -> p w t c", t=Te, c=64)[:, :, t0:t1, :]

                Tc = Te
                while Tc > 1:
                    if Tc & 1:
                        nc.vector.tensor_tensor(gv(0, 1), gv(0, 1),
                                                gv(Tc - 1, Tc), op=OP.add)
                        Tc -= 1
                    H = Tc // 2
                    nc.vector.tensor_tensor(gv(0, H), gv(0, H),
                                            gv(H, 2 * H), op=OP.add)
                    Tc = H
                gcol0 = G2[:, :nw * Te * 64].rearrange(
                    "p (w x) -> p w x", x=Te * 64)[:, :, 0:64]
                dvb = dinv_bw[:, w0:w0 + nw, None].to_broadcast([P, nw, 64])
                dvf = dinv_sb[:, w0:w0 + nw, None].to_broadcast([P, nw, 64])
                if not last:
                    zf = wp.tile([P, NWMAX * 64], F32, tag="zf")
                    zfv = zf[:, :nw * 64].rearrange("p (w c) -> p w c", c=64)
                    nc.vector.tensor_tensor(zfv, gcol0, dvb, op=OP.mult)
                    nc.vector.tensor_tensor(
                        zfv, zfv,
                        b2row_sb[:, None, :].to_broadcast([P, nw, 64]),
                        op=OP.add)
                    u = wp.tile([P, NWMAX * 64], F32, tag="u")
                    uv = u[:, :nw * 64].rearrange("p (w c) -> p w c", c=64)
                    nc.vector.tensor_tensor(uv, zfv, dvf, op=OP.mult)
                    v = wp.tile([P, NWMAX * 64], F32, tag="v")
                    nc.vector.tensor_scalar(v[:, :nw * 64], u[:, :nw * 64],
                                            0.01, None, OP.mult)
                    T3b = wp.tile([P, NWMAX * 64], BF16, tag="T3b")
                    nc.vector.tensor_tensor(T3b[:, :nw * 64], u[:, :nw * 64],
                                            v[:, :nw * 64], op=OP.max)
                    nc.sync.dma_start(
                        T3_local[bass.ds(w0 * P, nw * P), :]
                            .rearrange("(w p) c -> p w c", p=P),
                        T3b[:, :nw * 64].rearrange("p (w c) -> p w c", c=64))
                else:
                    z3b = wp.tile([P, NWMAX * 64], F32, tag="z3b")
                    nc.vector.tensor_tensor(
                        z3b[:, :nw * 64].rearrange("p (w c) -> p w c", c=64),
                        gcol0, dvb, op=OP.mult)
                    for wi in range(nw):
                        w = w0 + wi
                        z3s = wp.tile([P, 65], F32, tag="z3s")
                        nc.vector.tensor_copy(
                            z3s[:, 0:64], z3b[:, wi * 64:(wi + 1) * 64])
                        nc.vector.tensor_copy(z3s[:, 64:65], ones4_sb[:, 0:1])
                        ps2 = pp.tile([P, 512], F32, tag="ps")
                        z3T_ps = ps2[0:65, 0:128]
                        g3_ps = ps2[:, 128:256]
                        nc.tensor.transpose(out=z3T_ps, in_=z3s[:],
                                            identity=ident_sb[:])
                        z3T = wp.tile([65, P], F32, tag="z3Ts")
                        nc.vector.tensor_copy(z3T[:], z3T_ps)
                        nc.tensor.matmul(g3_ps, z3T[:], W3_sb[:],
                                         start=True, stop=True)
                        g3 = wp.tile([P, P], F32, tag="g3s")
                        nc.scalar.activation(g3[:], g3_ps, AF.Lrelu, alpha=0.01)
                        Mp = wp.tile([P, 256], F32, tag="Mp")
                        nc.vector.tensor_scalar(
                            Mp[:], iota_sb[:], bl_sb[:, bass.ds(w, 1)], None,
                            OP.is_equal)
                        nc.tensor.matmul(pool_psT[:], g3[:], Mp[:],
                                         start=False, stop=True)

            with nc.named_scope("gcn1"):
                for gi in range(len(EQGROUPS)):
                    gcn_group(gi, T2_full, last=False)

            tc.strict_bb_all_engine_barrier()
            with nc.named_scope("ag2"):
                nc.gpsimd.collective_compute(
                    "AllGather", OP.bypass, replica_groups=RG,
                    ins=[T3_local.ap().opt()], outs=[T3_full.ap().opt()])
            tc.strict_bb_all_engine_barrier()

            with nc.named_scope("gcn2"):
                # zero the AllReduce input (rows not covered by this core)
                zb = wp.tile([P, 128], BF16, tag="zb")
                nc.vector.memset(zb[:], 0.0)
                r0 = 0
                while r0 < BROWS:
                    r1 = min(r0 + P, BROWS)
                    nc.sync.dma_start(AR_in[r0:r1, :], zb[:r1 - r0, :])
                    r0 = r1
                for gi in range(len(EQGROUPS)):
                    gcn_group(gi, T3_full, last=True)

                crec = wp.tile([P, 2], F32, tag="crec")
                nc.vector.reciprocal(crec[:], cnt_sb[:])
                poolTs = wp.tile([P, 256], F32, tag="poolTs")
                nc.vector.tensor_copy(poolTs[:], pool_psT[:])
                for k in range(2):
                    tp2 = pp.tile([P, 512], F32, tag="ps")
                    nc.tensor.transpose(out=tp2[:, 0:128],
                                        in_=poolTs[:, k * 128:(k + 1) * 128],
                                        identity=ident_sb[:])
                    pooled = wp.tile([P, 128], BF16, tag="pooled")
                    nc.vector.tensor_scalar(pooled[:], tp2[:, 0:128],
                                            crec[:, k:k + 1], None, OP.mult)
                    nc.gpsimd.indirect_dma_start(
                        out=AR_in[:], out_offset=bass.IndirectOffsetOnAxis(
                            ap=scat_sb[:, k:k + 1], axis=0),
                        in_=pooled[:], in_offset=None)

            tc.strict_bb_all_engine_barrier()
            with nc.named_scope("ar"):
                nc.gpsimd.collective_compute(
                    "AllReduce", OP.add, replica_groups=RG,
                    ins=[AR_in.ap().opt()], outs=[AR_out.ap().opt()])
            tc.strict_bb_all_engine_barrier()

            if DBG:
                dt_ = sq.tile([P, 64], BF16, tag="dbg_t")
                for i in range(LOCAL // P):
                    nc.sync.dma_start(dt_[:], T2_local[i * P:(i + 1) * P, :])
                    nc.sync.dma_start(dbg_T2[i * P:(i + 1) * P, :], dt_[:])
                    nc.sync.dma_start(dt_[:], T3_local[i * P:(i + 1) * P, :])
                    nc.sync.dma_start(dbg_T3[i * P:(i + 1) * P, :], dt_[:])
                dt2 = sq.tile([P, 128], BF16, tag="dbg_t2")
                r0 = 0
                while r0 < BROWS:
                    r1 = min(r0 + P, BROWS)
                    nc.sync.dma_start(dt2[:r1 - r0, :], AR_out[r0:r1, :])
                    nc.sync.dma_start(dbg_AR[r0:r1, :], dt2[:r1 - r0, :])
                    r0 = r1

            # ================= fusion + classifier (BPC batches) ==========
            with nc.named_scope("fuse"):
                prow = sq.tile([P, 128], BF16, tag="prow")
                nc.gpsimd.indirect_dma_start(
                    out=prow[:], out_offset=None,
                    in_=AR_out[:],
                    in_offset=bass.IndirectOffsetOnAxis(
                        ap=rows_sb[:, 0:1], axis=0))
                prow32 = sq.tile([P, 128], F32, tag="prow32")
                nc.vector.tensor_copy(prow32[:], prow[:])
                fps_ = pp.tile([P, 512], F32, tag="ps")
                tp_ps = fps_[:, 0:128]
                ups = fps_[:, 128:256]
                vps = fps_[0:64, 256:384]
                ops_ = fps_[0:1, 384:512]
                nc.tensor.transpose(out=tp_ps, in_=prow32[:], identity=ident_sb[:])
                poolT = sq.tile([P, BPC], F32, tag="poolT")
                nc.vector.tensor_copy(poolT[:], tp_ps)

                nc.tensor.matmul(ups[:, :BPC], fusW0_sb[:], poolT[:],
                                 start=True, stop=False)
                nc.tensor.matmul(ups[:, :BPC], fusW1_sb[:], sT[:],
                                 start=False, stop=False)
                nc.tensor.matmul(ups[:, :BPC], fusb_sb[:], onesr_sb[:],
                                 start=False, stop=True)
                combT = sq.tile([P, BPC], F32, tag="combT")
                vlrelu(combT[:], ups[:, :BPC], "fu_a", sq, BPC)
                nc.tensor.matmul(vps[:, :BPC], cls1W_sb[:], combT[:],
                                 start=True, stop=False)
                nc.tensor.matmul(vps[:, :BPC], cls1b_sb[:], onesr_sb[:],
                                 start=False, stop=True)
                c1T = sq.tile([64, BPC], F32, tag="c1T")
                vlrelu(c1T[:], vps[:, :BPC], "fu_b", sq, BPC)
                nc.tensor.matmul(ops_[:, :BPC], cls3W_sb[:], c1T[:],
                                 start=True, stop=True)
                out_sb = sq.tile([1, BPC], F32, tag="out_sb")
                nc.vector.tensor_scalar(
                    out_sb[:], ops_[:, :BPC], cls3b_sb[0:1, 0:1], None, OP.add)
                nc.sync.dma_start(out[:], out_sb[:])

    nc.compile()
    return nc


# --------------------------------------------------------------------------
# entry point
# --------------------------------------------------------------------------

_CACHE = {}
LAST_RESULT = None


def kernel(**inputs):
    kh = hash((np.asarray(inputs['edge_index']).tobytes(),
               np.asarray(inputs['x']).tobytes()))
    if kh not in _CACHE:
        per_core, baked = host_prep(inputs)
        nc = build_nc(baked)
        _CACHE[kh] = (per_core, baked, nc)
    per_core, baked, nc = _CACHE[kh]

    wts = fold_weights(inputs)
    Bsz = baked['Bsz']
    BPC = Bsz // NC_CORES
    seq = np.asarray(inputs['seq_data'], np.float32)      # [B, 30, 20]
    seqT = np.ascontiguousarray(seq.transpose(1, 0, 2))   # [30, B, 20]
    shared = dict(
        iota256=np.ascontiguousarray(
            np.broadcast_to(np.arange(256, dtype=np.float32), (P, 256))),
        ident=np.eye(P, dtype=np.float32),
        ones4=np.ones((P, 4), np.float32),
        onesrow=np.ones((1, BPC), np.float32),
        **wts)
    in_maps = []
    for c in range(NC_CORES):
        m = dict(shared)
        m.update(per_core[c])
        m['xseq'] = np.ascontiguousarray(
            seqT[:, c * BPC:(c + 1) * BPC, :]).reshape(30, BPC * 20)
        in_maps.append(m)

    global LAST_RESULT
    res = run_bass_kernel_spmd(
        nc, in_maps, core_ids=list(range(NC_CORES)),
        trace=bool(os.environ.get('BASS_KERNEL_TRACE')))
    LAST_RESULT = res
    if os.environ.get('K_DEBUG'):
        np.savez('/tmp/kdbg.npz',
                 **{f"{k}_{c}": res.results[c][k] for c in range(NC_CORES)
                    for k in ("dbg_T2", "dbg_T3", "dbg_AR")})
    o = np.concatenate([res.results[c]["out"].reshape(-1)
                        for c in range(NC_CORES)]).reshape(Bsz, 1)
    return o.astype(np.float32)


# revision 28
# speedup vs baseline: 10.3022x; 1.0091x over previous
"""Trainium2 Bass kernel for nn_DeepCPP (GAT + 2xGCN graph branch, conv1d seq
branch, fusion MLP), SPMD over 8 NeuronCores.

Sharding/strategy:
 - Nodes partitioned across cores in natural order (keeps sorted `batch`
   contiguous per core); within a core nodes are sorted by in-degree so
   128-node windows have near-uniform max degree (node-major slot grids).
 - GAT attention logits per edge slot are computed with block-diagonal
   batched matmuls (8 slot-columns per matmul); exp(leakyrelu(a_s+a_d)) is
   factorized as max(P_e*T_d, R_e) with P=exp(a_s), R=exp(0.2*a_s),
   T=exp(0.8*a_d); the per-dst factor exp(-0.2*a_d) cancels in the softmax.
 - GCN layers gather 256B rows (dinv-prescaled h) from an AllGathered table
   with ONE batched indirect DMA per pair of 128-node windows; aggregation
   is a strided vector reduction.
 - Mean-pool via one-hot selection matmuls into persistent PSUM, AllReduce
   of partials; seq branch and fusion MLP are sharded by batch (128/core).
 - All loops fully unrolled (no hardware loops); non-Exp pointwise work runs
   on the Vector engine so the Scalar activation table stays loaded.
"""

import os
import sys

sys.path.insert(0, '/opt/trn_rl_repo')

import numpy as np
import ml_dtypes

import concourse.bass as bass
import concourse.mybir as mybir
import concourse.tile as tile
from concourse import bacc
from concourse.bass_utils import run_bass_kernel_spmd

F32 = mybir.dt.float32
BF16 = mybir.dt.bfloat16
I32 = mybir.dt.int32
AF = mybir.ActivationFunctionType
OP = mybir.AluOpType
AX = mybir.AxisListType

NC_CORES = 8
P = 128


# --------------------------------------------------------------------------
# host-side prep (layout/indexing only; cached per (x, edge_index))
# --------------------------------------------------------------------------

def host_prep(inputs):
    x = np.asarray(inputs['x'], np.float32)
    ei = np.asarray(inputs['edge_index'], np.int64)
    batch = np.asarray(inputs['batch'], np.int64)
    N = x.shape[0]
    Bsz = int(np.asarray(inputs['seq_data']).shape[0])
    assert N % NC_CORES == 0
    REAL = N // NC_CORES
    WPC = (REAL + P - 1) // P
    LOCAL = WPC * P
    NTOT = LOCAL * NC_CORES
    SENT = REAL if REAL < LOCAL else REAL - 1   # sentinel zero row in core 0

    src2 = np.concatenate([ei[0], np.arange(N)])
    dst2 = np.concatenate([ei[1], np.arange(N)])
    deg = np.bincount(dst2, minlength=N)

    local_rank = np.zeros(N, np.int64)
    rowid = np.zeros(N, np.int64)
    node_at = np.full((NC_CORES, LOCAL), -1, np.int64)
    for c in range(NC_CORES):
        ns = np.arange(c * REAL, (c + 1) * REAL)
        order = ns[np.argsort(-deg[ns], kind='stable')]
        local_rank[order] = np.arange(REAL)
        rowid[order] = c * LOCAL + np.arange(REAL)
        node_at[c, :REAL] = order

    # per-window max degree (shared across cores so the program is SPMD)
    Tw = np.ones(WPC, np.int64)
    for c in range(NC_CORES):
        first = node_at[c, ::P]
        for w in range(WPC):
            if first[w] >= 0:
                Tw[w] = max(Tw[w], deg[first[w]])
    T8w = ((Tw + 7) // 8) * 8
    gcol = np.concatenate([[0], np.cumsum(Tw)])       # GCN grid col offsets
    acol = np.concatenate([[0], np.cumsum(T8w)])      # GAT grid col offsets
    SLOTS = int(gcol[-1])
    SLOTS8 = int(acol[-1])
    GTOT = SLOTS8 // 8
    assert T8w.max() * 4 <= 160, "packed PSUM layout needs T8 <= 40"

    # GCN gather: equal-width window groups (pad to group-max T) so the
    # tree-sum and z-postprocessing batch across windows in single ops
    EQCAP = int(os.environ.get('K_EQCAP', '56'))
    EQGROUPS = []          # list of (w0, nw, Te)
    w = 0
    while w < WPC:
        Te = int(Tw[w])    # Tw nonincreasing -> group max
        nw = min(max(1, EQCAP // Te), WPC - w)
        EQGROUPS.append((w, nw, Te))
        w += nw
    eoff = []
    off = 0
    for (_, nw, Te) in EQGROUPS:
        eoff.append(off)
        off += nw * Te
    SLOTSR = off

    e_dst = rowid[dst2]
    e_src = src2
    o = np.argsort(e_dst, kind='stable')
    e_dst = e_dst[o]
    e_src = e_src[o]
    grp_start = np.searchsorted(e_dst, np.arange(NTOT), side='left')
    t_of = np.arange(len(e_dst)) - grp_start[e_dst]
    c_of = e_dst // LOCAL
    lrow = e_dst % LOCAL
    w_of = lrow // P
    p_of = lrow % P
    assert (t_of < Tw[w_of]).all()
    col_g = gcol[w_of] + t_of
    col_a = acol[w_of] + t_of

    slot_node_g = np.full((NC_CORES, P, SLOTS), N, np.int64)
    slot_node_g[c_of, p_of, col_g] = e_src
    slot_node_a = np.full((NC_CORES, P, SLOTS8), N, np.int64)
    slot_node_a[c_of, p_of, col_a] = e_src

    x_pad = np.vstack([x, np.zeros((1, x.shape[1]), np.float32)])
    rowid_pad = np.concatenate([rowid, [SENT]]).astype(np.int32)

    cnt = np.bincount(batch, minlength=Bsz).astype(np.float32)
    per_core = []
    for c in range(NC_CORES):
        sna = slot_node_a[c]                       # [P, SLOTS8], N = pad
        xs = x_pad[sna]                            # [P, SLOTS8, 9]
        xslots = np.empty((P, SLOTS8 * 9), np.float32)
        for w in range(WPC):
            a0, T8 = int(acol[w]), int(T8w[w])
            xslots[:, a0 * 9:(a0 + T8) * 9] = np.ascontiguousarray(
                xs[:, a0:a0 + T8, :].transpose(0, 2, 1)).reshape(P, T8 * 9)
        xslots = xslots.astype(ml_dtypes.bfloat16)
        xTl = np.zeros((16, SLOTS8, P), np.float32)
        xTl[0:9] = xs.transpose(2, 1, 0)
        xTl[9] = (sna.T == N).astype(np.float32)   # pad flag
        # [16j+f, (group)*128 + p] = xTl[f, 8*group+j, p]
        xgrp = np.ascontiguousarray(
            xTl.reshape(16, GTOT, 8, P).transpose(2, 0, 1, 3)
               .reshape(128, GTOT * P)).astype(ml_dtypes.bfloat16)
        srg = rowid_pad[slot_node_g[c]]            # [P, SLOTS]
        srcrowR = np.full((P, SLOTSR), SENT, np.int32)
        for g, (w0, nw, Te) in enumerate(EQGROUPS):
            for wi in range(nw):
                w = w0 + wi
                col = eoff[g] + wi * Te
                srcrowR[:, col:col + int(Tw[w])] = \
                    srg[:, gcol[w]:gcol[w] + int(Tw[w])]

        valid = node_at[c] >= 0
        xloc = np.zeros((9, LOCAL), np.float32)
        xloc[0:9, valid] = x[node_at[c][valid]].T

        dinv = np.zeros(LOCAL, np.float32)
        dinv[valid] = 1.0 / np.sqrt(deg[node_at[c][valid]])
        dinv_w = np.ascontiguousarray(dinv.reshape(WPC, P).T)

        bl = np.full(LOCAL, -1.0, np.float32)
        b_base = int(batch[c * REAL])
        bl[valid] = batch[node_at[c][valid]] - b_base
        assert bl.max() < 256, "batch window exceeded 256"
        bl_w = np.ascontiguousarray(bl.reshape(WPC, P).T)

        cnt_l = np.ones(256, np.float32)
        hi = min(256, Bsz - b_base)
        cnt_l[:hi] = np.maximum(cnt[b_base:b_base + hi], 1.0)
        scatv = np.zeros(256, np.int32)
        for j in range(256):
            scatv[j] = b_base + j if b_base + j < Bsz else Bsz + (j % 8)

        per_core.append(dict(
            xslots=xslots, xgrp=xgrp, srcrow=srcrowR,
            xlocT=xloc, dinv_w=dinv_w, bl_w=bl_w,
            cnt_l=np.ascontiguousarray(cnt_l.reshape(2, P).T),
            scat=np.ascontiguousarray(scatv.reshape(2, P).T),
            rows128=(c * P + np.arange(P, dtype=np.int32)).reshape(P, 1),
        ))

    baked = dict(N=N, REAL=REAL, WPC=WPC, LOCAL=LOCAL, NTOT=NTOT,
                 SLOTS=SLOTS, SLOTS8=SLOTS8, GTOT=GTOT, SLOTSR=SLOTSR,
                 EQGROUPS=EQGROUPS, eoff=eoff,
                 Tw=[int(t) for t in Tw], T8w=[int(t) for t in T8w],
                 gcol=[int(t) for t in gcol], acol=[int(t) for t in acol],
                 Bsz=Bsz)
    return per_core, baked


def fold_weights(inputs):
    w = {k: np.asarray(v, np.float32) for k, v in inputs.items()
         if k not in ('x', 'edge_index', 'batch')}
    H, C = 4, 32
    Wg = w['W_gat']
    was = np.einsum('fhc,hc->fh', Wg.reshape(9, H, C), w['att_src'])
    wad = np.einsum('fhc,hc->fh', Wg.reshape(9, H, C), w['att_dst'])
    was_aug = np.zeros((16, 4), np.float32)
    was_aug[0:9] = was
    was_aug[9] = -80.0
    wad_aug = np.zeros((9, 4), np.float32)
    wad_aug[0:9] = wad
    # block-diagonal was for 8 slot-columns per matmul
    wasD = np.zeros((128, 32), np.float32)
    for j in range(8):
        wasD[16 * j:16 * j + 16, 4 * j:4 * j + 4] = was_aug
    # [40,128] compact GAT weight: rows (10h+f) f<9 = W_gat, f=9 = bias
    wg40 = np.zeros((40, 128), np.float32)
    for h in range(H):
        wg40[h * 9:h * 9 + 9, h * 32:(h + 1) * 32] = Wg[:, h * 32:(h + 1) * 32]
        wg40[36 + h, h * 32:(h + 1) * 32] = w['b_gat'][h * 32:(h + 1) * 32]
    W3_aug = np.zeros((65, 128), np.float32)
    W3_aug[0:64] = w['W3']
    W3_aug[64] = w['b3']

    def fold(cw, cb, g, be, m, v):
        s = g / np.sqrt(v + 1e-5)
        return cw * s[:, None, None], (cb - m) * s + be

    c1w, c1b = fold(w['conv1_w'], w['conv1_b'], w['bn1_g'], w['bn1_b'],
                    w['bn1_m'], w['bn1_v'])
    c2w, c2b = fold(w['conv2_w'], w['conv2_b'], w['bn2_g'], w['bn2_b'],
                    w['bn2_m'], w['bn2_v'])
    # [cin, k, cout] flattened so slice k -> [cin, cout]
    w1k = np.ascontiguousarray(c1w.transpose(1, 2, 0)).reshape(30, 3 * 64)
    w2k = np.ascontiguousarray(c2w.transpose(1, 2, 0)).reshape(64, 3 * 64)
    fc1_Wr = np.ascontiguousarray(w['fc1_W'].reshape(64, 16 * 64))

    return dict(
        wasD=wasD.astype(ml_dtypes.bfloat16), wad_aug=wad_aug, wg40=wg40,
        W2=w['W2'], b2row=np.ascontiguousarray(np.broadcast_to(w['b2'], (P, 64))),
        W3_aug=W3_aug,
        w1k=w1k, b1=np.ascontiguousarray(c1b.reshape(64, 1)),
        w2k=w2k, b2c=np.ascontiguousarray(c2b.reshape(64, 1)),
        fc1_Wr=fc1_Wr, fc1_b=np.ascontiguousarray(w['fc1_b'].reshape(64, 1)),
        fus_W0=np.ascontiguousarray(w['fus_W'][0:128]),
        fus_W1=np.ascontiguousarray(w['fus_W'][128:192]),
        fus_b=np.ascontiguousarray(w['fus_b'].reshape(1, 128)),
        cls1_W=w['cls1_W'],
        cls1_b=np.ascontiguousarray(w['cls1_b'].reshape(1, 64)),
        cls3_W=w['cls3_W'],
        cls3_b_t=np.array([[float(w['cls3_b'][0])]], np.float32),
    )


# --------------------------------------------------------------------------
# device program
# --------------------------------------------------------------------------

def build_nc(baked):
    WPC, LOCAL, NTOT = baked['WPC'], baked['LOCAL'], baked['NTOT']
    SLOTS, SLOTS8, GTOT = baked['SLOTS'], baked['SLOTS8'], baked['GTOT']
    Tw, T8w, gcol, acol = baked['Tw'], baked['T8w'], baked['gcol'], baked['acol']
    SLOTSR = baked['SLOTSR']
    EQGROUPS, eoff = baked['EQGROUPS'], baked['eoff']
    NWMAX = max(nw for (_, nw, _) in EQGROUPS)
    ECAPMAX = max(nw * Te for (_, nw, Te) in EQGROUPS)
    Bsz = baked['Bsz']
    BROWS = Bsz + 8
    BPC = Bsz // NC_CORES                      # batches per core (fusion/seq)
    RG = [list(range(NC_CORES))]
    T8MAX = max(T8w)
    GMAX = T8MAX // 8

    nc = bacc.Bacc("TRN2", target_bir_lowering=False, debug=False,
                   num_devices=NC_CORES)

    def inp(name, shape, dt=F32):
        return nc.dram_tensor(name, shape, dt, kind="ExternalInput")

    xgrp = inp("xgrp", [128, GTOT * P], BF16)
    xslots = inp("xslots", [P, SLOTS8 * 9], BF16)
    srcrow = inp("srcrow", [P, SLOTSR], I32)
    xlocT = inp("xlocT", [9, LOCAL])
    dinv_w = inp("dinv_w", [P, WPC])
    bl_w = inp("bl_w", [P, WPC])
    cnt_l = inp("cnt_l", [P, 2])
    scat = inp("scat", [P, 2], I32)
    rows128 = inp("rows128", [P, 1], I32)
    iota256 = inp("iota256", [P, 256])
    ident = inp("ident", [P, P])
    ones4 = inp("ones4", [P, 4])
    onesrow = inp("onesrow", [1, BPC])
    wasD = inp("wasD", [128, 32], BF16)
    wad_aug = inp("wad_aug", [9, 4])
    wg40 = inp("wg40", [40, 128])
    W2 = inp("W2", [128, 64])
    b2row = inp("b2row", [P, 64])
    W3_aug = inp("W3_aug", [65, 128])
    w1k = inp("w1k", [30, 3 * 64])
    b1 = inp("b1", [64, 1])
    w2k = inp("w2k", [64, 3 * 64])
    b2c = inp("b2c", [64, 1])
    fc1_Wr = inp("fc1_Wr", [64, 16 * 64])
    fc1_b = inp("fc1_b", [64, 1])
    fus_W0 = inp("fus_W0", [128, 128])
    fus_W1 = inp("fus_W1", [64, 128])
    fus_b = inp("fus_b", [1, 128])
    cls1_W = inp("cls1_W", [128, 64])
    cls1_b = inp("cls1_b", [1, 64])
    cls3_W = inp("cls3_W", [64, 1])
    cls3_b_t = inp("cls3_b_t", [1, 1])
    xseq = inp("xseq", [30, BPC * 20])

    out = nc.dram_tensor("out", [1, BPC], F32, kind="ExternalOutput")
    DBG = bool(os.environ.get('K_DEBUG'))
    dbg_T2 = (nc.dram_tensor("dbg_T2", [LOCAL, 64], BF16, kind="ExternalOutput")
              if DBG else None)
    dbg_T3 = (nc.dram_tensor("dbg_T3", [LOCAL, 64], BF16, kind="ExternalOutput")
              if DBG else None)
    dbg_AR = (nc.dram_tensor("dbg_AR", [BROWS, 128], BF16, kind="ExternalOutput")
              if DBG else None)

    T2_local = nc.dram_tensor("T2_local", [LOCAL, 64], BF16)
    T2_full = nc.dram_tensor("T2_full", [NTOT, 64], BF16)
    T3_local = nc.dram_tensor("T3_local", [LOCAL, 64], BF16)
    T3_full = nc.dram_tensor("T3_full", [NTOT, 64], BF16)
    AR_in = nc.dram_tensor("AR_in", [BROWS, 128], BF16)
    AR_out = nc.dram_tensor("AR_out", [BROWS, 128], BF16)

    with tile.TileContext(nc) as tc:
        with tc.tile_pool(name="const", bufs=1) as cp, \
             tc.tile_pool(name="work", bufs=3) as wp, \
             tc.tile_pool(name="gath", bufs=3) as g2p, \
             tc.tile_pool(name="gat", bufs=3) as gp, \
             tc.tile_pool(name="psum", bufs=5, space="PSUM") as pp, \
             tc.tile_pool(name="spsum", bufs=2, space="PSUM") as spp, \
             tc.tile_pool(name="ppool", bufs=1, space="PSUM") as ppool, \
             tc.tile_pool(name="seq", bufs=1) as sq:

            def c_load(ap, shape, dt=F32):
                t = cp.tile(shape, dt, tag=f"c_{ap.name}")
                nc.sync.dma_start(t[:], ap[:])
                return t

            srcrow_sb = c_load(srcrow, [P, SLOTSR], I32)
            dinv_sb = c_load(dinv_w, [P, WPC])
            bl_sb = c_load(bl_w, [P, WPC])
            cnt_sb = c_load(cnt_l, [P, 2])
            scat_sb = c_load(scat, [P, 2], I32)
            rows_sb = c_load(rows128, [P, 1], I32)
            iota_sb = c_load(iota256, [P, 256])
            ident_sb = c_load(ident, [P, P])
            ones4_sb = c_load(ones4, [P, 4])
            onesr_sb = c_load(onesrow, [1, BPC])
            wasD_sb = c_load(wasD, [128, 32], BF16)
            wad_sb = c_load(wad_aug, [9, 4])
            wg40_sb = c_load(wg40, [40, 128])
            W2_sb = c_load(W2, [128, 64])
            b2row_sb = c_load(b2row, [P, 64])
            W3_sb = c_load(W3_aug, [65, 128])
            xloc_sb = c_load(xlocT, [9, LOCAL])
            w1_sb = c_load(w1k, [30, 3 * 64])
            b1_sb = c_load(b1, [64, 1])
            w2_sb = c_load(w2k, [64, 3 * 64])
            b2c_sb = c_load(b2c, [64, 1])
            fc1_sb = c_load(fc1_Wr, [64, 16 * 64])
            fc1b_sb = c_load(fc1_b, [64, 1])
            fusW0_sb = c_load(fus_W0, [128, 128])
            fusW1_sb = c_load(fus_W1, [64, 128])
            fusb_sb = c_load(fus_b, [1, 128])
            cls1W_sb = c_load(cls1_W, [128, 64])
            cls1b_sb = c_load(cls1_b, [1, 64])
            cls3W_sb = c_load(cls3_W, [64, 1])
            cls3b_sb = c_load(cls3_b_t, [1, 1])

            # persistent pooling PSUM [feat, 256 local batches], zeroed via
            # K=1 matmul (sets has_written)
            pool_psT = ppool.tile([P, 256], F32, tag="poolT")
            zrow = cp.tile([1, P], F32)
            zrow256 = cp.tile([1, 256], F32)
            nc.vector.memset(zrow[:], 0.0)
            nc.vector.memset(zrow256[:], 0.0)
            nc.tensor.matmul(pool_psT[:], zrow[:], zrow256[:],
                             start=True, stop=True)

            def vlrelu(dst, src, tmp_tag, pool, n):
                """dst = leakyrelu(src, 0.01) on the vector engine."""
                t = pool.tile([src.shape[0], n], F32, tag=tmp_tag)
                nc.vector.tensor_scalar(t[:], src, 0.01, None, OP.mult)
                nc.vector.tensor_tensor(dst, src, t[:], op=OP.max)

            # ================= seq branch (BPC batches, overlaps GAT) =====
            xsf = sq.tile([30, BPC * 20], F32, tag="xsf")
            nc.sync.dma_start(xsf[:], xseq[:])
            s1_sb = sq.tile([64, BPC * 18], F32, tag="s1")
            CH1 = 28
            for ci in range((BPC + CH1 - 1) // CH1):
                b0 = ci * CH1
                bn = min(CH1, BPC - b0)
                cps = spp.tile([64, 512], F32, tag="sps")
                for k in range(3):
                    nc.tensor.matmul(
                        cps[:, :bn * 18],
                        w1_sb[:, 64 * k:64 * (k + 1)],
                        xsf[:].rearrange("c (b t) -> c b t", t=20)[:, b0:b0 + bn, k:k + 18],
                        start=(k == 0), stop=(k == 2))
                t0 = sq.tile([64, CH1 * 18], F32, tag="sq_t0")
                t1 = sq.tile([64, CH1 * 18], F32, tag="sq_t1")
                nc.vector.tensor_scalar(t0[:, :bn * 18], cps[:, :bn * 18],
                                        b1_sb[:, 0:1], None, OP.add)
                nc.vector.tensor_scalar(t1[:, :bn * 18], cps[:, :bn * 18],
                                        b1_sb[:, 0:1], 0.01, OP.add, OP.mult)
                nc.vector.tensor_tensor(s1_sb[:, b0 * 18:(b0 + bn) * 18],
                                        t0[:, :bn * 18], t1[:, :bn * 18],
                                        op=OP.max)
            s2_sb = sq.tile([64, BPC * 16], F32, tag="s2")
            CH2 = 32
            for ci in range((BPC + CH2 - 1) // CH2):
                b0 = ci * CH2
                bn = min(CH2, BPC - b0)
                cps2 = spp.tile([64, 512], F32, tag="sps")
                for k in range(3):
                    nc.tensor.matmul(
                        cps2[:, :bn * 16],
                        w2_sb[:, 64 * k:64 * (k + 1)],
                        s1_sb[:].rearrange("c (b t) -> c b t", t=18)[:, b0:b0 + bn, k:k + 16],
                        start=(k == 0), stop=(k == 2))
                t0 = sq.tile([64, CH2 * 16], F32, tag="sq_u0")
                t1 = sq.tile([64, CH2 * 16], F32, tag="sq_u1")
                nc.vector.tensor_scalar(t0[:, :bn * 16], cps2[:, :bn * 16],
                                        b2c_sb[:, 0:1], None, OP.add)
                nc.vector.tensor_scalar(t1[:, :bn * 16], cps2[:, :bn * 16],
                                        b2c_sb[:, 0:1], 0.01, OP.add, OP.mult)
                nc.vector.tensor_tensor(s2_sb[:, b0 * 16:(b0 + bn) * 16],
                                        t0[:, :bn * 16], t1[:, :bn * 16],
                                        op=OP.max)
            fps = spp.tile([64, 512], F32, tag="sps")
            for t in range(16):
                nc.tensor.matmul(
                    fps[:, :BPC],
                    fc1_sb[:].rearrange("c (t j) -> c t j", j=64)[:, t, :],
                    s2_sb[:].rearrange("c (b t) -> c b t", t=16)[:, :, t:t + 1],
                    start=(t == 0), stop=(t == 15))
            sT = sq.tile([64, BPC], F32, tag="sT")
            nc.vector.tensor_scalar(sT[:], fps[:, :BPC], fc1b_sb[:, 0:1], None, OP.add)

            # ================= GAT =================
            def gat_body(w):
                T8 = T8w[w]
                G = T8 // 8
                gbase = acol[w] // 8
                ps = pp.tile([P, 512], F32, tag="ps")   # one PSUM bank/window
                ad_ps = ps[:, 0:4]
                as_ps = ps[:, 32:32 + 4 * T8MAX]
                zT_ps = ps[0:40, 192:320]
                g1_ps = ps[:, 320:448]
                h2_ps = ps[:, 448:512]
                nc.tensor.matmul(ad_ps, xloc_sb[:, bass.ds(w * P, P)],
                                 wad_sb[:], start=True, stop=True)
                T_d = gp.tile([P, 4], BF16, tag="Td")
                nc.scalar.activation(T_d[:], ad_ps, AF.Exp, scale=0.8)

                xg = gp.tile([128, GMAX * P], BF16, tag="xg")
                nc.sync.dma_start(xg[:, :G * P],
                                  xgrp[:, bass.ds(gbase * P, G * P)])
                for g in range(G):
                    nc.tensor.matmul(as_ps[:, 32 * g:32 * g + 32],
                                     xg[:, P * g:P * (g + 1)], wasD_sb[:],
                                     start=True, stop=True)
                Pt = gp.tile([P, 4 * T8MAX], BF16, tag="Pt")
                Rt = gp.tile([P, 4 * T8MAX], BF16, tag="Rt")
                nc.scalar.activation(Pt[:, :4 * T8], as_ps[:, :4 * T8],
                                     AF.Exp, scale=1.0)
                nc.scalar.activation(Rt[:, :4 * T8], as_ps[:, :4 * T8],
                                     AF.Exp, scale=0.2)

                EX = gp.tile([P, 4 * T8MAX], BF16, tag="EX")
                nc.vector.tensor_tensor(
                    EX[:, :4 * T8].rearrange("p (t h) -> p t h", h=4),
                    Pt[:, :4 * T8].rearrange("p (t h) -> p t h", h=4),
                    T_d[:, None, :].to_broadcast([P, T8, 4]),
                    op=OP.mult)
                nc.vector.tensor_tensor(EX[:, :4 * T8], EX[:, :4 * T8],
                                        Rt[:, :4 * T8], op=OP.max)
                S4 = gp.tile([P, 4], F32, tag="S4")
                nc.vector.tensor_reduce(
                    S4[:, :, None],
                    EX[:, :4 * T8].rearrange("p (t h) -> p h t", h=4),
                    axis=AX.X, op=OP.add)
                nc.vector.reciprocal(S4[:], S4[:])
                S4b = gp.tile([P, 4], BF16, tag="S4b")
                nc.vector.tensor_copy(S4b[:], S4[:])
                # AL in (h, t) layout -> ZR/zaug reduce become stride-1
                AL = gp.tile([P, 4 * T8MAX], BF16, tag="AL")
                nc.vector.tensor_tensor(
                    AL[:, :4 * T8].rearrange("p (h t) -> p t h", t=T8),
                    EX[:, :4 * T8].rearrange("p (t h) -> p t h", h=4),
                    S4b[:, None, :].to_broadcast([P, T8, 4]),
                    op=OP.mult)

                XS = gp.tile([P, 9 * T8MAX], BF16, tag="XS")
                nc.sync.dma_start(XS[:, :9 * T8],
                                  xslots[:, bass.ds(acol[w] * 9, T8 * 9)])
                ZR = gp.tile([P, 36 * T8MAX], BF16, tag="ZR")
                nc.vector.tensor_tensor(
                    ZR[:, :36 * T8].rearrange("p (h f t) -> p h f t", f=9, t=T8),
                    XS[:, :9 * T8].rearrange("p (f t) -> p f t", t=T8)[:, None, :, :]
                        .to_broadcast([P, 4, 9, T8]),
                    AL[:, :4 * T8].rearrange("p (h t) -> p h t", t=T8)[:, :, None, :]
                        .to_broadcast([P, 4, 9, T8]),
                    op=OP.mult)
                zaug = gp.tile([P, 40], F32, tag="zaug")
                nc.vector.tensor_copy(zaug[:, 36:40], ones4_sb[:])
                nc.vector.tensor_reduce(
                    zaug[:, 0:36][:, :, None],
                    ZR[:, :36 * T8].rearrange("p (q t) -> p q t", t=T8),
                    axis=AX.X, op=OP.add)
                nc.tensor.transpose(out=zT_ps, in_=zaug[:], identity=ident_sb[:])
                zT = gp.tile([40, P], F32, tag="zTs")
                nc.vector.tensor_copy(zT[:], zT_ps)
                nc.tensor.matmul(g1_ps, zT[:], wg40_sb[:],
                                 start=True, stop=True)
                g1T = gp.tile([P, P], F32, tag="g1T")
                vlrelu(g1T[:], g1_ps, "g1a", gp, P)
                nc.tensor.matmul(h2_ps, g1T[:], W2_sb[:], start=True, stop=True)
                T2s = gp.tile([P, 64], BF16, tag="T2s")
                nc.vector.tensor_scalar(T2s[:], h2_ps,
                                        dinv_sb[:, bass.ds(w, 1)], None, OP.mult)
                nc.sync.dma_start(T2_local[bass.ds(w * P, P), :], T2s[:])

            with nc.named_scope("gat"):
                for w in range(WPC):
                    gat_body(w)

            tc.strict_bb_all_engine_barrier()
            with nc.named_scope("ag1"):
                nc.gpsimd.collective_compute(
                    "AllGather", OP.bypass, replica_groups=RG,
                    ins=[T2_local.ap().opt()], outs=[T2_full.ap().opt()])
            tc.strict_bb_all_engine_barrier()

            # ================= GCN layers =================
            dinv_bw = cp.tile([P, WPC], BF16, tag="dinv_bw")
            nc.vector.tensor_copy(dinv_bw[:], dinv_sb[:])

            def gcn_group(gi, table, last):
                w0, nw, Te = EQGROUPS[gi]
                G2 = g2p.tile([P, ECAPMAX * 64], BF16, tag="G2")
                nc.gpsimd.indirect_dma_start(
                    out=G2[:, :nw * Te * 64], out_offset=None,
                    in_=table[:],
                    in_offset=bass.IndirectOffsetOnAxis(
                        ap=srcrow_sb[:, eoff[gi]:eoff[gi] + nw * Te], axis=0))

                def gv(t0, t1):
                    return G2[:, :nw * Te * 64].rearrange(
                        "p (w t c) -> p w t c", t=Te, c=64)[:, :, t0:t1, :]

                Tc = Te
                while Tc > 1:
                    if Tc & 1:
                        nc.vector.tensor_tensor(gv(0, 1), gv(0, 1),
                                                gv(Tc - 1, Tc), op=OP.add)
                        Tc -= 1
                    H = Tc // 2
                    nc.vector.tensor_tensor(gv(0, H), gv(0, H),
                                            gv(H, 2 * H), op=OP.add)
                    Tc = H
                gcol0 = G2[:, :nw * Te * 64].rearrange(
                    "p (w x) -> p w x", x=Te * 64)[:, :, 0:64]
                dvb = dinv_bw[:, w0:w0 + nw, None].to_broadcast([P, nw, 64])
                dvf = dinv_sb[:, w0:w0 + nw, None].to_broadcast([P, nw, 64])
                if not last:
                    zf = wp.tile([P, NWMAX * 64], F32, tag="zf")
                    zfv = zf[:, :nw * 64].rearrange("p (w c) -> p w c", c=64)
                    nc.vector.tensor_tensor(zfv, gcol0, dvb, op=OP.mult)
                    nc.vector.tensor_tensor(
                        zfv, zfv,
                        b2row_sb[:, None, :].to_broadcast([P, nw, 64]),
                        op=OP.add)
                    u = wp.tile([P, NWMAX * 64], F32, tag="u")
                    uv = u[:, :nw * 64].rearrange("p (w c) -> p w c", c=64)
                    nc.vector.tensor_tensor(uv, zfv, dvf, op=OP.mult)
                    v = wp.tile([P, NWMAX * 64], F32, tag="v")
                    nc.vector.tensor_scalar(v[:, :nw * 64], u[:, :nw * 64],
                                            0.01, None, OP.mult)
                    T3b = wp.tile([P, NWMAX * 64], BF16, tag="T3b")
                    nc.vector.tensor_tensor(T3b[:, :nw * 64], u[:, :nw * 64],
                                            v[:, :nw * 64], op=OP.max)
                    nc.sync.dma_start(
                        T3_local[bass.ds(w0 * P, nw * P), :]
                            .rearrange("(w p) c -> p w c", p=P),
                        T3b[:, :nw * 64].rearrange("p (w c) -> p w c", c=64))
                else:
                    z3b = wp.tile([P, NWMAX * 64], F32, tag="z3b")
                    nc.vector.tensor_tensor(
                        z3b[:, :nw * 64].rearrange("p (w c) -> p w c", c=64),
                        gcol0, dvb, op=OP.mult)
                    for wi in range(nw):
                        w = w0 + wi
                        z3s = wp.tile([P, 65], F32, tag="z3s")
                        nc.vector.tensor_copy(
                            z3s[:, 0:64], z3b[:, wi * 64:(wi + 1) * 64])
                        nc.vector.tensor_copy(z3s[:, 64:65], ones4_sb[:, 0:1])
                        ps2 = pp.tile([P, 512], F32, tag="ps")
                        z3T_ps = ps2[0:65, 0:128]
                        g3_ps = ps2[:, 128:256]
                        nc.tensor.transpose(out=z3T_ps, in_=z3s[:],
                                            identity=ident_sb[:])
                        z3T = wp.tile([65, P], F32, tag="z3Ts")
                        nc.vector.tensor_copy(z3T[:], z3T_ps)
                        nc.tensor.matmul(g3_ps, z3T[:], W3_sb[:],
                                         start=True, stop=True)
                        g3 = wp.tile([P, P], F32, tag="g3s")
                        nc.scalar.activation(g3[:], g3_ps, AF.Lrelu, alpha=0.01)
                        Mp = wp.tile([P, 256], F32, tag="Mp")
                        nc.vector.tensor_scalar(
                            Mp[:], iota_sb[:], bl_sb[:, bass.ds(w, 1)], None,
                            OP.is_equal)
                        nc.tensor.matmul(pool_psT[:], g3[:], Mp[:],
                                         start=False, stop=True)

            with nc.named_scope("gcn1"):
                for gi in range(len(EQGROUPS)):
                    gcn_group(gi, T2_full, last=False)

            tc.strict_bb_all_engine_barrier()
            with nc.named_scope("ag2"):
                nc.gpsimd.collective_compute(
                    "AllGather", OP.bypass, replica_groups=RG,
                    ins=[T3_local.ap().opt()], outs=[T3_full.ap().opt()])
            tc.strict_bb_all_engine_barrier()

            with nc.named_scope("gcn2"):
                # zero the AllReduce input (rows not covered by this core)
                zb = wp.tile([P, 128], BF16, tag="zb")
                nc.vector.memset(zb[:], 0.0)
                r0 = 0
                while r0 < BROWS:
                    r1 = min(r0 + P, BROWS)
                    nc.sync.dma_start(AR_in[r0:r1, :], zb[:r1 - r0, :])
                    r0 = r1
                for gi in range(len(EQGROUPS)):
                    gcn_group(gi, T3_full, last=True)

                crec = wp.tile([P, 2], F32, tag="crec")
                nc.vector.reciprocal(crec[:], cnt_sb[:])
                poolTs = wp.tile([P, 256], F32, tag="poolTs")
                nc.vector.tensor_copy(poolTs[:], pool_psT[:])
                for k in range(2):
                    tp2 = pp.tile([P, 512], F32, tag="ps")
                    nc.tensor.transpose(out=tp2[:, 0:128],
                                        in_=poolTs[:, k * 128:(k + 1) * 128],
                                        identity=ident_sb[:])
                    pooled = wp.tile([P, 128], BF16, tag="pooled")
                    nc.vector.tensor_scalar(pooled[:], tp2[:, 0:128],
                                            crec[:, k:k + 1], None, OP.mult)
                    nc.gpsimd.indirect_dma_start(
                        out=AR_in[:], out_offset=bass.IndirectOffsetOnAxis(
                            ap=scat_sb[:, k:k + 1], axis=0),
                        in_=pooled[:], in_offset=None)

            tc.strict_bb_all_engine_barrier()
            with nc.named_scope("ar"):
                nc.gpsimd.collective_compute(
                    "AllReduce", OP.add, replica_groups=RG,
                    ins=[AR_in.ap().opt()], outs=[AR_out.ap().opt()])
            tc.strict_bb_all_engine_barrier()

            if DBG:
                dt_ = sq.tile([P, 64], BF16, tag="dbg_t")
                for i in range(LOCAL // P):
                    nc.sync.dma_start(dt_[:], T2_local[i * P:(i + 1) * P, :])
                    nc.sync.dma_start(dbg_T2[i * P:(i + 1) * P, :], dt_[:])
                    nc.sync.dma_start(dt_[:], T3_local[i * P:(i + 1) * P, :])
                    nc.sync.dma_start(dbg_T3[i * P:(i + 1) * P, :], dt_[:])
                dt2 = sq.tile([P, 128], BF16, tag="dbg_t2")
                r0 = 0
                while r0 < BROWS:
                    r1 = min(r0 + P, BROWS)
                    nc.sync.dma_start(dt2[:r1 - r0, :], AR_out[r0:r1, :])
                    nc.sync.dma_start(dbg_AR[r0:r1, :], dt2[:r1 - r0, :])
                    r0 = r1

            # ================= fusion + classifier (BPC batches) ==========
            with nc.named_scope("fuse"):
                prow = sq.tile([P, 128], BF16, tag="prow")
                nc.gpsimd.indirect_dma_start(
                    out=prow[:], out_offset=None,
                    in_=AR_out[:],
                    in_offset=bass.IndirectOffsetOnAxis(
                        ap=rows_sb[:, 0:1], axis=0))
                prow32 = sq.tile([P, 128], F32, tag="prow32")
                nc.vector.tensor_copy(prow32[:], prow[:])
                fps_ = pp.tile([P, 512], F32, tag="ps")
                tp_ps = fps_[:, 0:128]
                ups = fps_[:, 128:256]
                vps = fps_[0:64, 256:384]
                ops_ = fps_[0:1, 384:512]
                nc.tensor.transpose(out=tp_ps, in_=prow32[:], identity=ident_sb[:])
                poolT = sq.tile([P, BPC], F32, tag="poolT")
                nc.vector.tensor_copy(poolT[:], tp_ps)

                nc.tensor.matmul(ups[:, :BPC], fusW0_sb[:], poolT[:],
                                 start=True, stop=False)
                nc.tensor.matmul(ups[:, :BPC], fusW1_sb[:], sT[:],
                                 start=False, stop=False)
                nc.tensor.matmul(ups[:, :BPC], fusb_sb[:], onesr_sb[:],
                                 start=False, stop=True)
                combT = sq.tile([P, BPC], F32, tag="combT")
                vlrelu(combT[:], ups[:, :BPC], "fu_a", sq, BPC)
                nc.tensor.matmul(vps[:, :BPC], cls1W_sb[:], combT[:],
                                 start=True, stop=False)
                nc.tensor.matmul(vps[:, :BPC], cls1b_sb[:], onesr_sb[:],
                                 start=False, stop=True)
                c1T = sq.tile([64, BPC], F32, tag="c1T")
                vlrelu(c1T[:], vps[:, :BPC], "fu_b", sq, BPC)
                nc.tensor.matmul(ops_[:, :BPC], cls3W_sb[:], c1T[:],
                                 start=True, stop=True)
                out_sb = sq.tile([1, BPC], F32, tag="out_sb")
                nc.vector.tensor_scalar(
                    out_sb[:], ops_[:, :BPC], cls3b_sb[0:1, 0:1], None, OP.add)
                nc.sync.dma_start(out[:], out_sb[:])

    nc.compile()
    return nc


# --------------------------------------------------------------------------
# entry point
# --------------------------------------------------------------------------

_CACHE = {}
LAST_RESULT = None


def kernel(**inputs):
    kh = hash((np.asarray(inputs['edge_index']).tobytes(),
               np.asarray(inputs['x']).tobytes()))
    if kh not in _CACHE:
        per_core, baked = host_prep(inputs)
        nc = build_nc(baked)
        _CACHE[kh] = (per_core, baked, nc)
    per_core, baked, nc = _CACHE[kh]

    wts = fold_weights(inputs)
    Bsz = baked['Bsz']
    BPC = Bsz // NC_CORES
    seq = np.asarray(inputs['seq_data'], np.float32)      # [B, 30, 20]
    seqT = np.ascontiguousarray(seq.transpose(1, 0, 2))   # [30, B, 20]
    shared = dict(
        iota256=np.ascontiguousarray(
            np.broadcast_to(np.arange(256, dtype=np.float32), (P, 256))),
        ident=np.eye(P, dtype=np.float32),
        ones4=np.ones((P, 4), np.float32),
        onesrow=np.ones((1, BPC), np.float32),
        **wts)
    in_maps = []
    for c in range(NC_CORES):
        m = dict(shared)
        m.update(per_core[c])
        m['xseq'] = np.ascontiguousarray(
            seqT[:, c * BPC:(c + 1) * BPC, :]).reshape(30, BPC * 20)
        in_maps.append(m)

    global LAST_RESULT
    res = run_bass_kernel_spmd(
        nc, in_maps, core_ids=list(range(NC_CORES)),
        trace=bool(os.environ.get('BASS_KERNEL_TRACE')))
    LAST_RESULT = res
    if os.environ.get('K_DEBUG'):
        np.savez('/tmp/kdbg.npz',
                 **{f"{k}_{c}": res.results[c][k] for c in range(NC_CORES)
                    for k in ("dbg_T2", "dbg_T3", "dbg_AR")})
    o = np.concatenate([res.results[c]["out"].reshape(-1)
                        for c in range(NC_CORES)]).reshape(Bsz, 1)
    return o.astype(np.float32)
